# revision 1
# baseline (speedup 1.0000x reference)
"""Multi-head attention (B=4, S=2048, E=768, H=12, Dh=64) on 8 TRN2 NeuronCores.

Sharding: batch x head-group tensor parallel. Core c handles batch b = c//2 and
head group g = c%2 (6 heads each). Each core computes its heads' Q/K/V
projections, full attention over the 2048-token sequence, and a partial
out-projection over its 384 concat-features. The host sums the two partials per
batch and adds the output bias.

Device layout notes:
 - Host pre-transposes activations to x^T [E, S] and casts to bf16, so the
   contraction dim (E) lands on SBUF partitions with contiguous DMA loads.
 - Q^T/K^T are produced feature-major [384, S] (head pairs per 128-partition
   tile); V is token-major, each head augmented with 64 ones columns so the
   PV matmul emits the softmax denominator replicated on psum partitions
   64-127 (normalization is then one DVE reciprocal + one multiply-cast).
 - Scores are computed transposed (S^T tiles [128 keys, S queries]) and
   exponentiated on ScalarE straight out of PSUM (no max-subtraction: logits
   are ~N(0, 0.3), so exp is numerically safe, matching softmax exactly).
 - PSUM (8 banks) holds proj (2) + double-buffered S^T (4) + ctx (2)
   concurrently; the PE stream is software-pipelined by hand (next score
   tile issued before the current context matmul) and projection /
   out-projection work is injected into the exp-bound attention windows.
"""

import math
import os
import sys
from contextlib import ExitStack

import numpy as np

for _p in ("/opt/trn_rl_repo", "/root/.axon_site/_ro/trn_rl_repo"):
    if os.path.isdir(_p) and _p not in sys.path:
        sys.path.append(_p)

# NTFF tracing hooks (antenv.axon_hooks) don't exist in this container;
# make sure an ambient BASS_TRACE can't route execution into that path.
os.environ["BASS_NEVER_TRACE"] = "1"

import ml_dtypes  # noqa: E402

import concourse.bass as bass  # noqa: E402
import concourse.tile as tile  # noqa: E402
from concourse import bacc, mybir  # noqa: E402
from concourse.bass_utils import run_bass_kernel_spmd  # noqa: E402

BF16 = mybir.dt.bfloat16
F32 = mybir.dt.float32
NP_BF16 = ml_dtypes.bfloat16

B, S, E, H, DH = 4, 2048, 768, 12, 64
N_CORES = 8
G = H // 2  # heads per core (6)


def build_nc(T=S, EMB=E, NH=G, dh=DH, OUT=E, trace_label=""):
    """Emit the per-core Bass/Tile program. All cores run this same program.

    T: sequence length; EMB: model dim; NH: heads on this core (even);
    dh: head dim (64); OUT: out-projection output width.
    """
    assert T % 128 == 0 and EMB % 128 == 0 and dh == 64 and NH % 2 == 0
    FEAT = NH * dh
    assert FEAT % 128 == 0
    EC = EMB // 128  # contraction chunks for projections
    TT = T // 128  # token tiles
    FT = FEAT // 128  # feature tiles (head pairs)
    SCH = min(512, T)  # matmul moving free-dim chunk
    NSCH = T // SCH
    T2 = max(128, T // 2)  # attention query-half width (2 PSUM banks)
    NSH = T // T2  # query halves per head
    SCH2 = min(512, T2)
    NSCH2 = T2 // SCH2
    _ock = OUT // 2 if 128 < OUT <= 1024 and OUT % 2 == 0 else 512
    OCHUNKS = [(o, min(_ock, OUT - o)) for o in range(0, OUT, _ock)]
    scale = 1.0 / math.sqrt(dh)

    nc = bacc.Bacc("TRN2", target_bir_lowering=False, debug=False, num_devices=N_CORES)

    # ---- DRAM I/O ----
    xqT_d = nc.dram_tensor("xqT", [EMB, T], BF16, kind="ExternalInput").ap()
    xkT_d = nc.dram_tensor("xkT", [EMB, T], BF16, kind="ExternalInput").ap()
    xvT_d = nc.dram_tensor("xvT", [EMB, T], BF16, kind="ExternalInput").ap()
    wq_d = nc.dram_tensor("wq", [EMB, FEAT], BF16, kind="ExternalInput").ap()
    wk_d = nc.dram_tensor("wk", [EMB, FEAT], BF16, kind="ExternalInput").ap()
    wv_d = nc.dram_tensor("wv", [EMB, FEAT], BF16, kind="ExternalInput").ap()
    wo_d = nc.dram_tensor("wo", [FEAT, OUT], BF16, kind="ExternalInput").ap()
    bq_d = nc.dram_tensor("bq", [1, FEAT], BF16, kind="ExternalInput").ap()
    bk_d = nc.dram_tensor("bk", [1, FEAT], BF16, kind="ExternalInput").ap()
    bv_d = nc.dram_tensor("bv", [1, FEAT], BF16, kind="ExternalInput").ap()
    out_d = nc.dram_tensor("out", [T, OUT], F32, kind="ExternalOutput").ap()

    with tile.TileContext(nc) as tc, ExitStack() as ctx:
        persist = ctx.enter_context(tc.tile_pool(name="persist", bufs=1))

        # ---- persistent SBUF tensors ----
        wq_sb = [persist.tile([128, FEAT], BF16, tag=f"wq{j}", name=f"wq{j}") for j in range(EC)]
        wk_sb = [persist.tile([128, FEAT], BF16, tag=f"wk{j}", name=f"wk{j}") for j in range(EC)]
        wv_sb = [persist.tile([128, FEAT], BF16, tag=f"wv{j}", name=f"wv{j}") for j in range(EC)]
        wo_sb = [persist.tile([128, OUT], BF16, tag=f"wo{j}", name=f"wo{j}") for j in range(FT)]
        bq_sb = persist.tile([1, FEAT], BF16, tag="bq", name="bq")
        bk_sb = persist.tile([1, FEAT], BF16, tag="bk", name="bk")
        bv_sb = persist.tile([1, FEAT], BF16, tag="bv", name="bv")
        ones_row = persist.tile([1, T], BF16, tag="ones_row", name="ones_row")
        xqT_sb = [persist.tile([128, T], BF16, tag=f"xq{j}", name=f"xq{j}") for j in range(EC)]
        xkT_sb = [persist.tile([128, T], BF16, tag=f"xk{j}", name=f"xk{j}") for j in range(EC)]
        xvT_sb = [persist.tile([128, T], BF16, tag=f"xv{j}", name=f"xv{j}") for j in range(EC)]
        qT_sb = [persist.tile([128, T], BF16, tag=f"qT{j}", name=f"qT{j}") for j in range(FT)]
        kT_sb = [persist.tile([128, T], BF16, tag=f"kT{j}", name=f"kT{j}") for j in range(FT)]
        # V token-major, each head augmented with 64 ones columns so the PV
        # matmul emits the softmax denominator replicated on partitions 64-127
        v_sb = [persist.tile([128, NH * (dh + 64)], BF16, tag=f"v{i}", name=f"v{i}") for i in range(TT)]
        cn_sb = [persist.tile([128, T], BF16, tag=f"cn{j}", name=f"cn{j}") for j in range(FT)]

        # ---- weight/bias/x loads (Q/K path first: it gates head 0) ----
        nc.sync.dma_start(bq_sb[:], bq_d[:])
        nc.sync.dma_start(bk_sb[:], bk_d[:])
        for j in range(EC):
            nc.sync.dma_start(wq_sb[j][:], wq_d[j * 128 : (j + 1) * 128, :])
            nc.sync.dma_start(xqT_sb[j][:], xqT_d[j * 128 : (j + 1) * 128, :])
            nc.sync.dma_start(wk_sb[j][:], wk_d[j * 128 : (j + 1) * 128, :])
            nc.sync.dma_start(xkT_sb[j][:], xkT_d[j * 128 : (j + 1) * 128, :])
        nc.sync.dma_start(bv_sb[:], bv_d[:])
        for j in range(EC):
            nc.sync.dma_start(wv_sb[j][:], wv_d[j * 128 : (j + 1) * 128, :])
            nc.sync.dma_start(xvT_sb[j][:], xvT_d[j * 128 : (j + 1) * 128, :])
        for j in range(FT):
            nc.sync.dma_start(wo_sb[j][:], wo_d[j * 128 : (j + 1) * 128, :])
        nc.vector.memset(ones_row[:], 1.0)
        # ones columns of augmented V (written once)
        for i in range(TT):
            vview = v_sb[i][:].rearrange("p (h x) -> p h x", x=dh + 64)
            nc.vector.memset(vview[:, :, dh:], 1.0)

        # ---- compute: projections + attention + out-projection ----
        # PSUM budget (8 banks): proj 2 (bufs=2 x 1 bank) + ST 4 (bufs=2 x 2)
        # + ctx 2 (bufs=1 x 2). Everything coexists, so Tile can overlap the
        # phases; PE instruction order is software-pipelined by hand.
        with (
            tc.tile_pool(name="ppsum", bufs=2, space="PSUM") as ppool,
            tc.tile_pool(name="stpsum", bufs=2, space="PSUM") as stpool,
            tc.tile_pool(name="ctpsum", bufs=1, space="PSUM") as ctpool,
            tc.tile_pool(name="ptpool", bufs=5) as ptpool,
            tc.tile_pool(name="normpool", bufs=3) as npool,
            tc.tile_pool(name="outsb", bufs=4) as osbpool,
        ):

            def proj_qk(j, part=None, nparts=1):
                groups = [
                    (t, n)
                    for t in range(2)
                    for n in range(NSCH)
                ]
                if part is not None:
                    groups = groups[
                        (len(groups) * part) // nparts : (len(groups) * (part + 1)) // nparts
                    ]
                qk = (
                    (wq_sb, bq_sb, xqT_sb, qT_sb),
                    (wk_sb, bk_sb, xkT_sb, kT_sb),
                )
                for t, n in groups:
                    w_sb, b_sb, x_sb, dst = qk[t]
                    if True:
                        ps = ppool.tile([128, SCH], F32, tag="proj", name="proj")
                        # bias first (K=1 rank-1 update): depends only on the
                        # tiny bias DMA, so the group starts before x arrives
                        nc.tensor.matmul(
                            ps[:],
                            b_sb[:, j * 128 : (j + 1) * 128],
                            ones_row[:, 0:SCH],
                            start=True,
                            stop=False,
                        )
                        for e in range(EC):
                            nc.tensor.matmul(
                                ps[:],
                                w_sb[e][:, j * 128 : (j + 1) * 128],
                                x_sb[e][:, n * SCH : (n + 1) * SCH],
                                start=False,
                                stop=(e == EC - 1),
                            )
                        nc.vector.tensor_copy(dst[j][:, n * SCH : (n + 1) * SCH], ps[:])

            def proj_v(tiles=None):
                for i in tiles if tiles is not None else range(TT):
                    ps = ppool.tile([128, FEAT], F32, tag="proj", name="proj")
                    nc.tensor.matmul(
                        ps[:], ones_row[:, 0:128], bv_sb[:], start=True, stop=False
                    )
                    for e in range(EC):
                        nc.tensor.matmul(
                            ps[:],
                            xvT_sb[e][:, i * 128 : (i + 1) * 128],
                            wv_sb[e][:],
                            start=False,
                            stop=(e == EC - 1),
                        )
                    dst = v_sb[i][:].rearrange("p (h x) -> p h x", x=dh + 64)[:, :, 0:dh]
                    srcv = ps[:].rearrange("p (h d) -> p h d", d=dh)
                    nc.vector.tensor_copy(dst, srcv)

            def st_tile(i, kT_h, qT_h, s0):
                st = stpool.tile([128, T2], F32, tag="st", name="st")
                for n in range(NSCH2):
                    nc.tensor.matmul(
                        st[:, n * SCH2 : (n + 1) * SCH2],
                        kT_h[:, i * 128 : (i + 1) * 128],
                        qT_h[:, s0 + n * SCH2 : s0 + (n + 1) * SCH2],
                        start=True,
                        stop=True,
                    )
                return st

            pending_sts = []

            def head_args(h, sh):
                ft, half = h // 2, (h % 2) * 64
                return (
                    kT_sb[ft][half : half + 64, :],
                    qT_sb[ft][half : half + 64, :],
                    sh * T2,
                )

            def head(h, sh, filler=None, nxt=None):
                # keeps 2 score tiles in flight and pre-issues the NEXT
                # head's first 2 before this head's last context matmul, so
                # the ScalarE exp stream never stalls at head boundaries
                ft, half = h // 2, (h % 2) * 64
                kT_h, qT_h, s0 = head_args(h, sh)
                ct = ctpool.tile([128, T2], F32, tag="ct", name="ct")
                sts = pending_sts[:]
                del pending_sts[:]
                while len(sts) < min(2, TT):
                    sts.append(st_tile(len(sts), kT_h, qT_h, s0))
                nissued = 0
                for i in range(TT):
                    st = sts.pop(0)
                    pt = ptpool.tile([128, T2], BF16, tag="pt", name="pt")
                    nc.scalar.activation(
                        pt[:], st[:], mybir.ActivationFunctionType.Exp, scale=scale
                    )
                    if i + 2 < TT:
                        sts.append(st_tile(i + 2, kT_h, qT_h, s0))
                    elif nxt is not None and nissued < min(2, TT):
                        pending_sts.append(st_tile(nissued, *head_args(*nxt)))
                        nissued += 1
                    if filler is not None:
                        filler(i)
                    for n in range(NSCH2):
                        nc.tensor.matmul(
                            ct[:, n * SCH2 : (n + 1) * SCH2],
                            v_sb[i][:, h * (dh + 64) : (h + 1) * (dh + 64)],
                            pt[:, n * SCH2 : (n + 1) * SCH2],
                            start=(i == 0),
                            stop=(i == TT - 1),
                        )

                # normalize: cn[f, s] = ct[f, s] * (1 / ct[64.., s])
                recip = npool.tile([64, T2], F32, tag="recip", name="recip")
                nc.vector.reciprocal(recip[:], ct[64:128, :])
                nc.vector.tensor_tensor(
                    cn_sb[ft][half : half + 64, s0 : s0 + T2],
                    ct[0:64, :],
                    recip[:],
                    op=mybir.AluOpType.mult,
                )

            def outproj(tiles):
                for i in tiles:
                    osb = osbpool.tile([128, OUT], F32, tag="osb", name="osb")
                    for oc, ow in OCHUNKS:
                        ps = ppool.tile([128, ow], F32, tag="proj", name="proj")
                        for f in range(FT):
                            nc.tensor.matmul(
                                ps[:],
                                cn_sb[f][:, i * 128 : (i + 1) * 128],
                                wo_sb[f][:, oc : oc + ow],
                                start=(f == 0),
                                stop=(f == FT - 1),
                            )
                        nc.vector.tensor_copy(osb[:, oc : oc + ow], ps[:])
                    nc.sync.dma_start(out_d[i * 128 : (i + 1) * 128, :], osb[:])

            proj_qk(0)
            # pre-issue head 0's first score tiles BEFORE any V work: V
            # depends on the last-arriving xvT DMAs and must not gate exp_0
            for z in range(min(2, TT)):
                pending_sts.append(st_tile(z, *head_args(0, 0)))
            # V tile i is first needed at head 0's CT step i: emit tile 0/1
            # up front and drip the rest into head 0's pipeline
            proj_v(range(2))

            def v_filler(i):
                if i + 2 < TT:
                    proj_v([i + 2])

            half_tiles = T2 // 128 if NSH == 2 else 0
            seq = [
                (2 * p + z, sh)
                for p in range(NH // 2)
                for sh in range(NSH)
                for z in (0, 1)
            ]
            pos = 0
            for p in range(NH // 2):
                last = 2 * p + 1 == NH - 1
                for sh in range(NSH):
                    nxt = seq[pos + 1] if pos + 1 < len(seq) else None
                    head(2 * p, sh, v_filler if (p, sh) == (0, 0) else None, nxt=nxt)
                    pos += 1
                    # spread the next feature-tile's projections over this
                    # pair's ACT-bound windows (3 injection points)
                    if p + 1 < FT and NSH == 2:
                        proj_qk(p + 1, part=2 * sh, nparts=3)
                    if last and sh == 1 and NSH == 2:
                        outproj(range(half_tiles // 2, half_tiles))
                    nxt = seq[pos + 1] if pos + 1 < len(seq) else None
                    head(2 * p + 1, sh, nxt=nxt)
                    pos += 1
                    if p + 1 < FT and NSH == 2 and sh == 0:
                        proj_qk(p + 1, part=1, nparts=3)
                    if p + 1 < FT and NSH == 1:
                        proj_qk(p + 1)
                    if last and sh == 0 and NSH == 2:
                        # heads done for queries [0, T2): drip their out-proj
                        # tiles into the remaining windows
                        outproj(range(half_tiles // 2))
            outproj(range(half_tiles, TT))

    nc.compile()
    return nc


def shard_inputs(query, key, value, wq, bq, wk, bk, wv, bv, wo):
    """Build the 8 per-core input maps (host-side cast/transpose/slice)."""
    in_maps = []
    xT = {}
    for b in range(B):
        xT[b] = (
            np.ascontiguousarray(query[b].T).astype(NP_BF16),
            np.ascontiguousarray(key[b].T).astype(NP_BF16),
            np.ascontiguousarray(value[b].T).astype(NP_BF16),
        )
    gw = {}
    for g in range(2):
        hs = slice(g * G, (g + 1) * G)
        gw[g] = dict(
            wq=np.ascontiguousarray(wq[hs].transpose(1, 0, 2).reshape(E, G * DH)).astype(NP_BF16),
            wk=np.ascontiguousarray(wk[hs].transpose(1, 0, 2).reshape(E, G * DH)).astype(NP_BF16),
            wv=np.ascontiguousarray(wv[hs].transpose(1, 0, 2).reshape(E, G * DH)).astype(NP_BF16),
            wo=np.ascontiguousarray(wo[g * G * DH : (g + 1) * G * DH, :]).astype(NP_BF16),
            bq=np.ascontiguousarray(bq[hs].reshape(1, G * DH)).astype(NP_BF16),
            bk=np.ascontiguousarray(bk[hs].reshape(1, G * DH)).astype(NP_BF16),
            bv=np.ascontiguousarray(bv[hs].reshape(1, G * DH)).astype(NP_BF16),
        )
    for c in range(N_CORES):
        b, g = c // 2, c % 2
        m = dict(xqT=xT[b][0], xkT=xT[b][1], xvT=xT[b][2])
        m.update(gw[g])
        in_maps.append(m)
    return in_maps


_CACHED_NC = None


def kernel(query, key, value, wq, bq, wk, bk, wv, bv, wo, bo):
    global _CACHED_NC
    query, key, value = (np.asarray(a, np.float32) for a in (query, key, value))
    wq, bq, wk, bk, wv, bv, wo, bo = (
        np.asarray(a, np.float32) for a in (wq, bq, wk, bk, wv, bv, wo, bo)
    )
    in_maps = shard_inputs(query, key, value, wq, bq, wk, bk, wv, bv, wo)
    if _CACHED_NC is None:
        _CACHED_NC = build_nc()
    res = run_bass_kernel_spmd(_CACHED_NC, in_maps, list(range(N_CORES)))
    out = np.empty((B, S, E), np.float32)
    for b in range(B):
        out[b] = res.results[2 * b]["out"] + res.results[2 * b + 1]["out"] + bo[None, :]
    return out



# revision 3
# speedup vs baseline: 1.1632x; 1.1632x over previous
"""Multi-head attention (B=4, S=2048, E=768, H=12, Dh=64) on 8 TRN2 NeuronCores.

Sharding: batch x head-group tensor parallel. Core c handles batch b = c//2 and
head group g = c%2 (6 heads each). Each core computes its heads' Q/K/V
projections, full attention over the 2048-token sequence, and a partial
out-projection over its 384 concat-features. The host sums the two partials per
batch and adds the output bias.

Device layout notes:
 - All projection inputs (x^T, w) are hosted in fp8e4 with the contraction dim
   folded [64, 2, .] so every projection matmul runs in DoubleRow perf mode
   (2 contraction rows/cycle). Weights are host-scaled x16 to clear fp8e4's
   subnormal range; the 1/16 is folded into the PSUM->SBUF copy.
 - Q^T/K^T are produced feature-major [128, 2T] fp8e4 with a zeroed upper half:
   score matmuls run DoubleRow with k-tile 0 = the real 64-row dh contraction
   and k-tile 1 = zeros, so scores also stream 2 output cols/cycle.
 - V is token-major bf16, each head augmented with 64 ones columns so the PV
   matmul emits the softmax denominator replicated on psum partitions 64-127.
 - Scores are computed transposed (S^T tiles [128 keys, S queries]); softmax
   exp runs split across TWO engines: most tiles on ScalarE (table exp straight
   out of PSUM), a tunable fraction on VectorE via a Schraudolph bitcast
   approximation (i16 = round(x*128/ln2 + 16249); bitcast to bf16 ~= e^x to
   +-4%, which washes out under the ~2048-token softmax average).
 - PSUM (8 banks) holds proj (2) + double-buffered S^T (4) + ctx (2)
   concurrently; the PE stream is software-pipelined by hand (next score
   tile issued before the current context matmul) and projection /
   out-projection work is injected into the exp-bound attention windows.
"""

import math
import os
import sys
from contextlib import ExitStack

import numpy as np

for _p in ("/opt/trn_rl_repo", "/root/.axon_site/_ro/trn_rl_repo"):
    if os.path.isdir(_p) and _p not in sys.path:
        sys.path.append(_p)

# NTFF tracing hooks (antenv.axon_hooks) don't exist in this container;
# make sure an ambient BASS_TRACE can't route execution into that path.
os.environ["BASS_NEVER_TRACE"] = "1"

import ml_dtypes  # noqa: E402

import concourse.bass as bass  # noqa: E402
import concourse.tile as tile  # noqa: E402
from concourse import bacc, mybir  # noqa: E402
from concourse.bass_utils import run_bass_kernel_spmd  # noqa: E402

BF16 = mybir.dt.bfloat16
F32 = mybir.dt.float32
FP8 = mybir.dt.float8e4
I16 = mybir.dt.int16
NP_BF16 = ml_dtypes.bfloat16
NP_FP8 = ml_dtypes.float8_e4m3

B, S, E, H, DH = 4, 2048, 768, 12, 64
N_CORES = 8
G = H // 2  # heads per core (6)

W_SCALE = 16.0  # host premultiplier on wq/wk/wv/bq/bk/bv (fp8 subnormal dodge)

# Schraudolph exp-approx constants (bf16 bitcast): i16 = st*SCH_A1 + SCH_B
SCH_A = 128.0 / math.log(2.0)
SCH_B = 16256.0 - 7.4 + 0.5
# fraction of exp tiles routed to VectorE instead of ScalarE
DVE_EXP_FRAC = float(os.environ.get("DVE_EXP_FRAC", "0.22"))

DR = mybir.MatmulPerfMode.DoubleRow


def build_nc(T=S, EMB=E, NH=G, dh=DH, OUT=E, trace_label=""):
    """Emit the per-core Bass/Tile program. All cores run this same program."""
    assert T % 128 == 0 and EMB % 128 == 0 and dh == 64 and NH % 2 == 0
    FEAT = NH * dh
    assert FEAT % 128 == 0
    EC = EMB // 128  # 128-row contraction chunks for projections
    EP = EC // 2  # fp8-folded [64,2,...] chunk-pairs per 256 emb rows
    TT = T // 128  # token tiles
    FT = FEAT // 128  # feature tiles (head pairs)
    SCH = min(512, T)  # matmul moving free-dim chunk
    NSCH = T // SCH
    T2 = max(128, T // 2)  # attention query-half width (2 PSUM banks)
    NSH = T // T2  # query halves per head
    SCH2 = min(512, T2)
    NSCH2 = T2 // SCH2
    _ock = OUT // 2 if 128 < OUT <= 1024 and OUT % 2 == 0 else 512
    OCHUNKS = [(o, min(_ock, OUT - o)) for o in range(0, OUT, _ock)]
    scale = 1.0 / math.sqrt(dh)
    inv_w = 1.0 / W_SCALE

    nc = bacc.Bacc("TRN2", target_bir_lowering=False, debug=False, num_devices=N_CORES)

    # ---- DRAM I/O ----
    # x^T and projection weights fp8, contraction-folded: tile j holds emb rows
    # [256j, 256j+256) as [c*64+p, t*T + s] with e = 256j + 128c + 64t + p.
    xqT_d = nc.dram_tensor("xqT", [128, EP * 2 * T], FP8, kind="ExternalInput").ap()
    xkT_d = nc.dram_tensor("xkT", [128, EP * 2 * T], FP8, kind="ExternalInput").ap()
    xvT_d = nc.dram_tensor("xvT", [128, EP * 2 * T], FP8, kind="ExternalInput").ap()
    wq_d = nc.dram_tensor("wq", [128, EP * 2 * FEAT], FP8, kind="ExternalInput").ap()
    wk_d = nc.dram_tensor("wk", [128, EP * 2 * FEAT], FP8, kind="ExternalInput").ap()
    wv_d = nc.dram_tensor("wv", [128, EP * 2 * FEAT], FP8, kind="ExternalInput").ap()
    wo_d = nc.dram_tensor("wo", [FEAT, OUT], BF16, kind="ExternalInput").ap()
    bq_d = nc.dram_tensor("bq", [1, FEAT], BF16, kind="ExternalInput").ap()
    bk_d = nc.dram_tensor("bk", [1, FEAT], BF16, kind="ExternalInput").ap()
    bv_d = nc.dram_tensor("bv", [1, FEAT], BF16, kind="ExternalInput").ap()
    out_d = nc.dram_tensor("out", [T, OUT], F32, kind="ExternalOutput").ap()

    with tile.TileContext(nc) as tc, ExitStack() as ctx:
        persist = ctx.enter_context(tc.tile_pool(name="persist", bufs=1))

        # ---- persistent SBUF tensors ----
        wq_sb = [persist.tile([128, 2 * FEAT], FP8, tag=f"wq{j}", name=f"wq{j}") for j in range(EP)]
        wk_sb = [persist.tile([128, 2 * FEAT], FP8, tag=f"wk{j}", name=f"wk{j}") for j in range(EP)]
        wv_sb = [persist.tile([128, 2 * FEAT], FP8, tag=f"wv{j}", name=f"wv{j}") for j in range(EP)]
        wo_sb = [persist.tile([128, OUT], BF16, tag=f"wo{j}", name=f"wo{j}") for j in range(FT)]
        bq_sb = persist.tile([1, FEAT], BF16, tag="bq", name="bq")
        bk_sb = persist.tile([1, FEAT], BF16, tag="bk", name="bk")
        bv_sb = persist.tile([1, FEAT], BF16, tag="bv", name="bv")
        ones_row = persist.tile([1, T], BF16, tag="ones_row", name="ones_row")
        xqT_sb = [persist.tile([128, 2 * T], FP8, tag=f"xq{j}", name=f"xq{j}") for j in range(EP)]
        xkT_sb = [persist.tile([128, 2 * T], FP8, tag=f"xk{j}", name=f"xk{j}") for j in range(EP)]
        xvT_sb = [persist.tile([128, 2 * T], FP8, tag=f"xv{j}", name=f"xv{j}") for j in range(EP)]
        # q^T/k^T fp8, upper T columns zero (DoubleRow zero k-tile)
        qT_sb = [persist.tile([128, 2 * T], FP8, tag=f"qT{j}", name=f"qT{j}") for j in range(FT)]
        kT_sb = [persist.tile([128, 2 * T], FP8, tag=f"kT{j}", name=f"kT{j}") for j in range(FT)]
        # V token-major, each head augmented with 64 ones columns so the PV
        # matmul emits the softmax denominator replicated on partitions 64-127
        v_sb = [persist.tile([128, NH * (dh + 64)], BF16, tag=f"v{i}", name=f"v{i}") for i in range(TT)]
        cn_sb = [persist.tile([128, T], BF16, tag=f"cn{j}", name=f"cn{j}") for j in range(FT)]

        # ---- weight/bias/x loads (Q/K path first: it gates head 0) ----
        nc.sync.dma_start(bq_sb[:], bq_d[:])
        nc.sync.dma_start(bk_sb[:], bk_d[:])
        for j in range(EP):
            nc.sync.dma_start(wq_sb[j][:], wq_d[:, j * 2 * FEAT : (j + 1) * 2 * FEAT])
            nc.sync.dma_start(xqT_sb[j][:], xqT_d[:, j * 2 * T : (j + 1) * 2 * T])
            nc.sync.dma_start(wk_sb[j][:], wk_d[:, j * 2 * FEAT : (j + 1) * 2 * FEAT])
            nc.sync.dma_start(xkT_sb[j][:], xkT_d[:, j * 2 * T : (j + 1) * 2 * T])
        nc.sync.dma_start(bv_sb[:], bv_d[:])
        for j in range(EP):
            nc.sync.dma_start(wv_sb[j][:], wv_d[:, j * 2 * FEAT : (j + 1) * 2 * FEAT])
            nc.sync.dma_start(xvT_sb[j][:], xvT_d[:, j * 2 * T : (j + 1) * 2 * T])
        for j in range(FT):
            nc.sync.dma_start(wo_sb[j][:], wo_d[j * 128 : (j + 1) * 128, :])
        nc.vector.memset(ones_row[:], 1.0)
        # zero halves of q^T/k^T (DoubleRow zero k-tile; never rewritten) on
        # the otherwise-idle Pool engine
        for j in range(FT):
            nc.gpsimd.memset(qT_sb[j][:, T : 2 * T], 0.0)
            nc.gpsimd.memset(kT_sb[j][:, T : 2 * T], 0.0)
        # ones columns of augmented V (written once)
        for i in range(TT):
            vview = v_sb[i][:].rearrange("p (h x) -> p h x", x=dh + 64)
            nc.gpsimd.memset(vview[:, :, dh:], 1.0)

        def dr(ap2w):
            """[p, (2,W)] fp8-folded view of a [128, 2W] tile slice."""
            return ap2w.rearrange("p (t w) -> p t w", t=2)

        # ---- compute: projections + attention + out-projection ----
        # PSUM budget (8 banks): proj 2 (bufs=2 x 1 bank) + ST 4 (bufs=2 x 2)
        # + ctx 2 (bufs=1 x 2). Everything coexists, so Tile can overlap the
        # phases; PE instruction order is software-pipelined by hand.
        with (
            tc.tile_pool(name="ppsum", bufs=2, space="PSUM") as ppool,
            tc.tile_pool(name="stpsum", bufs=2, space="PSUM") as stpool,
            tc.tile_pool(name="ctpsum", bufs=1, space="PSUM") as ctpool,
            tc.tile_pool(name="ptpool", bufs=5) as ptpool,
            tc.tile_pool(name="normpool", bufs=3) as npool,
            tc.tile_pool(name="outsb", bufs=4) as osbpool,
        ):

            def proj_qk(j, part=None, nparts=1):
                groups = [(t, n) for t in range(2) for n in range(NSCH)]
                if part is not None:
                    groups = groups[
                        (len(groups) * part) // nparts : (len(groups) * (part + 1)) // nparts
                    ]
                qk = (
                    (wq_sb, bq_sb, xqT_sb, qT_sb),
                    (wk_sb, bk_sb, xkT_sb, kT_sb),
                )
                for t, n in groups:
                    w_sb, b_sb, x_sb, dst = qk[t]
                    ps = ppool.tile([128, SCH], F32, tag="proj", name="proj")
                    # bias first (K=1 rank-1 update): depends only on the
                    # tiny bias DMA, so the group starts before x arrives
                    nc.tensor.matmul(
                        ps[:],
                        b_sb[:, j * 128 : (j + 1) * 128],
                        ones_row[:, 0:SCH],
                        start=True,
                        stop=False,
                    )
                    for e in range(EP):
                        nc.tensor.matmul(
                            ps[:],
                            dr(w_sb[e][:])[:, :, j * 128 : (j + 1) * 128],
                            dr(x_sb[e][:])[:, :, n * SCH : (n + 1) * SCH],
                            start=False,
                            stop=(e == EP - 1),
                            perf_mode=DR,
                        )
                    # fold the x16 weight prescale out while casting to fp8
                    nc.vector.tensor_scalar(
                        dst[j][:, n * SCH : (n + 1) * SCH],
                        ps[:],
                        inv_w,
                        None,
                        mybir.AluOpType.mult,
                    )

            def proj_v(tiles=None):
                for i in tiles if tiles is not None else range(TT):
                    ps = ppool.tile([128, FEAT], F32, tag="proj", name="proj")
                    nc.tensor.matmul(
                        ps[:], ones_row[:, 0:128], bv_sb[:], start=True, stop=False
                    )
                    for e in range(EP):
                        nc.tensor.matmul(
                            ps[:],
                            dr(xvT_sb[e][:])[:, :, i * 128 : (i + 1) * 128],
                            dr(wv_sb[e][:]),
                            start=False,
                            stop=(e == EP - 1),
                            perf_mode=DR,
                        )
                    dst = v_sb[i][:].rearrange("p (h x) -> p h x", x=dh + 64)[:, :, 0:dh]
                    srcv = ps[:].rearrange("p (h d) -> p h d", d=dh)
                    nc.vector.tensor_scalar(dst, srcv, inv_w, None, mybir.AluOpType.mult)

            def st_tile(i, kT_h, qT_h, s0):
                st = stpool.tile([128, T2], F32, tag="st", name="st")
                for n in range(NSCH2):
                    nc.tensor.matmul(
                        st[:, n * SCH2 : (n + 1) * SCH2],
                        dr(kT_h)[:, :, i * 128 : (i + 1) * 128],
                        dr(qT_h)[:, :, s0 + n * SCH2 : s0 + (n + 1) * SCH2],
                        start=True,
                        stop=True,
                        perf_mode=DR,
                    )
                return st

            pending_sts = []
            exp_counter = [0, 0]  # [tiles seen, tiles sent to DVE]

            def exp_tile(st):
                """Exp one score tile; route a DVE_EXP_FRAC slice to VectorE."""
                exp_counter[0] += 1
                want = int(exp_counter[0] * DVE_EXP_FRAC)
                if want > exp_counter[1]:
                    exp_counter[1] += 1
                    pt = ptpool.tile([128, T2], I16, tag="pt", name="pt")
                    nc.vector.tensor_scalar(
                        pt[:],
                        st[:],
                        SCH_A * scale,
                        SCH_B,
                        mybir.AluOpType.mult,
                        mybir.AluOpType.add,
                    )
                    return pt[:].bitcast(BF16)
                pt = ptpool.tile([128, T2], BF16, tag="pt", name="pt")
                nc.scalar.activation(
                    pt[:], st[:], mybir.ActivationFunctionType.Exp, scale=scale
                )
                return pt[:]

            def head_args(h, sh):
                ft, half = h // 2, (h % 2) * 64
                return (
                    kT_sb[ft][half : half + 64, :],
                    qT_sb[ft][half : half + 64, :],
                    sh * T2,
                )

            def head(h, sh, filler=None, nxt=None):
                # keeps 2 score tiles in flight and pre-issues the NEXT
                # head's first 2 before this head's last context matmul, so
                # the exp stream never stalls at head boundaries
                ft, half = h // 2, (h % 2) * 64
                kT_h, qT_h, s0 = head_args(h, sh)
                ct = ctpool.tile([128, T2], F32, tag="ct", name="ct")
                sts = pending_sts[:]
                del pending_sts[:]
                while len(sts) < min(2, TT):
                    sts.append(st_tile(len(sts), kT_h, qT_h, s0))
                nissued = 0
                for i in range(TT):
                    st = sts.pop(0)
                    pt = exp_tile(st)
                    if i + 2 < TT:
                        sts.append(st_tile(i + 2, kT_h, qT_h, s0))
                    elif nxt is not None and nissued < min(2, TT):
                        pending_sts.append(st_tile(nissued, *head_args(*nxt)))
                        nissued += 1
                    if filler is not None:
                        filler(i)
                    for n in range(NSCH2):
                        nc.tensor.matmul(
                            ct[:, n * SCH2 : (n + 1) * SCH2],
                            v_sb[i][:, h * (dh + 64) : (h + 1) * (dh + 64)],
                            pt[:, n * SCH2 : (n + 1) * SCH2],
                            start=(i == 0),
                            stop=(i == TT - 1),
                        )

                # normalize: cn[f, s] = ct[f, s] * (1 / ct[64.., s])
                recip = npool.tile([64, T2], F32, tag="recip", name="recip")
                nc.vector.reciprocal(recip[:], ct[64:128, :])
                nc.vector.tensor_tensor(
                    cn_sb[ft][half : half + 64, s0 : s0 + T2],
                    ct[0:64, :],
                    recip[:],
                    op=mybir.AluOpType.mult,
                )

            def outproj(tiles):
                for i in tiles:
                    osb = osbpool.tile([128, OUT], F32, tag="osb", name="osb")
                    for oc, ow in OCHUNKS:
                        ps = ppool.tile([128, ow], F32, tag="proj", name="proj")
                        for f in range(FT):
                            nc.tensor.matmul(
                                ps[:],
                                cn_sb[f][:, i * 128 : (i + 1) * 128],
                                wo_sb[f][:, oc : oc + ow],
                                start=(f == 0),
                                stop=(f == FT - 1),
                            )
                        nc.vector.tensor_copy(osb[:, oc : oc + ow], ps[:])
                    nc.sync.dma_start(out_d[i * 128 : (i + 1) * 128, :], osb[:])

            proj_qk(0)
            # pre-issue head 0's first score tiles BEFORE any V work: V
            # depends on the last-arriving xvT DMAs and must not gate exp_0
            for z in range(min(2, TT)):
                pending_sts.append(st_tile(z, *head_args(0, 0)))
            # V tile i is first needed at head 0's CT step i: emit tile 0/1
            # up front and drip the rest into head 0's pipeline
            proj_v(range(2))

            def v_filler(i):
                if i + 2 < TT:
                    proj_v([i + 2])

            half_tiles = T2 // 128 if NSH == 2 else 0
            seq = [
                (2 * p + z, sh)
                for p in range(NH // 2)
                for sh in range(NSH)
                for z in (0, 1)
            ]
            pos = 0
            for p in range(NH // 2):
                last = 2 * p + 1 == NH - 1
                for sh in range(NSH):
                    nxt = seq[pos + 1] if pos + 1 < len(seq) else None
                    head(2 * p, sh, v_filler if (p, sh) == (0, 0) else None, nxt=nxt)
                    pos += 1
                    # spread the next feature-tile's projections over this
                    # pair's ACT-bound windows (3 injection points)
                    if p + 1 < FT and NSH == 2:
                        proj_qk(p + 1, part=2 * sh, nparts=3)
                    if last and sh == 1 and NSH == 2:
                        outproj(range(half_tiles // 2, half_tiles))
                    nxt = seq[pos + 1] if pos + 1 < len(seq) else None
                    head(2 * p + 1, sh, nxt=nxt)
                    pos += 1
                    if p + 1 < FT and NSH == 2 and sh == 0:
                        proj_qk(p + 1, part=1, nparts=3)
                    if p + 1 < FT and NSH == 1:
                        proj_qk(p + 1)
                    if last and sh == 0 and NSH == 2:
                        # heads done for queries [0, T2): drip their out-proj
                        # tiles into the remaining windows
                        outproj(range(half_tiles // 2))
            outproj(range(half_tiles, TT))

    nc.compile()
    return nc


def _fold_fp8(arr2d, ncols):
    """[768, ncols] f32 -> [3, 128, 2*ncols] fp8 contraction-folded."""
    a = arr2d.reshape(3, 2, 2, 64, ncols).transpose(0, 1, 3, 2, 4)
    a = a.reshape(3, 128, 2 * ncols).transpose(1, 0, 2).reshape(128, 3 * 2 * ncols)
    return np.ascontiguousarray(a).astype(NP_FP8)


def shard_inputs(query, key, value, wq, bq, wk, bk, wv, bv, wo):
    """Build the 8 per-core input maps (host-side cast/fold/slice)."""
    in_maps = []
    xT = {}
    for b in range(B):
        xT[b] = (
            _fold_fp8(np.ascontiguousarray(query[b].T), S),
            _fold_fp8(np.ascontiguousarray(key[b].T), S),
            _fold_fp8(np.ascontiguousarray(value[b].T), S),
        )
    gw = {}
    for g in range(2):
        hs = slice(g * G, (g + 1) * G)
        gw[g] = dict(
            wq=_fold_fp8(W_SCALE * wq[hs].transpose(1, 0, 2).reshape(E, G * DH), G * DH),
            wk=_fold_fp8(W_SCALE * wk[hs].transpose(1, 0, 2).reshape(E, G * DH), G * DH),
            wv=_fold_fp8(W_SCALE * wv[hs].transpose(1, 0, 2).reshape(E, G * DH), G * DH),
            wo=np.ascontiguousarray(wo[g * G * DH : (g + 1) * G * DH, :]).astype(NP_BF16),
            bq=np.ascontiguousarray(W_SCALE * bq[hs].reshape(1, G * DH)).astype(NP_BF16),
            bk=np.ascontiguousarray(W_SCALE * bk[hs].reshape(1, G * DH)).astype(NP_BF16),
            bv=np.ascontiguousarray(W_SCALE * bv[hs].reshape(1, G * DH)).astype(NP_BF16),
        )
    for c in range(N_CORES):
        b, g = c // 2, c % 2
        m = dict(xqT=xT[b][0], xkT=xT[b][1], xvT=xT[b][2])
        m.update(gw[g])
        in_maps.append(m)
    return in_maps


_CACHED_NC = None


def kernel(query, key, value, wq, bq, wk, bk, wv, bv, wo, bo):
    global _CACHED_NC
    query, key, value = (np.asarray(a, np.float32) for a in (query, key, value))
    wq, bq, wk, bk, wv, bv, wo, bo = (
        np.asarray(a, np.float32) for a in (wq, bq, wk, bk, wv, bv, wo, bo)
    )
    in_maps = shard_inputs(query, key, value, wq, bq, wk, bk, wv, bv, wo)
    if _CACHED_NC is None:
        _CACHED_NC = build_nc()
    res = run_bass_kernel_spmd(_CACHED_NC, in_maps, list(range(N_CORES)))
    out = np.empty((B, S, E), np.float32)
    for b in range(B):
        out[b] = res.results[2 * b]["out"] + res.results[2 * b + 1]["out"] + bo[None, :]
    return out


# revision 12
# speedup vs baseline: 1.1991x; 1.0309x over previous
"""Multi-head attention (B=4, S=2048, E=768, H=12, Dh=64) on 8 TRN2 NeuronCores.

Sharding: batch x head-group tensor parallel. Core c handles batch b = c//2 and
head group g = c%2 (6 heads each). Each core computes its heads' Q/K/V
projections, full attention over the 2048-token sequence, and a partial
out-projection over its 384 concat-features. The host sums the two partials per
batch and adds the output bias.

Device layout notes:
 - All projection inputs (x^T, w) are hosted in fp8e4 with the contraction dim
   folded [64, 2, .] so every projection matmul runs in DoubleRow perf mode
   (2 contraction rows/cycle). Weights are host-scaled x16 to clear fp8e4's
   subnormal range; the 1/16 is folded into the PSUM->SBUF copy.
 - Q^T/K^T are produced feature-major [128, 2T] fp8e4 with a zeroed upper half:
   score matmuls run DoubleRow with k-tile 0 = the real 64-row dh contraction
   and k-tile 1 = zeros, so scores also stream 2 output cols/cycle.
 - V is token-major bf16, each head augmented with 64 ones columns so the PV
   matmul emits the softmax denominator replicated on psum partitions 64-127.
 - Scores are computed transposed (S^T tiles [128 keys, S queries]); softmax
   exp runs split across TWO engines: most tiles on ScalarE (table exp straight
   out of PSUM), a tunable fraction on VectorE via a Schraudolph bitcast
   approximation (i16 = round(x*128/ln2 + 16249); bitcast to bf16 ~= e^x to
   +-4%, which washes out under the ~2048-token softmax average).
 - PSUM (8 banks) holds proj (2) + double-buffered S^T (4) + ctx (2)
   concurrently; the PE stream is software-pipelined by hand (next score
   tile issued before the current context matmul) and projection /
   out-projection work is injected into the exp-bound attention windows.
"""

import math
import os
import sys
from contextlib import ExitStack

import numpy as np

for _p in ("/opt/trn_rl_repo", "/root/.axon_site/_ro/trn_rl_repo"):
    if os.path.isdir(_p) and _p not in sys.path:
        sys.path.append(_p)

# NTFF tracing hooks (antenv.axon_hooks) don't exist in this container;
# make sure an ambient BASS_TRACE can't route execution into that path.
os.environ["BASS_NEVER_TRACE"] = "1"

import ml_dtypes  # noqa: E402

import concourse.bass as bass  # noqa: E402
import concourse.tile as tile  # noqa: E402
from concourse import bacc, mybir  # noqa: E402
from concourse.bass_utils import run_bass_kernel_spmd  # noqa: E402

BF16 = mybir.dt.bfloat16
F32 = mybir.dt.float32
FP8 = mybir.dt.float8e4
I16 = mybir.dt.int16
NP_BF16 = ml_dtypes.bfloat16
NP_FP8 = ml_dtypes.float8_e4m3

B, S, E, H, DH = 4, 2048, 768, 12, 64
N_CORES = 8
G = H // 2  # heads per core (6)

W_SCALE = 16.0  # host premultiplier on wq/wk/wv/bq/bk/bv (fp8 subnormal dodge)

# Schraudolph exp-approx constants (bf16 bitcast): i16 = st*SCH_A1 + SCH_B
SCH_A = 128.0 / math.log(2.0)
SCH_B = 16256.0 - 7.4 + 0.5
# fraction of exp tiles routed to VectorE instead of ScalarE
DVE_EXP_FRAC = float(os.environ.get("DVE_EXP_FRAC", "0.36"))

DR = mybir.MatmulPerfMode.DoubleRow


def build_nc(T=S, EMB=E, NH=G, dh=DH, OUT=E, trace_label=""):
    """Emit the per-core Bass/Tile program. All cores run this same program."""
    assert T % 128 == 0 and EMB % 128 == 0 and dh == 64 and NH % 2 == 0
    FEAT = NH * dh
    assert FEAT % 128 == 0
    EC = EMB // 128  # 128-row contraction chunks for projections
    EP = EC // 2  # fp8-folded [64,2,...] chunk-pairs per 256 emb rows
    TT = T // 128  # token tiles
    FT = FEAT // 128  # feature tiles (head pairs)
    SCH = min(512, T)  # matmul moving free-dim chunk
    NSCH = T // SCH
    T2 = max(128, T // 2)  # attention query-half width (2 PSUM banks)
    NSH = T // T2  # query halves per head
    SCH2 = min(512, T2)
    NSCH2 = T2 // SCH2
    _ock = OUT // 2 if 128 < OUT <= 1024 and OUT % 2 == 0 else 512
    OCHUNKS = [(o, min(_ock, OUT - o)) for o in range(0, OUT, _ock)]
    scale = 1.0 / math.sqrt(dh)
    inv_w = 1.0 / W_SCALE

    nc = bacc.Bacc("TRN2", target_bir_lowering=False, debug=False, num_devices=N_CORES)

    # ---- DRAM I/O ----
    # x^T and projection weights fp8, contraction-folded: tile j holds emb rows
    # [256j, 256j+256) as [c*64+p, t*T + s] with e = 256j + 128c + 64t + p.
    xqT_d = nc.dram_tensor("xqT", [128, EP * 2 * T], FP8, kind="ExternalInput").ap()
    xkT_d = nc.dram_tensor("xkT", [128, EP * 2 * T], FP8, kind="ExternalInput").ap()
    xvT_d = nc.dram_tensor("xvT", [128, EC * T], BF16, kind="ExternalInput").ap()
    wq_d = nc.dram_tensor("wq", [128, EP * 2 * FEAT], FP8, kind="ExternalInput").ap()
    wk_d = nc.dram_tensor("wk", [128, EP * 2 * FEAT], FP8, kind="ExternalInput").ap()
    wv_d = nc.dram_tensor("wv", [128, EC * FEAT], BF16, kind="ExternalInput").ap()
    wo_d = nc.dram_tensor("wo", [FEAT, OUT], BF16, kind="ExternalInput").ap()
    bq_d = nc.dram_tensor("bq", [1, 2 * FEAT], FP8, kind="ExternalInput").ap()
    bk_d = nc.dram_tensor("bk", [1, 2 * FEAT], FP8, kind="ExternalInput").ap()
    bv_d = nc.dram_tensor("bv", [1, FEAT], BF16, kind="ExternalInput").ap()
    out_d = nc.dram_tensor("out", [T, OUT], BF16, kind="ExternalOutput").ap()

    with tile.TileContext(nc) as tc, ExitStack() as ctx:
        persist = ctx.enter_context(tc.tile_pool(name="persist", bufs=1))

        # ---- persistent SBUF tensors ----
        wq_big = persist.tile([128, EP * 2 * FEAT], FP8, tag="wq", name="wq")
        wk_big = persist.tile([128, EP * 2 * FEAT], FP8, tag="wk", name="wk")
        wv_big = persist.tile([128, EC * FEAT], BF16, tag="wv", name="wv")
        wq_sb = [wq_big[:, j * 2 * FEAT : (j + 1) * 2 * FEAT] for j in range(EP)]
        wk_sb = [wk_big[:, j * 2 * FEAT : (j + 1) * 2 * FEAT] for j in range(EP)]
        wv_sb = [wv_big[:, j * FEAT : (j + 1) * FEAT] for j in range(EC)]
        wo_sb = [persist.tile([128, OUT], BF16, tag=f"wo{j}", name=f"wo{j}") for j in range(FT)]
        bq_sb = persist.tile([1, 2 * FEAT], FP8, tag="bq", name="bq")
        bk_sb = persist.tile([1, 2 * FEAT], FP8, tag="bk", name="bk")
        bv_sb = persist.tile([1, FEAT], BF16, tag="bv", name="bv")
        ones_row = persist.tile([1, T], BF16, tag="ones_row", name="ones_row")
        ones8 = persist.tile([1, 2 * T], FP8, tag="ones8", name="ones8")
        xq_big = persist.tile([128, EP * 2 * T], FP8, tag="xq", name="xq")
        xk_big = persist.tile([128, EP * 2 * T], FP8, tag="xk", name="xk")
        xv_big = persist.tile([128, EC * T], BF16, tag="xv", name="xv")
        xqT_sb = [xq_big[:, j * 2 * T : (j + 1) * 2 * T] for j in range(EP)]
        xkT_sb = [xk_big[:, j * 2 * T : (j + 1) * 2 * T] for j in range(EP)]
        xvT_sb = [xv_big[:, j * T : (j + 1) * T] for j in range(EC)]
        # q^T/k^T fp8, upper T columns zero (DoubleRow zero k-tile)
        qT_sb = [persist.tile([128, 2 * T], FP8, tag=f"qT{j}", name=f"qT{j}") for j in range(FT)]
        kT_sb = [persist.tile([128, 2 * T], FP8, tag=f"kT{j}", name=f"kT{j}") for j in range(FT)]
        # V token-major, each head augmented with 64 ones columns so the PV
        # matmul emits the softmax denominator replicated on partitions 64-127
        v_sb = [persist.tile([128, NH * (dh + 64)], BF16, tag=f"v{i}", name=f"v{i}") for i in range(TT)]
        cn_sb = [persist.tile([128, T], BF16, tag=f"cn{j}", name=f"cn{j}") for j in range(FT)]

        # ---- weight/bias/x loads (Q path first: it gates head 0; then K,
        # then V which head 0's PV needs, then the out-proj weights) ----
        nc.sync.dma_start(wq_big[:], wq_d[:])
        nc.sync.dma_start(xq_big[:], xqT_d[:])
        nc.sync.dma_start(wk_big[:], wk_d[:])
        nc.sync.dma_start(xk_big[:], xkT_d[:])
        nc.sync.dma_start(bq_sb[:], bq_d[:])
        nc.sync.dma_start(bk_sb[:], bk_d[:])
        nc.sync.dma_start(bv_sb[:], bv_d[:])
        nc.sync.dma_start(wv_big[:], wv_d[:])
        for j in range(EC):
            nc.sync.dma_start(
                xv_big[:, j * T : (j + 1) * T], xvT_d[:, j * T : (j + 1) * T]
            )
        for j in range(FT):
            nc.sync.dma_start(wo_sb[j][:], wo_d[j * 128 : (j + 1) * 128, :])
        nc.vector.memset(ones_row[:], 1.0)
        nc.vector.memset(ones8[:], 1.0)
        # zero halves of q^T/k^T (DoubleRow zero k-tile; never rewritten) on
        # the otherwise-idle Pool engine
        for j in range(FT):
            nc.gpsimd.memset(qT_sb[j][:, T : 2 * T], 0.0)
            nc.gpsimd.memset(kT_sb[j][:, T : 2 * T], 0.0)
        # ones columns of augmented V (written once)
        for i in range(TT):
            vview = v_sb[i][:].rearrange("p (h x) -> p h x", x=dh + 64)
            nc.gpsimd.memset(vview[:, :, dh:], 1.0)

        def dr(ap2w):
            """[p, (2,W)] fp8-folded view of a [128, 2W] tile slice."""
            return ap2w.rearrange("p (t w) -> p t w", t=2)

        # ---- compute: projections + attention + out-projection ----
        # PSUM budget (8 banks): proj 2 (bufs=2 x 1 bank) + ST 4 (bufs=2 x 2)
        # + ctx 2 (bufs=1 x 2). Everything coexists, so Tile can overlap the
        # phases; PE instruction order is software-pipelined by hand.
        with (
            tc.tile_pool(name="stpsum", bufs=2, space="PSUM") as stpool,
            tc.tile_pool(name="dvpsum", bufs=2, space="PSUM") as dvpool,
            tc.tile_pool(name="ctpsum", bufs=1, space="PSUM") as ctpool,
            tc.tile_pool(name="ptpool", bufs=6) as ptpool,
            tc.tile_pool(name="normpool", bufs=3) as npool,
            tc.tile_pool(name="outsb", bufs=4) as osbpool,
        ):

            def proj_qk(j, groups):
                qk = (
                    (wq_sb, bq_sb, xqT_sb, qT_sb),
                    (wk_sb, bk_sb, xkT_sb, kT_sb),
                )
                for t, n in groups:
                    w_sb, b_sb, x_sb, dst = qk[t]
                    ps0 = dvpool.tile([128, SCH2], F32, tag="dv", name="dv")
                    ps = ps0[:, 0:SCH]
                    for e in range(EP):
                        nc.tensor.matmul(
                            ps,
                            dr(w_sb[e])[:, :, j * 128 : (j + 1) * 128],
                            dr(x_sb[e])[:, :, n * SCH : (n + 1) * SCH],
                            start=(e == 0),
                            stop=False,
                            perf_mode=DR,
                        )
                    # bias last (fp8 DoubleRow rank-1 update), so the bias
                    # DMAs can trail the big x/w loads
                    nc.tensor.matmul(
                        ps,
                        dr(b_sb[:])[:, :, j * 128 : (j + 1) * 128],
                        dr(ones8[:])[:, :, 0:SCH],
                        start=False,
                        stop=True,
                        perf_mode=DR,
                    )
                    # fold the x16 weight prescale out while casting to fp8
                    nc.vector.tensor_scalar(
                        dst[j][:, n * SCH : (n + 1) * SCH],
                        ps,
                        inv_w,
                        None,
                        mybir.AluOpType.mult,
                    )

            def proj_v(tiles=None):
                for i in tiles if tiles is not None else range(TT):
                    ps = dvpool.tile([128, SCH2], F32, tag="dv", name="dv")
                    for e in range(EC):
                        nc.tensor.matmul(
                            ps[:, 0:FEAT],
                            xvT_sb[e][:, i * 128 : (i + 1) * 128],
                            wv_sb[e],
                            start=(e == 0),
                            stop=False,
                        )
                    nc.tensor.matmul(
                        ps[:, 0:FEAT], ones_row[:, 0:128], bv_sb[:], start=False, stop=True
                    )
                    dst = v_sb[i][:].rearrange("p (h x) -> p h x", x=dh + 64)[:, :, 0:dh]
                    srcv = ps[:, 0:FEAT].rearrange("p (h d) -> p h d", d=dh)
                    nc.vector.tensor_copy(dst, srcv)

            first_head = [True]
            exp_counter = [0, 0]  # [tiles seen, tiles sent to DVE]

            def st_tile(i, kT_h, qT_h, s0):
                # route a DVE_EXP_FRAC slice of score tiles to a dedicated
                # 1-bank psum pool + VectorE exp, so the ScalarE stream's
                # buffer rotation never blocks on them. Head (0,0) is
                # PE-bound on the JIT V projections so it stays on ScalarE.
                if not first_head[0] and i < TT - 3:
                    exp_counter[0] += 1
                on_dve = int(exp_counter[0] * DVE_EXP_FRAC) > exp_counter[1]
                if on_dve:
                    exp_counter[1] += 1
                    halves = []
                    for n in range(NSCH2):
                        h = dvpool.tile([128, SCH2], F32, tag="dv", name="dv")
                        nc.tensor.matmul(
                            h[:],
                            dr(kT_h)[:, :, i * 128 : (i + 1) * 128],
                            dr(qT_h)[:, :, s0 + n * SCH2 : s0 + (n + 1) * SCH2],
                            start=True,
                            stop=True,
                            perf_mode=DR,
                        )
                        halves.append(h)
                    return ("dve", halves)
                st = stpool.tile([128, T2], F32, tag="st", name="st")
                for n in range(NSCH2):
                    nc.tensor.matmul(
                        st[:, n * SCH2 : (n + 1) * SCH2],
                        dr(kT_h)[:, :, i * 128 : (i + 1) * 128],
                        dr(qT_h)[:, :, s0 + n * SCH2 : s0 + (n + 1) * SCH2],
                        start=True,
                        stop=True,
                        perf_mode=DR,
                    )
                return ("act", st)

            pending_sts = []
            work_q = []

            def exp_tile(kind_st):
                kind, st = kind_st
                if kind == "dve":
                    pt = ptpool.tile([128, T2], I16, tag="pt", name="pt")
                    for n, h in enumerate(st):
                        nc.vector.tensor_scalar(
                            pt[:, n * SCH2 : (n + 1) * SCH2],
                            h[:],
                            SCH_A * scale,
                            SCH_B,
                            mybir.AluOpType.mult,
                            mybir.AluOpType.add,
                        )
                    return pt[:].bitcast(BF16)
                pt = ptpool.tile([128, T2], BF16, tag="pt", name="pt")
                nc.scalar.activation(
                    pt[:], st[:], mybir.ActivationFunctionType.Exp, scale=scale
                )
                return pt[:]

            def head_args(h, sh):
                ft, half = h // 2, (h % 2) * 64
                return (
                    kT_sb[ft][half : half + 64, :],
                    qT_sb[ft][half : half + 64, :],
                    sh * T2,
                )

            def head(h, sh, filler=None, nxt=None):
                # keeps 2 score tiles in flight and pre-issues the NEXT
                # head's first 2 before this head's last context matmul, so
                # the exp stream never stalls at head boundaries
                ft, half = h // 2, (h % 2) * 64
                kT_h, qT_h, s0 = head_args(h, sh)
                ct = ctpool.tile([128, T2], F32, tag="ct", name="ct")
                sts = pending_sts[:]
                del pending_sts[:]
                while len(sts) < min(2, TT):
                    sts.append(st_tile(len(sts), kT_h, qT_h, s0))
                look = len(sts)
                nissued = 0
                for i in range(TT):
                    st = sts.pop(0)
                    pt = exp_tile(st)
                    if look == 2 and i == 0:
                        # issue TWO score tiles before this head's first
                        # context matmul: PV_0 stalls on the previous head's
                        # normalize (single ctx psum buffer) and would
                        # otherwise block the next score tile in PE order
                        sts.append(st_tile(2, kT_h, qT_h, s0))
                        sts.append(st_tile(3, kT_h, qT_h, s0))
                    elif look == 2 and i == 1:
                        pass
                    elif i + look < TT:
                        sts.append(st_tile(i + look, kT_h, qT_h, s0))
                    elif nxt is not None and nissued < min(2, TT):
                        pending_sts.append(st_tile(nissued, *head_args(*nxt)))
                        nissued += 1
                    if filler is not None:
                        filler(i)
                    elif work_q:
                        work_q.pop(0)()
                    for n in range(NSCH2):
                        nc.tensor.matmul(
                            ct[:, n * SCH2 : (n + 1) * SCH2],
                            v_sb[i][:, h * (dh + 64) : (h + 1) * (dh + 64)],
                            pt[:, n * SCH2 : (n + 1) * SCH2],
                            start=(i == 0),
                            stop=(i == TT - 1),
                        )

                # normalize: cn[f, s] = ct[f, s] * (1 / ct[64.., s]); split
                # so downstream out-proj tiles unblock per 512-query chunk
                recip = npool.tile([64, T2], F32, tag="recip", name="recip")
                for n in range(NSCH2):
                    c = slice(n * SCH2, (n + 1) * SCH2)
                    nc.vector.reciprocal(recip[:, c], ct[64:128, c])
                    nc.vector.tensor_tensor(
                        cn_sb[ft][half : half + 64, s0 + n * SCH2 : s0 + (n + 1) * SCH2],
                        ct[0:64, c],
                        recip[:, c],
                        op=mybir.AluOpType.mult,
                    )

            osb_state = {}
            tail_mode = [False]
            tail_ctr = [0]

            def outproj_chunk(i, ci):
                from_st = False
                if tail_mode[0]:
                    tail_ctr[0] += 1
                    from_st = tail_ctr[0] % 2 == 0
                oc, ow = OCHUNKS[ci]
                if ci == 0:
                    osb_state[i] = osbpool.tile([128, OUT], BF16, tag="osb", name="osb")
                osb = osb_state[i]
                if from_st:
                    ps0 = stpool.tile([128, T2], F32, tag="st", name="st")
                else:
                    ps0 = dvpool.tile([128, SCH2], F32, tag="dv", name="dv")
                ps = ps0[:, 0:ow]
                for f in range(FT):
                    nc.tensor.matmul(
                        ps,
                        cn_sb[f][:, i * 128 : (i + 1) * 128],
                        wo_sb[f][:, oc : oc + ow],
                        start=(f == 0),
                        stop=(f == FT - 1),
                    )
                if tail_mode[0] and tail_ctr[0] % 2 == 0:
                    nc.scalar.copy(osb[:, oc : oc + ow], ps)
                else:
                    nc.vector.tensor_copy(osb[:, oc : oc + ow], ps)
                nc.sync.dma_start(out_d[i * 128 : (i + 1) * 128, oc : oc + ow], osb[:, oc : oc + ow])
                if ci == len(OCHUNKS) - 1:
                    del osb_state[i]

            # ---- startup: emit exactly the projection groups the first two
            # score tiles need, pre-issue those tiles, then the rest ----
            proj_qk(0, [(0, 0), (0, 1), (1, 0)])
            for z in range(min(2, TT)):
                pending_sts.append(st_tile(z, *head_args(0, 0)))
            proj_qk(0, [(1, 1), (0, 2), (0, 3), (1, 2), (1, 3)])
            # two more score tiles BEFORE any V work, so the exp stream is
            # never gated by the V matmuls waiting on the late xv DMA
            for z in range(2, 4):
                pending_sts.append(st_tile(z, *head_args(0, 0)))
            # V tile i is first needed at head 0's CT step i: emit tile 0/1
            # up front and drip the rest into head 0's pipeline
            proj_v(range(2))

            def v_filler(i):
                if i + 2 < TT:
                    proj_v([i + 2])
                if i == TT - 1:
                    first_head[0] = False

            half_tiles = T2 // 128 if NSH == 2 else 0
            # sh-outer order: all pairs finish queries [0,T2) first, so that
            # block's out-projection drips through the whole sh=1 phase
            seq = [
                (2 * p + z, sh)
                for sh in range(NSH)
                for p in range(NH // 2)
                for z in (0, 1)
            ]
            for pos, (h, sh) in enumerate(seq):
                p = h // 2
                nxt = seq[pos + 1] if pos + 1 < len(seq) else None
                head(h, sh, v_filler if (h, sh) == (0, 0) else None, nxt=nxt)
                if h % 2 == 0 and sh == 0 and p + 1 < FT:
                    # queue pair p+1's projections a full head before pair
                    # p+1 starts, so its pre-issued score tiles never block
                    work_q.extend(
                        (lambda jj=p + 1, g=(t, n): proj_qk(jj, [g]))
                        for t in range(2)
                        for n in range(NSCH)
                    )
                if h % 2 == 1:
                    # after the LAST pair at this sh: that query block's
                    # out-projection becomes computable; drip it
                    if p == NH // 2 - 1:
                        tiles = range(sh * half_tiles, (sh + 1) * half_tiles)
                        work_q.extend(
                            (lambda ii=i, cc=ci: outproj_chunk(ii, cc))
                            for i in tiles
                            for ci in range(len(OCHUNKS))
                        )
            # tail: whatever the windows didn't absorb, pipelined 4-deep
            # across both free psum pools
            tail_mode[0] = True
            for w in work_q:
                w()
            del work_q[:]

    nc.compile()
    return nc


def _pad_fp8_bias(b):
    """[1, F] f32 -> [1, 2F] fp8 with a zeroed second half (DoubleRow pad)."""
    out = np.zeros((1, 2 * b.shape[1]), np.float32)
    out[:, : b.shape[1]] = b
    return out.astype(NP_FP8)


def _fold_fp8(arr2d, ncols):
    """[768, ncols] f32 -> [3, 128, 2*ncols] fp8 contraction-folded."""
    a = arr2d.reshape(3, 2, 2, 64, ncols).transpose(0, 1, 3, 2, 4)
    a = a.reshape(3, 128, 2 * ncols).transpose(1, 0, 2).reshape(128, 3 * 2 * ncols)
    return np.ascontiguousarray(a).astype(NP_FP8)


def shard_inputs(query, key, value, wq, bq, wk, bk, wv, bv, wo):
    """Build the 8 per-core input maps (host-side cast/fold/slice)."""
    in_maps = []
    xT = {}
    for b in range(B):
        xT[b] = (
            _fold_fp8(np.ascontiguousarray(query[b].T), S),
            _fold_fp8(np.ascontiguousarray(key[b].T), S),
            np.ascontiguousarray(value[b].T.reshape(6, 128, S).transpose(1, 0, 2).reshape(128, 6 * S)).astype(NP_BF16),
        )
    gw = {}
    for g in range(2):
        hs = slice(g * G, (g + 1) * G)
        gw[g] = dict(
            wq=_fold_fp8(W_SCALE * wq[hs].transpose(1, 0, 2).reshape(E, G * DH), G * DH),
            wk=_fold_fp8(W_SCALE * wk[hs].transpose(1, 0, 2).reshape(E, G * DH), G * DH),
            wv=np.ascontiguousarray(
                wv[hs].transpose(1, 0, 2).reshape(E, G * DH).reshape(6, 128, G * DH).transpose(1, 0, 2).reshape(128, 6 * G * DH)
            ).astype(NP_BF16),
            wo=np.ascontiguousarray(wo[g * G * DH : (g + 1) * G * DH, :]).astype(NP_BF16),
            bq=_pad_fp8_bias(W_SCALE * bq[hs].reshape(1, G * DH)),
            bk=_pad_fp8_bias(W_SCALE * bk[hs].reshape(1, G * DH)),
            bv=np.ascontiguousarray(bv[hs].reshape(1, G * DH)).astype(NP_BF16),
        )
    for c in range(N_CORES):
        b, g = c // 2, c % 2
        m = dict(xqT=xT[b][0], xkT=xT[b][1], xvT=xT[b][2])
        m.update(gw[g])
        in_maps.append(m)
    return in_maps


_CACHED_NC = None


def kernel(query, key, value, wq, bq, wk, bk, wv, bv, wo, bo):
    global _CACHED_NC
    query, key, value = (np.asarray(a, np.float32) for a in (query, key, value))
    wq, bq, wk, bk, wv, bv, wo, bo = (
        np.asarray(a, np.float32) for a in (wq, bq, wk, bk, wv, bv, wo, bo)
    )
    in_maps = shard_inputs(query, key, value, wq, bq, wk, bk, wv, bv, wo)
    if _CACHED_NC is None:
        _CACHED_NC = build_nc()
    res = run_bass_kernel_spmd(_CACHED_NC, in_maps, list(range(N_CORES)))
    out = np.empty((B, S, E), np.float32)
    for b in range(B):
        out[b] = (
            res.results[2 * b]["out"].astype(np.float32)
            + res.results[2 * b + 1]["out"].astype(np.float32)
            + bo[None, :]
        )
    return out


# revision 25
# speedup vs baseline: 1.2055x; 1.0053x over previous
"""Multi-head attention (B=4, S=2048, E=768, H=12, Dh=64) on 8 TRN2 NeuronCores.

Sharding: batch x head-group tensor parallel. Core c handles batch b = c//2 and
head group g = c%2 (6 heads each). Each core computes its heads' Q/K/V
projections, full attention over the 2048-token sequence, and a partial
out-projection over its 384 concat-features. The host sums the two partials per
batch and adds the output bias.

Device layout notes:
 - All projection inputs (x^T, w) are hosted in fp8e4 with the contraction dim
   folded [64, 2, .] so every projection matmul runs in DoubleRow perf mode
   (2 contraction rows/cycle). Weights are host-scaled x16 to clear fp8e4's
   subnormal range; the 1/16 is folded into the PSUM->SBUF copy.
 - Q^T/K^T are produced feature-major [128, 2T] fp8e4 with a zeroed upper half:
   score matmuls run DoubleRow with k-tile 0 = the real 64-row dh contraction
   and k-tile 1 = zeros, so scores also stream 2 output cols/cycle.
 - V is token-major bf16, each head augmented with 64 ones columns so the PV
   matmul emits the softmax denominator replicated on psum partitions 64-127.
 - Scores are computed transposed (S^T tiles [128 keys, S queries]); softmax
   exp runs split across TWO engines: most tiles on ScalarE (table exp straight
   out of PSUM), a tunable fraction on VectorE via a Schraudolph bitcast
   approximation (i16 = round(x*128/ln2 + 16249); bitcast to bf16 ~= e^x to
   +-4%, which washes out under the ~2048-token softmax average).
 - PSUM (8 banks) holds proj (2) + double-buffered S^T (4) + ctx (2)
   concurrently; the PE stream is software-pipelined by hand (next score
   tile issued before the current context matmul) and projection /
   out-projection work is injected into the exp-bound attention windows.
"""

import math
import os
import sys
from contextlib import ExitStack

import numpy as np

for _p in ("/opt/trn_rl_repo", "/root/.axon_site/_ro/trn_rl_repo"):
    if os.path.isdir(_p) and _p not in sys.path:
        sys.path.append(_p)

# NTFF tracing hooks (antenv.axon_hooks) don't exist in this container;
# make sure an ambient BASS_TRACE can't route execution into that path.
os.environ["BASS_NEVER_TRACE"] = "1"

import ml_dtypes  # noqa: E402

import concourse.bass as bass  # noqa: E402
import concourse.tile as tile  # noqa: E402
from concourse import bacc, mybir  # noqa: E402
from concourse.bass_utils import run_bass_kernel_spmd  # noqa: E402

BF16 = mybir.dt.bfloat16
F32 = mybir.dt.float32
FP8 = mybir.dt.float8e4
I16 = mybir.dt.int16
NP_BF16 = ml_dtypes.bfloat16
NP_FP8 = ml_dtypes.float8_e4m3

B, S, E, H, DH = 4, 2048, 768, 12, 64
N_CORES = 8
G = H // 2  # heads per core (6)

W_SCALE = 16.0  # host premultiplier on wq/wk/wv/bq/bk/bv (fp8 subnormal dodge)

# Schraudolph exp-approx constants (bf16 bitcast): i16 = st*SCH_A1 + SCH_B
SCH_A = 128.0 / math.log(2.0)
SCH_B = 16256.0 - 7.4 + 0.5
# fraction of exp tiles routed to VectorE instead of ScalarE
DVE_EXP_FRAC = float(os.environ.get("DVE_EXP_FRAC", "0.36"))

DR = mybir.MatmulPerfMode.DoubleRow


def build_nc(T=S, EMB=E, NH=G, dh=DH, OUT=E, trace_label=""):
    """Emit the per-core Bass/Tile program. All cores run this same program."""
    assert T % 128 == 0 and EMB % 128 == 0 and dh == 64 and NH % 2 == 0
    FEAT = NH * dh
    assert FEAT % 128 == 0
    EC = EMB // 128  # 128-row contraction chunks for projections
    EP = EC // 2  # fp8-folded [64,2,...] chunk-pairs per 256 emb rows
    TT = T // 128  # token tiles
    FT = FEAT // 128  # feature tiles (head pairs)
    SCH = min(512, T)  # matmul moving free-dim chunk
    NSCH = T // SCH
    T2 = max(128, T // 2)  # attention query-half width (2 PSUM banks)
    NSH = T // T2  # query halves per head
    SCH2 = min(512, T2)
    NSCH2 = T2 // SCH2
    _ock = OUT // 2 if 128 < OUT <= 1024 and OUT % 2 == 0 else 512
    OCHUNKS = [(o, min(_ock, OUT - o)) for o in range(0, OUT, _ock)]
    scale = 1.0 / math.sqrt(dh)
    inv_w = 1.0 / W_SCALE

    nc = bacc.Bacc("TRN2", target_bir_lowering=False, debug=False, num_devices=N_CORES)

    # ---- DRAM I/O ----
    # x^T and projection weights fp8, contraction-folded: tile j holds emb rows
    # [256j, 256j+256) as [c*64+p, t*T + s] with e = 256j + 128c + 64t + p.
    xqT_d = nc.dram_tensor("xqT", [128, EP * 2 * T], FP8, kind="ExternalInput").ap()
    xkT_d = nc.dram_tensor("xkT", [128, EP * 2 * T], FP8, kind="ExternalInput").ap()
    xvT_d = nc.dram_tensor("xvT", [128, EC * T], BF16, kind="ExternalInput").ap()
    wq_d = nc.dram_tensor("wq", [128, EP * 2 * FEAT], FP8, kind="ExternalInput").ap()
    wk_d = nc.dram_tensor("wk", [128, EP * 2 * FEAT], FP8, kind="ExternalInput").ap()
    wv_d = nc.dram_tensor("wv", [128, EC * FEAT], BF16, kind="ExternalInput").ap()
    wo_d = nc.dram_tensor("wo", [FEAT, OUT], BF16, kind="ExternalInput").ap()
    bq_d = nc.dram_tensor("bq", [1, 2 * FEAT], FP8, kind="ExternalInput").ap()
    bk_d = nc.dram_tensor("bk", [1, 2 * FEAT], FP8, kind="ExternalInput").ap()
    bv_d = nc.dram_tensor("bv", [1, FEAT], BF16, kind="ExternalInput").ap()
    out_d = nc.dram_tensor("out", [T, OUT], BF16, kind="ExternalOutput").ap()

    with tile.TileContext(nc) as tc, ExitStack() as ctx:
        persist = ctx.enter_context(tc.tile_pool(name="persist", bufs=1))

        # ---- persistent SBUF tensors ----
        wq_big = persist.tile([128, EP * 2 * FEAT], FP8, tag="wq", name="wq")
        wk_big = persist.tile([128, EP * 2 * FEAT], FP8, tag="wk", name="wk")
        wv_big = persist.tile([128, EC * FEAT], BF16, tag="wv", name="wv")
        wq_sb = [wq_big[:, j * 2 * FEAT : (j + 1) * 2 * FEAT] for j in range(EP)]
        wk_sb = [wk_big[:, j * 2 * FEAT : (j + 1) * 2 * FEAT] for j in range(EP)]
        wv_sb = [wv_big[:, j * FEAT : (j + 1) * FEAT] for j in range(EC)]
        wo_sb = [persist.tile([128, OUT], BF16, tag=f"wo{j}", name=f"wo{j}") for j in range(FT)]
        bq_sb = persist.tile([1, 2 * FEAT], FP8, tag="bq", name="bq")
        bk_sb = persist.tile([1, 2 * FEAT], FP8, tag="bk", name="bk")
        bv_sb = persist.tile([1, FEAT], BF16, tag="bv", name="bv")
        ones_row = persist.tile([1, T], BF16, tag="ones_row", name="ones_row")
        ones8 = persist.tile([1, 2 * T], FP8, tag="ones8", name="ones8")
        xq_big = persist.tile([128, EP * 2 * T], FP8, tag="xq", name="xq")
        xk_big = persist.tile([128, EP * 2 * T], FP8, tag="xk", name="xk")
        xv_big = persist.tile([128, EC * T], BF16, tag="xv", name="xv")
        xqT_sb = [xq_big[:, j * 2 * T : (j + 1) * 2 * T] for j in range(EP)]
        xkT_sb = [xk_big[:, j * 2 * T : (j + 1) * 2 * T] for j in range(EP)]
        xvT_sb = [xv_big[:, j * T : (j + 1) * T] for j in range(EC)]
        # q^T/k^T fp8, upper T columns zero (DoubleRow zero k-tile)
        qT_sb = [persist.tile([128, 2 * T], FP8, tag=f"qT{j}", name=f"qT{j}") for j in range(FT)]
        kT_sb = [persist.tile([128, 2 * T], FP8, tag=f"kT{j}", name=f"kT{j}") for j in range(FT)]
        # V token-major, each head augmented with 64 ones columns so the PV
        # matmul emits the softmax denominator replicated on partitions 64-127
        v_sb = [persist.tile([128, NH * (dh + 64)], BF16, tag=f"v{i}", name=f"v{i}") for i in range(TT)]
        cn_sb = [persist.tile([128, T], BF16, tag=f"cn{j}", name=f"cn{j}") for j in range(FT)]

        # ---- weight/bias/x loads (Q path first: it gates head 0; then K,
        # then V which head 0's PV needs, then the out-proj weights) ----
        nc.sync.dma_start(wq_big[:], wq_d[:])
        nc.sync.dma_start(xq_big[:], xqT_d[:])
        nc.sync.dma_start(wk_big[:], wk_d[:])
        nc.sync.dma_start(xk_big[:], xkT_d[:])
        nc.sync.dma_start(bq_sb[:], bq_d[:])
        nc.sync.dma_start(bk_sb[:], bk_d[:])
        nc.sync.dma_start(bv_sb[:], bv_d[:])
        nc.sync.dma_start(wv_big[:], wv_d[:])
        for j in range(EC):
            nc.sync.dma_start(
                xv_big[:, j * T : (j + 1) * T], xvT_d[:, j * T : (j + 1) * T]
            )
        for j in range(FT):
            nc.sync.dma_start(wo_sb[j][:], wo_d[j * 128 : (j + 1) * 128, :])
        nc.vector.memset(ones_row[:], 1.0)
        nc.vector.memset(ones8[:], 1.0)
        # zero halves of q^T/k^T (DoubleRow zero k-tile; never rewritten) on
        # the otherwise-idle Pool engine
        for j in range(FT):
            nc.gpsimd.memset(qT_sb[j][:, T : 2 * T], 0.0)
            nc.gpsimd.memset(kT_sb[j][:, T : 2 * T], 0.0)
        # ones columns of augmented V (written once)
        for i in range(TT):
            vview = v_sb[i][:].rearrange("p (h x) -> p h x", x=dh + 64)
            nc.gpsimd.memset(vview[:, :, dh:], 1.0)

        def dr(ap2w):
            """[p, (2,W)] fp8-folded view of a [128, 2W] tile slice."""
            return ap2w.rearrange("p (t w) -> p t w", t=2)

        # ---- compute: projections + attention + out-projection ----
        # PSUM budget (8 banks): proj 2 (bufs=2 x 1 bank) + ST 4 (bufs=2 x 2)
        # + ctx 2 (bufs=1 x 2). Everything coexists, so Tile can overlap the
        # phases; PE instruction order is software-pipelined by hand.
        with (
            tc.tile_pool(name="stpsum", bufs=2, space="PSUM") as stpool,
            tc.tile_pool(name="dvpsum", bufs=2, space="PSUM") as dvpool,
            tc.tile_pool(name="ctpsum", bufs=1, space="PSUM") as ctpool,
            tc.tile_pool(name="ptpool", bufs=6) as ptpool,
            tc.tile_pool(name="normpool", bufs=3) as npool,
            tc.tile_pool(name="outsb", bufs=4) as osbpool,
        ):

            def proj_qk(j, groups):
                qk = (
                    (wq_sb, bq_sb, xqT_sb, qT_sb),
                    (wk_sb, bk_sb, xkT_sb, kT_sb),
                )
                for t, n in groups:
                    w_sb, b_sb, x_sb, dst = qk[t]
                    ps0 = dvpool.tile([128, SCH2], F32, tag="dv", name="dv")
                    ps = ps0[:, 0:SCH]
                    for e in range(EP):
                        nc.tensor.matmul(
                            ps,
                            dr(w_sb[e])[:, :, j * 128 : (j + 1) * 128],
                            dr(x_sb[e])[:, :, n * SCH : (n + 1) * SCH],
                            start=(e == 0),
                            stop=False,
                            perf_mode=DR,
                        )
                    # bias last (fp8 DoubleRow rank-1 update), so the bias
                    # DMAs can trail the big x/w loads
                    nc.tensor.matmul(
                        ps,
                        dr(b_sb[:])[:, :, j * 128 : (j + 1) * 128],
                        dr(ones8[:])[:, :, 0:SCH],
                        start=False,
                        stop=True,
                        perf_mode=DR,
                    )
                    # fold the x16 weight prescale out while casting to fp8
                    nc.vector.tensor_scalar(
                        dst[j][:, n * SCH : (n + 1) * SCH],
                        ps,
                        inv_w,
                        None,
                        mybir.AluOpType.mult,
                    )

            def proj_v(tiles=None):
                for i in tiles if tiles is not None else range(TT):
                    ps = dvpool.tile([128, SCH2], F32, tag="dv", name="dv")
                    for e in range(EC):
                        nc.tensor.matmul(
                            ps[:, 0:FEAT],
                            xvT_sb[e][:, i * 128 : (i + 1) * 128],
                            wv_sb[e],
                            start=(e == 0),
                            stop=False,
                        )
                    nc.tensor.matmul(
                        ps[:, 0:FEAT], ones_row[:, 0:128], bv_sb[:], start=False, stop=True
                    )
                    dst = v_sb[i][:].rearrange("p (h x) -> p h x", x=dh + 64)[:, :, 0:dh]
                    srcv = ps[:, 0:FEAT].rearrange("p (h d) -> p h d", d=dh)
                    nc.vector.tensor_copy(dst, srcv)

            first_head = [True]
            exp_counter = [0, 0]  # [tiles seen, tiles sent to DVE]

            def st_tile(i, kT_h, qT_h, s0):
                # route a DVE_EXP_FRAC slice of score tiles to a dedicated
                # 1-bank psum pool + VectorE exp, so the ScalarE stream's
                # buffer rotation never blocks on them. Head (0,0) is
                # PE-bound on the JIT V projections so it stays on ScalarE.
                if not first_head[0]:
                    exp_counter[0] += 1
                on_dve = int(exp_counter[0] * DVE_EXP_FRAC) > exp_counter[1]
                if on_dve:
                    exp_counter[1] += 1
                    halves = []
                    for n in range(NSCH2):
                        h = dvpool.tile([128, SCH2], F32, tag="dv", name="dv")
                        nc.tensor.matmul(
                            h[:],
                            dr(kT_h)[:, :, i * 128 : (i + 1) * 128],
                            dr(qT_h)[:, :, s0 + n * SCH2 : s0 + (n + 1) * SCH2],
                            start=True,
                            stop=True,
                            perf_mode=DR,
                        )
                        halves.append(h)
                    return ("dve", halves)
                st = stpool.tile([128, T2], F32, tag="st", name="st")
                for n in range(NSCH2):
                    nc.tensor.matmul(
                        st[:, n * SCH2 : (n + 1) * SCH2],
                        dr(kT_h)[:, :, i * 128 : (i + 1) * 128],
                        dr(qT_h)[:, :, s0 + n * SCH2 : s0 + (n + 1) * SCH2],
                        start=True,
                        stop=True,
                        perf_mode=DR,
                    )
                return ("act", st)

            pending_sts = []
            work_q = []

            def exp_tile(kind_st):
                kind, st = kind_st
                if kind == "dve":
                    pt = ptpool.tile([128, T2], I16, tag="pt", name="pt")
                    for n, h in enumerate(st):
                        nc.vector.tensor_scalar(
                            pt[:, n * SCH2 : (n + 1) * SCH2],
                            h[:],
                            SCH_A * scale,
                            SCH_B,
                            mybir.AluOpType.mult,
                            mybir.AluOpType.add,
                        )
                    return pt[:].bitcast(BF16)
                pt = ptpool.tile([128, T2], BF16, tag="pt", name="pt")
                nc.scalar.activation(
                    pt[:], st[:], mybir.ActivationFunctionType.Exp, scale=scale
                )
                return pt[:]

            def head_args(h, sh):
                ft, half = h // 2, (h % 2) * 64
                return (
                    kT_sb[ft][half : half + 64, :],
                    qT_sb[ft][half : half + 64, :],
                    sh * T2,
                )

            def head(h, sh, filler=None, nxt=None):
                # keeps 2 score tiles in flight and pre-issues the NEXT
                # head's first 2 before this head's last context matmul, so
                # the exp stream never stalls at head boundaries
                ft, half = h // 2, (h % 2) * 64
                kT_h, qT_h, s0 = head_args(h, sh)
                ct = ctpool.tile([128, T2], F32, tag="ct", name="ct")
                sts = pending_sts[:]
                del pending_sts[:]
                while len(sts) < min(2, TT):
                    sts.append(st_tile(len(sts), kT_h, qT_h, s0))
                look = len(sts)
                nissued = 0
                for i in range(TT):
                    st = sts.pop(0)
                    pt = exp_tile(st)
                    if i + look < TT:
                        sts.append(st_tile(i + look, kT_h, qT_h, s0))
                    elif nxt is not None and nissued < min(2, TT):
                        pending_sts.append(st_tile(nissued, *head_args(*nxt)))
                        nissued += 1
                    if filler is not None:
                        filler(i)
                    elif work_q and i < TT - 2:
                        work_q.pop(0)()
                    for n in range(NSCH2):
                        nc.tensor.matmul(
                            ct[:, n * SCH2 : (n + 1) * SCH2],
                            v_sb[i][:, h * (dh + 64) : (h + 1) * (dh + 64)],
                            pt[:, n * SCH2 : (n + 1) * SCH2],
                            start=(i == 0),
                            stop=(i == TT - 1),
                        )

                # normalize: cn[f, s] = ct[f, s] * (1 / ct[64.., s]); split
                # so downstream out-proj tiles unblock per 512-query chunk
                recip = npool.tile([64, T2], F32, tag="recip", name="recip")
                for n in range(NSCH2):
                    c = slice(n * SCH2, (n + 1) * SCH2)
                    nc.vector.reciprocal(recip[:, c], ct[64:128, c])
                    nc.vector.tensor_tensor(
                        cn_sb[ft][half : half + 64, s0 + n * SCH2 : s0 + (n + 1) * SCH2],
                        ct[0:64, c],
                        recip[:, c],
                        op=mybir.AluOpType.mult,
                    )

            osb_state = {}
            tail_mode = [False]
            tail_ctr = [0]

            def outproj_chunk(i, ci):
                from_st = False
                if tail_mode[0]:
                    tail_ctr[0] += 1
                    from_st = tail_ctr[0] % 2 == 0
                oc, ow = OCHUNKS[ci]
                if ci == 0:
                    osb_state[i] = osbpool.tile([128, OUT], BF16, tag="osb", name="osb")
                osb = osb_state[i]
                if from_st:
                    ps0 = stpool.tile([128, T2], F32, tag="st", name="st")
                else:
                    ps0 = dvpool.tile([128, SCH2], F32, tag="dv", name="dv")
                ps = ps0[:, 0:ow]
                for f in range(FT):
                    nc.tensor.matmul(
                        ps,
                        cn_sb[f][:, i * 128 : (i + 1) * 128],
                        wo_sb[f][:, oc : oc + ow],
                        start=(f == 0),
                        stop=(f == FT - 1),
                    )
                if tail_mode[0] and tail_ctr[0] % 2 == 0:
                    nc.scalar.copy(osb[:, oc : oc + ow], ps)
                else:
                    nc.vector.tensor_copy(osb[:, oc : oc + ow], ps)
                nc.sync.dma_start(out_d[i * 128 : (i + 1) * 128, oc : oc + ow], osb[:, oc : oc + ow])
                if ci == len(OCHUNKS) - 1:
                    del osb_state[i]

            # ---- startup: emit exactly the projection groups the first two
            # score tiles need, pre-issue those tiles, then the rest ----
            proj_qk(0, [(0, 0), (0, 1), (1, 0)])
            for z in range(min(2, TT)):
                pending_sts.append(st_tile(z, *head_args(0, 0)))
            proj_qk(0, [(1, 1), (0, 2), (0, 3), (1, 2), (1, 3)])
            # two more score tiles BEFORE any V work, so the exp stream is
            # never gated by the V matmuls waiting on the late xv DMA
            for z in range(2, 4):
                pending_sts.append(st_tile(z, *head_args(0, 0)))
            # V tile i is first needed at head 0's CT step i: emit tile 0/1
            # up front and drip the rest into head 0's pipeline
            proj_v(range(2))

            def v_filler(i):
                if i + 2 < TT:
                    proj_v([i + 2])
                if i == TT - 1:
                    first_head[0] = False

            half_tiles = T2 // 128 if NSH == 2 else 0
            # sh-outer order: all pairs finish queries [0,T2) first, so that
            # block's out-projection drips through the whole sh=1 phase
            seq = [
                (2 * p + z, sh)
                for sh in range(NSH)
                for p in range(NH // 2)
                for z in (0, 1)
            ]
            for pos, (h, sh) in enumerate(seq):
                p = h // 2
                nxt = seq[pos + 1] if pos + 1 < len(seq) else None
                head(h, sh, v_filler if (h, sh) == (0, 0) else None, nxt=nxt)
                if h % 2 == 0 and sh == 0 and p + 1 < FT:
                    # queue pair p+1's projections a full head before pair
                    # p+1 starts, so its pre-issued score tiles never block
                    work_q.extend(
                        (lambda jj=p + 1, g=(t, n): proj_qk(jj, [g]))
                        for t in range(2)
                        for n in range(NSCH)
                    )
                if h % 2 == 1:
                    # after the LAST pair at this sh: that query block's
                    # out-projection becomes computable; drip it
                    if p == NH // 2 - 1:
                        tiles = range(sh * half_tiles, (sh + 1) * half_tiles)
                        work_q.extend(
                            (lambda ii=i, cc=ci: outproj_chunk(ii, cc))
                            for i in tiles
                            for ci in range(len(OCHUNKS))
                        )
            # tail: whatever the windows didn't absorb, pipelined 4-deep
            # across both free psum pools
            tail_mode[0] = True
            for w in work_q:
                w()
            del work_q[:]

    nc.compile()
    return nc


def _pad_fp8_bias(b):
    """[1, F] f32 -> [1, 2F] fp8 with a zeroed second half (DoubleRow pad)."""
    out = np.zeros((1, 2 * b.shape[1]), np.float32)
    out[:, : b.shape[1]] = b
    return out.astype(NP_FP8)


def _fold_fp8(arr2d, ncols):
    """[768, ncols] f32 -> [3, 128, 2*ncols] fp8 contraction-folded."""
    a = arr2d.reshape(3, 2, 2, 64, ncols).transpose(0, 1, 3, 2, 4)
    a = a.reshape(3, 128, 2 * ncols).transpose(1, 0, 2).reshape(128, 3 * 2 * ncols)
    return np.ascontiguousarray(a).astype(NP_FP8)


def shard_inputs(query, key, value, wq, bq, wk, bk, wv, bv, wo):
    """Build the 8 per-core input maps (host-side cast/fold/slice)."""
    in_maps = []
    xT = {}
    for b in range(B):
        xT[b] = (
            _fold_fp8(np.ascontiguousarray(query[b].T), S),
            _fold_fp8(np.ascontiguousarray(key[b].T), S),
            np.ascontiguousarray(value[b].T.reshape(6, 128, S).transpose(1, 0, 2).reshape(128, 6 * S)).astype(NP_BF16),
        )
    gw = {}
    for g in range(2):
        hs = slice(g * G, (g + 1) * G)
        gw[g] = dict(
            wq=_fold_fp8(W_SCALE * wq[hs].transpose(1, 0, 2).reshape(E, G * DH), G * DH),
            wk=_fold_fp8(W_SCALE * wk[hs].transpose(1, 0, 2).reshape(E, G * DH), G * DH),
            wv=np.ascontiguousarray(
                wv[hs].transpose(1, 0, 2).reshape(E, G * DH).reshape(6, 128, G * DH).transpose(1, 0, 2).reshape(128, 6 * G * DH)
            ).astype(NP_BF16),
            wo=np.ascontiguousarray(wo[g * G * DH : (g + 1) * G * DH, :]).astype(NP_BF16),
            bq=_pad_fp8_bias(W_SCALE * bq[hs].reshape(1, G * DH)),
            bk=_pad_fp8_bias(W_SCALE * bk[hs].reshape(1, G * DH)),
            bv=np.ascontiguousarray(bv[hs].reshape(1, G * DH)).astype(NP_BF16),
        )
    for c in range(N_CORES):
        b, g = c // 2, c % 2
        m = dict(xqT=xT[b][0], xkT=xT[b][1], xvT=xT[b][2])
        m.update(gw[g])
        in_maps.append(m)
    return in_maps


_CACHED_NC = None


def kernel(query, key, value, wq, bq, wk, bk, wv, bv, wo, bo):
    global _CACHED_NC
    query, key, value = (np.asarray(a, np.float32) for a in (query, key, value))
    wq, bq, wk, bk, wv, bv, wo, bo = (
        np.asarray(a, np.float32) for a in (wq, bq, wk, bk, wv, bv, wo, bo)
    )
    in_maps = shard_inputs(query, key, value, wq, bq, wk, bk, wv, bv, wo)
    if _CACHED_NC is None:
        _CACHED_NC = build_nc()
    res = run_bass_kernel_spmd(_CACHED_NC, in_maps, list(range(N_CORES)))
    out = np.empty((B, S, E), np.float32)
    for b in range(B):
        out[b] = (
            res.results[2 * b]["out"].astype(np.float32)
            + res.results[2 * b + 1]["out"].astype(np.float32)
            + bo[None, :]
        )
    return out


# revision 27
# speedup vs baseline: 1.2134x; 1.0066x over previous
"""Multi-head attention (B=4, S=2048, E=768, H=12, Dh=64) on 8 TRN2 NeuronCores.

Sharding: batch x head-group tensor parallel. Core c handles batch b = c//2 and
head group g = c%2 (6 heads each). Each core computes its heads' Q/K/V
projections, full attention over the 2048-token sequence, and a partial
out-projection over its 384 concat-features. The host sums the two partials per
batch and adds the output bias.

Device layout notes:
 - Q/K projection inputs (x^T, w, biases) are hosted in fp8e4 with the
   contraction dim folded [64, 2, .] so those matmuls run in DoubleRow perf
   mode (2 contraction rows/cycle). Weights are host-scaled x16 to clear
   fp8e4's subnormal range; the 1/16 is folded into the PSUM->SBUF cast.
 - Q^T/K^T are produced feature-major [128, 2T] fp8e4 with a zeroed upper
   half: score matmuls run DoubleRow with k-tile 0 = the real 64-row dh
   contraction and k-tile 1 = zeros, streaming 2 output cols/cycle.
 - The V path stays bf16: V quantization error enters the context linearly
   (unlike Q/K noise, which the softmax damps), and measured error triples
   with V in fp8.
 - V is token-major, each head augmented with 64 ones columns so the PV
   matmul emits the softmax denominator replicated on psum partitions 64-127.
 - Scores are computed transposed (S^T tiles [128 keys, S queries]); softmax
   exp is split across TWO engines: most tiles on ScalarE (table exp straight
   out of PSUM), a DVE_EXP_FRAC slice on VectorE via a Schraudolph bitcast
   approximation (i16 = round(x*128/ln2 + 16249); bitcast to bf16 ~= e^x to
   +-4%, which washes out under the ~2048-token softmax average). VectorE
   tiles use a dedicated 1-bank psum pool (two [128,512] halves) so the
   ScalarE stream's double-buffer rotation never waits on them.
 - PSUM (8 banks): ScalarE S^T 2x2 + VectorE-S^T/projection pool 2x1 +
   ctx 2. The PE stream is software-pipelined by hand; projection and
   out-projection chunks drain one-per-window from a work queue into the
   exp-bound attention windows (sh-outer head order so the first query
   block's out-projection overlaps the second block's attention).
 - Output partials are stored bf16 (summed in f32 on host with the bias);
   stores are per-384-column chunk so the tail drains while computing.
"""

import math
import os
import sys
from contextlib import ExitStack

import numpy as np

for _p in ("/opt/trn_rl_repo", "/root/.axon_site/_ro/trn_rl_repo"):
    if os.path.isdir(_p) and _p not in sys.path:
        sys.path.append(_p)

# NTFF tracing hooks (antenv.axon_hooks) don't exist in this container;
# make sure an ambient BASS_TRACE can't route execution into that path.
os.environ["BASS_NEVER_TRACE"] = "1"

import ml_dtypes  # noqa: E402

import concourse.bass as bass  # noqa: E402
import concourse.tile as tile  # noqa: E402
from concourse import bacc, mybir  # noqa: E402
from concourse.bass_utils import run_bass_kernel_spmd  # noqa: E402

BF16 = mybir.dt.bfloat16
F32 = mybir.dt.float32
FP8 = mybir.dt.float8e4
I16 = mybir.dt.int16
NP_BF16 = ml_dtypes.bfloat16
NP_FP8 = ml_dtypes.float8_e4m3

B, S, E, H, DH = 4, 2048, 768, 12, 64
N_CORES = 8
G = H // 2  # heads per core (6)

W_SCALE = 16.0  # host premultiplier on wq/wk/wv/bq/bk/bv (fp8 subnormal dodge)

# Schraudolph exp-approx constants (bf16 bitcast): i16 = st*SCH_A1 + SCH_B
SCH_A = 128.0 / math.log(2.0)
SCH_B = 16256.0 - 7.4 + 0.5
# fraction of exp tiles routed to VectorE instead of ScalarE
DVE_EXP_FRAC = float(os.environ.get("DVE_EXP_FRAC", "0.34"))

DR = mybir.MatmulPerfMode.DoubleRow


def build_nc(T=S, EMB=E, NH=G, dh=DH, OUT=E, trace_label=""):
    """Emit the per-core Bass/Tile program. All cores run this same program."""
    assert T % 128 == 0 and EMB % 128 == 0 and dh == 64 and NH % 2 == 0
    FEAT = NH * dh
    assert FEAT % 128 == 0
    EC = EMB // 128  # 128-row contraction chunks for projections
    EP = EC // 2  # fp8-folded [64,2,...] chunk-pairs per 256 emb rows
    TT = T // 128  # token tiles
    FT = FEAT // 128  # feature tiles (head pairs)
    SCH = min(512, T)  # matmul moving free-dim chunk
    NSCH = T // SCH
    T2 = max(128, T // 2)  # attention query-half width (2 PSUM banks)
    NSH = T // T2  # query halves per head
    SCH2 = min(512, T2)
    NSCH2 = T2 // SCH2
    _ock = OUT // 2 if 128 < OUT <= 1024 and OUT % 2 == 0 else 512
    OCHUNKS = [(o, min(_ock, OUT - o)) for o in range(0, OUT, _ock)]
    scale = 1.0 / math.sqrt(dh)
    inv_w = 1.0 / W_SCALE

    nc = bacc.Bacc("TRN2", target_bir_lowering=False, debug=False, num_devices=N_CORES)

    # ---- DRAM I/O ----
    # x^T and projection weights fp8, contraction-folded: tile j holds emb rows
    # [256j, 256j+256) as [c*64+p, t*T + s] with e = 256j + 128c + 64t + p.
    xqT_d = nc.dram_tensor("xqT", [128, EP * 2 * T], FP8, kind="ExternalInput").ap()
    xkT_d = nc.dram_tensor("xkT", [128, EP * 2 * T], FP8, kind="ExternalInput").ap()
    xvT_d = nc.dram_tensor("xvT", [128, EC * T], BF16, kind="ExternalInput").ap()
    wq_d = nc.dram_tensor("wq", [128, EP * 2 * FEAT], FP8, kind="ExternalInput").ap()
    wk_d = nc.dram_tensor("wk", [128, EP * 2 * FEAT], FP8, kind="ExternalInput").ap()
    wv_d = nc.dram_tensor("wv", [128, EC * FEAT], BF16, kind="ExternalInput").ap()
    wo_d = nc.dram_tensor("wo", [FEAT, OUT], BF16, kind="ExternalInput").ap()
    bq_d = nc.dram_tensor("bq", [1, 2 * FEAT], FP8, kind="ExternalInput").ap()
    bk_d = nc.dram_tensor("bk", [1, 2 * FEAT], FP8, kind="ExternalInput").ap()
    bv_d = nc.dram_tensor("bv", [1, FEAT], BF16, kind="ExternalInput").ap()
    out_d = nc.dram_tensor("out", [T, OUT], BF16, kind="ExternalOutput").ap()

    with tile.TileContext(nc) as tc, ExitStack() as ctx:
        persist = ctx.enter_context(tc.tile_pool(name="persist", bufs=1))

        # ---- persistent SBUF tensors ----
        wq_big = persist.tile([128, EP * 2 * FEAT], FP8, tag="wq", name="wq")
        wk_big = persist.tile([128, EP * 2 * FEAT], FP8, tag="wk", name="wk")
        wv_big = persist.tile([128, EC * FEAT], BF16, tag="wv", name="wv")
        wq_sb = [wq_big[:, j * 2 * FEAT : (j + 1) * 2 * FEAT] for j in range(EP)]
        wk_sb = [wk_big[:, j * 2 * FEAT : (j + 1) * 2 * FEAT] for j in range(EP)]
        wv_sb = [wv_big[:, j * FEAT : (j + 1) * FEAT] for j in range(EC)]
        wo_sb = [persist.tile([128, OUT], BF16, tag=f"wo{j}", name=f"wo{j}") for j in range(FT)]
        bq_sb = persist.tile([1, 2 * FEAT], FP8, tag="bq", name="bq")
        bk_sb = persist.tile([1, 2 * FEAT], FP8, tag="bk", name="bk")
        bv_sb = persist.tile([1, FEAT], BF16, tag="bv", name="bv")
        ones_row = persist.tile([1, T], BF16, tag="ones_row", name="ones_row")
        ones8 = persist.tile([1, 2 * T], FP8, tag="ones8", name="ones8")
        xq_big = persist.tile([128, EP * 2 * T], FP8, tag="xq", name="xq")
        xk_big = persist.tile([128, EP * 2 * T], FP8, tag="xk", name="xk")
        xv_big = persist.tile([128, EC * T], BF16, tag="xv", name="xv")
        xqT_sb = [xq_big[:, j * 2 * T : (j + 1) * 2 * T] for j in range(EP)]
        xkT_sb = [xk_big[:, j * 2 * T : (j + 1) * 2 * T] for j in range(EP)]
        xvT_sb = [xv_big[:, j * T : (j + 1) * T] for j in range(EC)]
        # q^T/k^T fp8, upper T columns zero (DoubleRow zero k-tile)
        qT_sb = [persist.tile([128, 2 * T], FP8, tag=f"qT{j}", name=f"qT{j}") for j in range(FT)]
        kT_sb = [persist.tile([128, 2 * T], FP8, tag=f"kT{j}", name=f"kT{j}") for j in range(FT)]
        # V token-major, each head augmented with 64 ones columns so the PV
        # matmul emits the softmax denominator replicated on partitions 64-127
        v_sb = [persist.tile([128, NH * (dh + 64)], BF16, tag=f"v{i}", name=f"v{i}") for i in range(TT)]
        cn_sb = [persist.tile([128, T], BF16, tag=f"cn{j}", name=f"cn{j}") for j in range(FT)]

        # ---- weight/bias/x loads (Q path first: it gates head 0; then K,
        # then V which head 0's PV needs, then the out-proj weights) ----
        nc.sync.dma_start(wq_big[:], wq_d[:])
        nc.sync.dma_start(xq_big[:], xqT_d[:])
        nc.sync.dma_start(wk_big[:], wk_d[:])
        nc.sync.dma_start(xk_big[:], xkT_d[:])
        nc.sync.dma_start(bq_sb[:], bq_d[:])
        nc.sync.dma_start(bk_sb[:], bk_d[:])
        nc.sync.dma_start(bv_sb[:], bv_d[:])
        nc.sync.dma_start(wv_big[:], wv_d[:])
        for j in range(EC):
            nc.sync.dma_start(
                xv_big[:, j * T : (j + 1) * T], xvT_d[:, j * T : (j + 1) * T]
            )
        for j in range(FT):
            nc.sync.dma_start(wo_sb[j][:], wo_d[j * 128 : (j + 1) * 128, :])
        nc.vector.memset(ones_row[:], 1.0)
        nc.vector.memset(ones8[:], 1.0)
        # zero halves of q^T/k^T (DoubleRow zero k-tile; never rewritten) on
        # the otherwise-idle Pool engine
        for j in range(FT):
            nc.gpsimd.memset(qT_sb[j][:, T : 2 * T], 0.0)
            nc.gpsimd.memset(kT_sb[j][:, T : 2 * T], 0.0)
        # ones columns of augmented V (written once)
        for i in range(TT):
            vview = v_sb[i][:].rearrange("p (h x) -> p h x", x=dh + 64)
            nc.gpsimd.memset(vview[:, :, dh:], 1.0)

        def dr(ap2w):
            """[p, (2,W)] fp8-folded view of a [128, 2W] tile slice."""
            return ap2w.rearrange("p (t w) -> p t w", t=2)

        # ---- compute: projections + attention + out-projection ----
        # PSUM budget (8 banks): proj 2 (bufs=2 x 1 bank) + ST 4 (bufs=2 x 2)
        # + ctx 2 (bufs=1 x 2). Everything coexists, so Tile can overlap the
        # phases; PE instruction order is software-pipelined by hand.
        with (
            tc.tile_pool(name="stpsum", bufs=2, space="PSUM") as stpool,
            tc.tile_pool(name="dvpsum", bufs=2, space="PSUM") as dvpool,
            tc.tile_pool(name="ctpsum", bufs=1, space="PSUM") as ctpool,
            tc.tile_pool(name="ptpool", bufs=6) as ptpool,
            tc.tile_pool(name="normpool", bufs=3) as npool,
            tc.tile_pool(name="outsb", bufs=4) as osbpool,
        ):

            def proj_qk(j, groups):
                qk = (
                    (wq_sb, bq_sb, xqT_sb, qT_sb),
                    (wk_sb, bk_sb, xkT_sb, kT_sb),
                )
                for t, n in groups:
                    w_sb, b_sb, x_sb, dst = qk[t]
                    ps0 = dvpool.tile([128, SCH2], F32, tag="dv", name="dv")
                    ps = ps0[:, 0:SCH]
                    for e in range(EP):
                        nc.tensor.matmul(
                            ps,
                            dr(w_sb[e])[:, :, j * 128 : (j + 1) * 128],
                            dr(x_sb[e])[:, :, n * SCH : (n + 1) * SCH],
                            start=(e == 0),
                            stop=False,
                            perf_mode=DR,
                        )
                    # bias last (fp8 DoubleRow rank-1 update), so the bias
                    # DMAs can trail the big x/w loads
                    nc.tensor.matmul(
                        ps,
                        dr(b_sb[:])[:, :, j * 128 : (j + 1) * 128],
                        dr(ones8[:])[:, :, 0:SCH],
                        start=False,
                        stop=True,
                        perf_mode=DR,
                    )
                    # fold the x16 weight prescale out while casting to fp8
                    nc.vector.tensor_scalar(
                        dst[j][:, n * SCH : (n + 1) * SCH],
                        ps,
                        inv_w,
                        None,
                        mybir.AluOpType.mult,
                    )

            def proj_v(tiles=None):
                for i in tiles if tiles is not None else range(TT):
                    ps = dvpool.tile([128, SCH2], F32, tag="dv", name="dv")
                    for e in range(EC):
                        nc.tensor.matmul(
                            ps[:, 0:FEAT],
                            xvT_sb[e][:, i * 128 : (i + 1) * 128],
                            wv_sb[e],
                            start=(e == 0),
                            stop=False,
                        )
                    nc.tensor.matmul(
                        ps[:, 0:FEAT], ones_row[:, 0:128], bv_sb[:], start=False, stop=True
                    )
                    dst = v_sb[i][:].rearrange("p (h x) -> p h x", x=dh + 64)[:, :, 0:dh]
                    srcv = ps[:, 0:FEAT].rearrange("p (h d) -> p h d", d=dh)
                    nc.vector.tensor_copy(dst, srcv)

            first_head = [True]
            exp_counter = [0, 0]  # [tiles seen, tiles sent to DVE]

            def st_tile(i, kT_h, qT_h, s0):
                # route a DVE_EXP_FRAC slice of score tiles to a dedicated
                # 1-bank psum pool + VectorE exp, so the ScalarE stream's
                # buffer rotation never blocks on them. Head (0,0) is
                # PE-bound on the JIT V projections so it stays on ScalarE.
                if not first_head[0]:
                    exp_counter[0] += 1
                on_dve = int(exp_counter[0] * DVE_EXP_FRAC) > exp_counter[1]
                if on_dve:
                    exp_counter[1] += 1
                    halves = []
                    for n in range(NSCH2):
                        h = dvpool.tile([128, SCH2], F32, tag="dv", name="dv")
                        nc.tensor.matmul(
                            h[:],
                            dr(kT_h)[:, :, i * 128 : (i + 1) * 128],
                            dr(qT_h)[:, :, s0 + n * SCH2 : s0 + (n + 1) * SCH2],
                            start=True,
                            stop=True,
                            perf_mode=DR,
                        )
                        halves.append(h)
                    return ("dve", halves)
                st = stpool.tile([128, T2], F32, tag="st", name="st")
                for n in range(NSCH2):
                    nc.tensor.matmul(
                        st[:, n * SCH2 : (n + 1) * SCH2],
                        dr(kT_h)[:, :, i * 128 : (i + 1) * 128],
                        dr(qT_h)[:, :, s0 + n * SCH2 : s0 + (n + 1) * SCH2],
                        start=True,
                        stop=True,
                        perf_mode=DR,
                    )
                return ("act", st)

            pending_sts = []
            work_q = []

            def exp_tile(kind_st):
                kind, st = kind_st
                if kind == "dve":
                    pt = ptpool.tile([128, T2], I16, tag="pt", name="pt")
                    for n, h in enumerate(st):
                        nc.vector.tensor_scalar(
                            pt[:, n * SCH2 : (n + 1) * SCH2],
                            h[:],
                            SCH_A * scale,
                            SCH_B,
                            mybir.AluOpType.mult,
                            mybir.AluOpType.add,
                        )
                    return pt[:].bitcast(BF16)
                pt = ptpool.tile([128, T2], BF16, tag="pt", name="pt")
                nc.scalar.activation(
                    pt[:], st[:], mybir.ActivationFunctionType.Exp, scale=scale
                )
                return pt[:]

            def head_args(h, sh):
                ft, half = h // 2, (h % 2) * 64
                return (
                    kT_sb[ft][half : half + 64, :],
                    qT_sb[ft][half : half + 64, :],
                    sh * T2,
                )

            def head(h, sh, filler=None, nxt=None):
                # keeps 2 score tiles in flight and pre-issues the NEXT
                # head's first 2 before this head's last context matmul, so
                # the exp stream never stalls at head boundaries
                ft, half = h // 2, (h % 2) * 64
                kT_h, qT_h, s0 = head_args(h, sh)
                ct = ctpool.tile([128, T2], F32, tag="ct", name="ct")
                sts = pending_sts[:]
                del pending_sts[:]
                while len(sts) < min(2, TT):
                    sts.append(st_tile(len(sts), kT_h, qT_h, s0))
                look = len(sts)
                nissued = 0
                for i in range(TT):
                    st = sts.pop(0)
                    pt = exp_tile(st)
                    if i + look < TT:
                        sts.append(st_tile(i + look, kT_h, qT_h, s0))
                    elif nxt is not None and nissued < min(2, TT):
                        pending_sts.append(st_tile(nissued, *head_args(*nxt)))
                        nissued += 1
                    if filler is not None:
                        filler(i)
                    elif work_q and i < TT - 2:
                        work_q.pop(0)()
                    for n in range(NSCH2):
                        nc.tensor.matmul(
                            ct[:, n * SCH2 : (n + 1) * SCH2],
                            v_sb[i][:, h * (dh + 64) : (h + 1) * (dh + 64)],
                            pt[:, n * SCH2 : (n + 1) * SCH2],
                            start=(i == 0),
                            stop=(i == TT - 1),
                        )

                # normalize: cn[f, s] = ct[f, s] * (1 / ct[64.., s]); split
                # so downstream out-proj tiles unblock per 512-query chunk
                recip = npool.tile([64, T2], F32, tag="recip", name="recip")
                for n in range(NSCH2):
                    c = slice(n * SCH2, (n + 1) * SCH2)
                    nc.vector.reciprocal(recip[:, c], ct[64:128, c])
                    nc.vector.tensor_tensor(
                        cn_sb[ft][half : half + 64, s0 + n * SCH2 : s0 + (n + 1) * SCH2],
                        ct[0:64, c],
                        recip[:, c],
                        op=mybir.AluOpType.mult,
                    )

            osb_state = {}
            tail_mode = [False]
            tail_ctr = [0]

            def outproj_chunk(i, ci):
                from_st = False
                if tail_mode[0]:
                    tail_ctr[0] += 1
                    from_st = tail_ctr[0] % 2 == 0
                oc, ow = OCHUNKS[ci]
                if ci == 0:
                    osb_state[i] = osbpool.tile([128, OUT], BF16, tag="osb", name="osb")
                osb = osb_state[i]
                if from_st:
                    ps0 = stpool.tile([128, T2], F32, tag="st", name="st")
                else:
                    ps0 = dvpool.tile([128, SCH2], F32, tag="dv", name="dv")
                ps = ps0[:, 0:ow]
                for f in range(FT):
                    nc.tensor.matmul(
                        ps,
                        cn_sb[f][:, i * 128 : (i + 1) * 128],
                        wo_sb[f][:, oc : oc + ow],
                        start=(f == 0),
                        stop=(f == FT - 1),
                    )
                if tail_mode[0] and tail_ctr[0] % 2 == 0:
                    nc.scalar.copy(osb[:, oc : oc + ow], ps)
                else:
                    nc.vector.tensor_copy(osb[:, oc : oc + ow], ps)
                nc.sync.dma_start(out_d[i * 128 : (i + 1) * 128, oc : oc + ow], osb[:, oc : oc + ow])
                if ci == len(OCHUNKS) - 1:
                    del osb_state[i]

            # ---- startup: emit exactly the projection groups the first two
            # score tiles need, pre-issue those tiles, then the rest ----
            proj_qk(0, [(0, 0), (0, 1), (1, 0)])
            for z in range(min(2, TT)):
                pending_sts.append(st_tile(z, *head_args(0, 0)))
            proj_qk(0, [(1, 1), (0, 2), (0, 3), (1, 2), (1, 3)])
            # two more score tiles BEFORE any V work, so the exp stream is
            # never gated by the V matmuls waiting on the late xv DMA
            for z in range(2, 4):
                pending_sts.append(st_tile(z, *head_args(0, 0)))
            # V tile i is first needed at head 0's CT step i: emit tile 0/1
            # up front and drip the rest into head 0's pipeline
            proj_v(range(2))

            def v_filler(i):
                if i + 2 < TT:
                    proj_v([i + 2])
                if i == TT - 1:
                    first_head[0] = False

            half_tiles = T2 // 128 if NSH == 2 else 0
            # sh-outer order: all pairs finish queries [0,T2) first, so that
            # block's out-projection drips through the whole sh=1 phase
            seq = [
                (2 * p + z, sh)
                for sh in range(NSH)
                for p in range(NH // 2)
                for z in (0, 1)
            ]
            for pos, (h, sh) in enumerate(seq):
                p = h // 2
                nxt = seq[pos + 1] if pos + 1 < len(seq) else None
                head(h, sh, v_filler if (h, sh) == (0, 0) else None, nxt=nxt)
                if h % 2 == 0 and sh == 0 and p + 1 < FT:
                    # queue pair p+1's projections a full head before pair
                    # p+1 starts, so its pre-issued score tiles never block
                    work_q.extend(
                        (lambda jj=p + 1, g=(t, n): proj_qk(jj, [g]))
                        for t in range(2)
                        for n in range(NSCH)
                    )
                if h % 2 == 1:
                    # after the LAST pair at this sh: that query block's
                    # out-projection becomes computable; drip it
                    if p == NH // 2 - 1:
                        tiles = range(sh * half_tiles, (sh + 1) * half_tiles)
                        work_q.extend(
                            (lambda ii=i, cc=ci: outproj_chunk(ii, cc))
                            for i in tiles
                            for ci in range(len(OCHUNKS))
                        )
            # tail: whatever the windows didn't absorb, pipelined 4-deep
            # across both free psum pools
            tail_mode[0] = True
            for w in work_q:
                w()
            del work_q[:]

    nc.compile()
    return nc


def _pad_fp8_bias(b):
    """[1, F] f32 -> [1, 2F] fp8 with a zeroed second half (DoubleRow pad)."""
    out = np.zeros((1, 2 * b.shape[1]), np.float32)
    out[:, : b.shape[1]] = b
    return out.astype(NP_FP8)


def _fold_fp8(arr2d, ncols):
    """[768, ncols] f32 -> [3, 128, 2*ncols] fp8 contraction-folded."""
    a = arr2d.reshape(3, 2, 2, 64, ncols).transpose(0, 1, 3, 2, 4)
    a = a.reshape(3, 128, 2 * ncols).transpose(1, 0, 2).reshape(128, 3 * 2 * ncols)
    return np.ascontiguousarray(a).astype(NP_FP8)


def shard_inputs(query, key, value, wq, bq, wk, bk, wv, bv, wo):
    """Build the 8 per-core input maps (host-side cast/fold/slice)."""
    in_maps = []
    xT = {}
    for b in range(B):
        xT[b] = (
            _fold_fp8(np.ascontiguousarray(query[b].T), S),
            _fold_fp8(np.ascontiguousarray(key[b].T), S),
            np.ascontiguousarray(value[b].T.reshape(6, 128, S).transpose(1, 0, 2).reshape(128, 6 * S)).astype(NP_BF16),
        )
    gw = {}
    for g in range(2):
        hs = slice(g * G, (g + 1) * G)
        gw[g] = dict(
            wq=_fold_fp8(W_SCALE * wq[hs].transpose(1, 0, 2).reshape(E, G * DH), G * DH),
            wk=_fold_fp8(W_SCALE * wk[hs].transpose(1, 0, 2).reshape(E, G * DH), G * DH),
            wv=np.ascontiguousarray(
                wv[hs].transpose(1, 0, 2).reshape(E, G * DH).reshape(6, 128, G * DH).transpose(1, 0, 2).reshape(128, 6 * G * DH)
            ).astype(NP_BF16),
            wo=np.ascontiguousarray(wo[g * G * DH : (g + 1) * G * DH, :]).astype(NP_BF16),
            bq=_pad_fp8_bias(W_SCALE * bq[hs].reshape(1, G * DH)),
            bk=_pad_fp8_bias(W_SCALE * bk[hs].reshape(1, G * DH)),
            bv=np.ascontiguousarray(bv[hs].reshape(1, G * DH)).astype(NP_BF16),
        )
    for c in range(N_CORES):
        b, g = c // 2, c % 2
        m = dict(xqT=xT[b][0], xkT=xT[b][1], xvT=xT[b][2])
        m.update(gw[g])
        in_maps.append(m)
    return in_maps


_CACHED_NC = None


def kernel(query, key, value, wq, bq, wk, bk, wv, bv, wo, bo):
    global _CACHED_NC
    query, key, value = (np.asarray(a, np.float32) for a in (query, key, value))
    wq, bq, wk, bk, wv, bv, wo, bo = (
        np.asarray(a, np.float32) for a in (wq, bq, wk, bk, wv, bv, wo, bo)
    )
    in_maps = shard_inputs(query, key, value, wq, bq, wk, bk, wv, bv, wo)
    if _CACHED_NC is None:
        _CACHED_NC = build_nc()
    res = run_bass_kernel_spmd(_CACHED_NC, in_maps, list(range(N_CORES)))
    out = np.empty((B, S, E), np.float32)
    for b in range(B):
        out[b] = (
            res.results[2 * b]["out"].astype(np.float32)
            + res.results[2 * b + 1]["out"].astype(np.float32)
            + bo[None, :]
        )
    return out


# revision 32
# speedup vs baseline: 1.2209x; 1.0062x over previous
"""Multi-head attention (B=4, S=2048, E=768, H=12, Dh=64) on 8 TRN2 NeuronCores.

Sharding: batch x head-group tensor parallel. Core c handles batch b = c//2 and
head group g = c%2 (6 heads each). Each core computes its heads' Q/K/V
projections, full attention over the 2048-token sequence, and a partial
out-projection over its 384 concat-features. The host sums the two partials per
batch and adds the output bias.

Device layout notes:
 - Q/K projection inputs (x^T, w, biases) are hosted in fp8e4 with the
   contraction dim folded [64, 2, .] so those matmuls run in DoubleRow perf
   mode (2 contraction rows/cycle). Weights are host-scaled x16 to clear
   fp8e4's subnormal range; the 1/16 is folded into the PSUM->SBUF cast.
 - Q^T/K^T are produced feature-major [128, 2T] fp8e4 with a zeroed upper
   half: score matmuls run DoubleRow with k-tile 0 = the real 64-row dh
   contraction and k-tile 1 = zeros, streaming 2 output cols/cycle.
 - The V path stays bf16: V quantization error enters the context linearly
   (unlike Q/K noise, which the softmax damps), and measured error triples
   with V in fp8.
 - V is token-major, each head augmented with 64 ones columns so the PV
   matmul emits the softmax denominator replicated on psum partitions 64-127.
 - Scores are computed transposed (S^T tiles [128 keys, S queries]); softmax
   exp is split across TWO engines: most tiles on ScalarE (table exp straight
   out of PSUM), a DVE_EXP_FRAC slice on VectorE via a Schraudolph bitcast
   approximation (i16 = round(x*128/ln2 + 16249); bitcast to bf16 ~= e^x to
   +-4%, which washes out under the ~2048-token softmax average). VectorE
   tiles use a dedicated 1-bank psum pool (two [128,512] halves) so the
   ScalarE stream's double-buffer rotation never waits on them.
 - PSUM (8 banks): ScalarE S^T 2x2 + VectorE-S^T/projection pool 2x1 +
   ctx 2. The PE stream is software-pipelined by hand; projection and
   out-projection chunks drain one-per-window from a work queue into the
   exp-bound attention windows (sh-outer head order so the first query
   block's out-projection overlaps the second block's attention).
 - Output partials are stored bf16 (summed in f32 on host with the bias);
   stores are per-384-column chunk so the tail drains while computing.
"""

import math
import os
import sys
from contextlib import ExitStack

import numpy as np

for _p in ("/opt/trn_rl_repo", "/root/.axon_site/_ro/trn_rl_repo"):
    if os.path.isdir(_p) and _p not in sys.path:
        sys.path.append(_p)

# NTFF tracing hooks (antenv.axon_hooks) don't exist in this container;
# make sure an ambient BASS_TRACE can't route execution into that path.
os.environ["BASS_NEVER_TRACE"] = "1"

import ml_dtypes  # noqa: E402

import concourse.bass as bass  # noqa: E402
import concourse.tile as tile  # noqa: E402
from concourse import bacc, mybir  # noqa: E402
from concourse.bass_utils import run_bass_kernel_spmd  # noqa: E402

BF16 = mybir.dt.bfloat16
F32 = mybir.dt.float32
FP8 = mybir.dt.float8e4
I16 = mybir.dt.int16
NP_BF16 = ml_dtypes.bfloat16
NP_FP8 = ml_dtypes.float8_e4m3

B, S, E, H, DH = 4, 2048, 768, 12, 64
N_CORES = 8
G = H // 2  # heads per core (6)

W_SCALE = 16.0  # host premultiplier on wq/wk/wv/bq/bk/bv (fp8 subnormal dodge)

# Schraudolph exp-approx constants (bf16 bitcast): i16 = st*SCH_A1 + SCH_B
SCH_A = 128.0 / math.log(2.0)
SCH_B = 16256.0 - 7.4 + 0.5
# fraction of exp tiles routed to VectorE instead of ScalarE
DVE_EXP_FRAC = float(os.environ.get("DVE_EXP_FRAC", "0.34"))

DR = mybir.MatmulPerfMode.DoubleRow


def build_nc(T=S, EMB=E, NH=G, dh=DH, OUT=E, trace_label=""):
    """Emit the per-core Bass/Tile program. All cores run this same program."""
    assert T % 128 == 0 and EMB % 128 == 0 and dh == 64 and NH % 2 == 0
    FEAT = NH * dh
    assert FEAT % 128 == 0
    EC = EMB // 128  # 128-row contraction chunks for projections
    EP = EC // 2  # fp8-folded [64,2,...] chunk-pairs per 256 emb rows
    TT = T // 128  # token tiles
    FT = FEAT // 128  # feature tiles (head pairs)
    SCH = min(512, T)  # matmul moving free-dim chunk
    NSCH = T // SCH
    T2 = max(128, T // 2)  # attention query-half width (2 PSUM banks)
    NSH = T // T2  # query halves per head
    SCH2 = min(512, T2)
    NSCH2 = T2 // SCH2
    _ock = OUT // 2 if 128 < OUT <= 1024 and OUT % 2 == 0 else 512
    OCHUNKS = [(o, min(_ock, OUT - o)) for o in range(0, OUT, _ock)]
    scale = 1.0 / math.sqrt(dh)
    inv_w = 1.0 / W_SCALE

    nc = bacc.Bacc("TRN2", target_bir_lowering=False, debug=False, num_devices=N_CORES)

    # ---- DRAM I/O ----
    # x^T and projection weights fp8, contraction-folded: tile j holds emb rows
    # [256j, 256j+256) as [c*64+p, t*T + s] with e = 256j + 128c + 64t + p.
    xqT_d = nc.dram_tensor("xqT", [128, EP * 2 * T], FP8, kind="ExternalInput").ap()
    xkT_d = nc.dram_tensor("xkT", [128, EP * 2 * T], FP8, kind="ExternalInput").ap()
    xvT_d = nc.dram_tensor("xvT", [128, EC * T], BF16, kind="ExternalInput").ap()
    wq_d = nc.dram_tensor("wq", [128, EP * 2 * FEAT], FP8, kind="ExternalInput").ap()
    wk_d = nc.dram_tensor("wk", [128, EP * 2 * FEAT], FP8, kind="ExternalInput").ap()
    wv_d = nc.dram_tensor("wv", [128, EC * FEAT], BF16, kind="ExternalInput").ap()
    wo_d = nc.dram_tensor("wo", [FEAT, OUT], BF16, kind="ExternalInput").ap()
    bq_d = nc.dram_tensor("bq", [128, FEAT // 128], F32, kind="ExternalInput").ap()
    bk_d = nc.dram_tensor("bk", [128, FEAT // 128], F32, kind="ExternalInput").ap()
    bv_d = nc.dram_tensor("bv", [1, FEAT], BF16, kind="ExternalInput").ap()
    out_d = nc.dram_tensor("out", [T, OUT], BF16, kind="ExternalOutput").ap()

    with tile.TileContext(nc) as tc, ExitStack() as ctx:
        persist = ctx.enter_context(tc.tile_pool(name="persist", bufs=1))

        # ---- persistent SBUF tensors ----
        wq_big = persist.tile([128, EP * 2 * FEAT], FP8, tag="wq", name="wq")
        wk_big = persist.tile([128, EP * 2 * FEAT], FP8, tag="wk", name="wk")
        wv_big = persist.tile([128, EC * FEAT], BF16, tag="wv", name="wv")
        wq_sb = [wq_big[:, j * 2 * FEAT : (j + 1) * 2 * FEAT] for j in range(EP)]
        wk_sb = [wk_big[:, j * 2 * FEAT : (j + 1) * 2 * FEAT] for j in range(EP)]
        wv_sb = [wv_big[:, j * FEAT : (j + 1) * FEAT] for j in range(EC)]
        wo_sb = [persist.tile([128, OUT], BF16, tag=f"wo{j}", name=f"wo{j}") for j in range(FT)]
        bq_sb = persist.tile([128, FEAT // 128], F32, tag="bq", name="bq")
        bk_sb = persist.tile([128, FEAT // 128], F32, tag="bk", name="bk")
        bv_sb = persist.tile([1, FEAT], BF16, tag="bv", name="bv")
        ones_row = persist.tile([1, T], BF16, tag="ones_row", name="ones_row")
        T2X = T // 2
        xq_lo = persist.tile([128, EP * 2 * T2X], FP8, tag="xqlo", name="xqlo")
        xq_hi = persist.tile([128, EP * 2 * T2X], FP8, tag="xqhi", name="xqhi")
        xk_lo = persist.tile([128, EP * 2 * T2X], FP8, tag="xklo", name="xklo")
        xk_hi = persist.tile([128, EP * 2 * T2X], FP8, tag="xkhi", name="xkhi")
        xv_big = persist.tile([128, EC * T], BF16, tag="xv", name="xv")
        # per-(chunk-pair, token-half) views [128, (t, T/2)]
        xqT_sb = [
            (xq_lo[:, j * 2 * T2X : (j + 1) * 2 * T2X], xq_hi[:, j * 2 * T2X : (j + 1) * 2 * T2X])
            for j in range(EP)
        ]
        xkT_sb = [
            (xk_lo[:, j * 2 * T2X : (j + 1) * 2 * T2X], xk_hi[:, j * 2 * T2X : (j + 1) * 2 * T2X])
            for j in range(EP)
        ]
        xvT_sb = [xv_big[:, j * T : (j + 1) * T] for j in range(EC)]
        # q^T/k^T fp8, upper T columns zero (DoubleRow zero k-tile)
        qT_sb = [persist.tile([128, 2 * T], FP8, tag=f"qT{j}", name=f"qT{j}") for j in range(FT)]
        kT_sb = [persist.tile([128, 2 * T], FP8, tag=f"kT{j}", name=f"kT{j}") for j in range(FT)]
        # V token-major, each head augmented with 64 ones columns so the PV
        # matmul emits the softmax denominator replicated on partitions 64-127
        v_sb = [persist.tile([128, NH * (dh + 64)], BF16, tag=f"v{i}", name=f"v{i}") for i in range(TT)]
        cn_sb = [persist.tile([128, T], BF16, tag=f"cn{j}", name=f"cn{j}") for j in range(FT)]

        # ---- weight/bias/x loads (Q path first: it gates head 0; then K,
        # then V which head 0's PV needs, then the out-proj weights) ----
        # token-split loads: the first score tiles need only q/k tokens
        # [0, T/2); load the lo halves first so exp_0 isn't gated on all of x
        HX = EP * 2 * T2X
        nc.sync.dma_start(wq_big[:], wq_d[:])
        nc.sync.dma_start(xq_lo[:], xqT_d[:, 0:HX])
        nc.sync.dma_start(bq_sb[:], bq_d[:])
        nc.sync.dma_start(bk_sb[:], bk_d[:])
        nc.sync.dma_start(wk_big[:], wk_d[:])
        nc.sync.dma_start(xk_lo[:], xkT_d[:, 0:HX])
        nc.sync.dma_start(bv_sb[:], bv_d[:])
        nc.sync.dma_start(xq_hi[:], xqT_d[:, HX:])
        nc.sync.dma_start(xk_hi[:], xkT_d[:, HX:])
        nc.sync.dma_start(wv_big[:], wv_d[:])
        for j in range(EC):
            nc.sync.dma_start(
                xv_big[:, j * T : (j + 1) * T], xvT_d[:, j * T : (j + 1) * T]
            )
        for j in range(FT):
            nc.sync.dma_start(wo_sb[j][:], wo_d[j * 128 : (j + 1) * 128, :])
        nc.gpsimd.memset(ones_row[:], 1.0)
        # zero halves of q^T/k^T (DoubleRow zero k-tile; never rewritten) on
        # the otherwise-idle Pool engine
        for j in range(FT):
            nc.gpsimd.memset(qT_sb[j][:, T : 2 * T], 0.0)
            nc.gpsimd.memset(kT_sb[j][:, T : 2 * T], 0.0)
        # ones columns of augmented V (written once)
        for i in range(TT):
            vview = v_sb[i][:].rearrange("p (h x) -> p h x", x=dh + 64)
            nc.gpsimd.memset(vview[:, :, dh:], 1.0)

        def dr(ap2w):
            """[p, (2,W)] fp8-folded view of a [128, 2W] tile slice."""
            return ap2w.rearrange("p (t w) -> p t w", t=2)

        def dr2(aphw):
            """[p, (2, T/2)] fp8-folded view of a token-half slice."""
            return aphw.rearrange("p (t w) -> p t w", t=2)

        # ---- compute: projections + attention + out-projection ----
        # PSUM budget (8 banks): proj 2 (bufs=2 x 1 bank) + ST 4 (bufs=2 x 2)
        # + ctx 2 (bufs=1 x 2). Everything coexists, so Tile can overlap the
        # phases; PE instruction order is software-pipelined by hand.
        with (
            tc.tile_pool(name="stpsum", bufs=2, space="PSUM") as stpool,
            tc.tile_pool(name="dvpsum", bufs=2, space="PSUM") as dvpool,
            tc.tile_pool(name="ctpsum", bufs=1, space="PSUM") as ctpool,
            tc.tile_pool(name="ptpool", bufs=6) as ptpool,
            tc.tile_pool(name="normpool", bufs=3) as npool,
            tc.tile_pool(name="outsb", bufs=4) as osbpool,
        ):

            def proj_qk(j, groups):
                qk = (
                    (wq_sb, bq_sb, xqT_sb, qT_sb),
                    (wk_sb, bk_sb, xkT_sb, kT_sb),
                )
                for t, n in groups:
                    w_sb, b_sb, x_sb, dst = qk[t]
                    ps0 = dvpool.tile([128, SCH2], F32, tag="dv", name="dv")
                    ps = ps0[:, 0:SCH]
                    nh2 = NSCH // 2
                    xsl = n * SCH - (n // nh2) * T2X
                    for e in range(EP):
                        nc.tensor.matmul(
                            ps,
                            dr(w_sb[e])[:, :, j * 128 : (j + 1) * 128],
                            dr2(x_sb[e][n // nh2])[:, :, xsl : xsl + SCH],
                            start=(e == 0),
                            stop=(e == EP - 1),
                            perf_mode=DR,
                        )
                    # bias is per-PSUM-partition here (features on partitions):
                    # add it in the copy and fold out the x16 weight prescale
                    nc.vector.tensor_scalar(
                        dst[j][:, n * SCH : (n + 1) * SCH],
                        ps,
                        b_sb[:, j : j + 1],
                        inv_w,
                        mybir.AluOpType.add,
                        mybir.AluOpType.mult,
                    )

            def proj_v(tiles=None):
                for i in tiles if tiles is not None else range(TT):
                    ps = dvpool.tile([128, SCH2], F32, tag="dv", name="dv")
                    for e in range(EC):
                        nc.tensor.matmul(
                            ps[:, 0:FEAT],
                            xvT_sb[e][:, i * 128 : (i + 1) * 128],
                            wv_sb[e],
                            start=(e == 0),
                            stop=False,
                        )
                    nc.tensor.matmul(
                        ps[:, 0:FEAT], ones_row[:, 0:128], bv_sb[:], start=False, stop=True
                    )
                    dst = v_sb[i][:].rearrange("p (h x) -> p h x", x=dh + 64)[:, :, 0:dh]
                    srcv = ps[:, 0:FEAT].rearrange("p (h d) -> p h d", d=dh)
                    nc.vector.tensor_copy(dst, srcv)

            first_head = [True]
            exp_counter = [0, 0]  # [tiles seen, tiles sent to DVE]

            def st_tile(i, kT_h, qT_h, s0):
                # route a DVE_EXP_FRAC slice of score tiles to a dedicated
                # 1-bank psum pool + VectorE exp, so the ScalarE stream's
                # buffer rotation never blocks on them. Head (0,0) is
                # PE-bound on the JIT V projections so it stays on ScalarE.
                if not first_head[0]:
                    exp_counter[0] += 1
                on_dve = int(exp_counter[0] * DVE_EXP_FRAC) > exp_counter[1]
                if on_dve:
                    exp_counter[1] += 1
                    halves = []
                    for n in range(NSCH2):
                        h = dvpool.tile([128, SCH2], F32, tag="dv", name="dv")
                        nc.tensor.matmul(
                            h[:],
                            dr(kT_h)[:, :, i * 128 : (i + 1) * 128],
                            dr(qT_h)[:, :, s0 + n * SCH2 : s0 + (n + 1) * SCH2],
                            start=True,
                            stop=True,
                            perf_mode=DR,
                        )
                        halves.append(h)
                    return ("dve", halves)
                st = stpool.tile([128, T2], F32, tag="st", name="st")
                for n in range(NSCH2):
                    nc.tensor.matmul(
                        st[:, n * SCH2 : (n + 1) * SCH2],
                        dr(kT_h)[:, :, i * 128 : (i + 1) * 128],
                        dr(qT_h)[:, :, s0 + n * SCH2 : s0 + (n + 1) * SCH2],
                        start=True,
                        stop=True,
                        perf_mode=DR,
                    )
                return ("act", st)

            pending_sts = []
            work_q = []

            def exp_tile(kind_st):
                kind, st = kind_st
                if kind == "dve":
                    pt = ptpool.tile([128, T2], I16, tag="pt", name="pt")
                    for n, h in enumerate(st):
                        nc.vector.tensor_scalar(
                            pt[:, n * SCH2 : (n + 1) * SCH2],
                            h[:],
                            SCH_A * scale,
                            SCH_B,
                            mybir.AluOpType.mult,
                            mybir.AluOpType.add,
                        )
                    return pt[:].bitcast(BF16)
                pt = ptpool.tile([128, T2], BF16, tag="pt", name="pt")
                nc.scalar.activation(
                    pt[:], st[:], mybir.ActivationFunctionType.Exp, scale=scale
                )
                return pt[:]

            def head_args(h, sh):
                ft, half = h // 2, (h % 2) * 64
                return (
                    kT_sb[ft][half : half + 64, :],
                    qT_sb[ft][half : half + 64, :],
                    sh * T2,
                )

            def head(h, sh, filler=None, nxt=None):
                # keeps 2 score tiles in flight and pre-issues the NEXT
                # head's first 2 before this head's last context matmul, so
                # the exp stream never stalls at head boundaries
                ft, half = h // 2, (h % 2) * 64
                kT_h, qT_h, s0 = head_args(h, sh)
                ct = ctpool.tile([128, T2], F32, tag="ct", name="ct")
                sts = pending_sts[:]
                del pending_sts[:]
                while len(sts) < min(2, TT):
                    sts.append(st_tile(len(sts), kT_h, qT_h, s0))
                look = len(sts)
                nissued = 0
                for i in range(TT):
                    st = sts.pop(0)
                    pt = exp_tile(st)
                    if i + look < TT:
                        sts.append(st_tile(i + look, kT_h, qT_h, s0))
                    elif nxt is not None and nissued < min(2, TT):
                        pending_sts.append(st_tile(nissued, *head_args(*nxt)))
                        nissued += 1
                    if filler is not None:
                        filler(i)
                    elif work_q and i < TT - 2:
                        work_q.pop(0)()
                    for n in range(NSCH2):
                        nc.tensor.matmul(
                            ct[:, n * SCH2 : (n + 1) * SCH2],
                            v_sb[i][:, h * (dh + 64) : (h + 1) * (dh + 64)],
                            pt[:, n * SCH2 : (n + 1) * SCH2],
                            start=(i == 0),
                            stop=(i == TT - 1),
                        )

                # normalize: cn[f, s] = ct[f, s] * (1 / ct[64.., s]); split
                # so downstream out-proj tiles unblock per 512-query chunk
                recip = npool.tile([64, T2], F32, tag="recip", name="recip")
                for n in range(NSCH2):
                    c = slice(n * SCH2, (n + 1) * SCH2)
                    nc.vector.reciprocal(recip[:, c], ct[64:128, c])
                    nc.vector.tensor_tensor(
                        cn_sb[ft][half : half + 64, s0 + n * SCH2 : s0 + (n + 1) * SCH2],
                        ct[0:64, c],
                        recip[:, c],
                        op=mybir.AluOpType.mult,
                    )

            osb_state = {}
            tail_mode = [False]
            tail_ctr = [0]

            def outproj_chunk(i, ci):
                from_st = False
                if tail_mode[0]:
                    tail_ctr[0] += 1
                    from_st = tail_ctr[0] % 2 == 0
                oc, ow = OCHUNKS[ci]
                if ci == 0:
                    osb_state[i] = osbpool.tile([128, OUT], BF16, tag="osb", name="osb")
                osb = osb_state[i]
                if from_st:
                    ps0 = stpool.tile([128, T2], F32, tag="st", name="st")
                else:
                    ps0 = dvpool.tile([128, SCH2], F32, tag="dv", name="dv")
                ps = ps0[:, 0:ow]
                for f in range(FT):
                    nc.tensor.matmul(
                        ps,
                        cn_sb[f][:, i * 128 : (i + 1) * 128],
                        wo_sb[f][:, oc : oc + ow],
                        start=(f == 0),
                        stop=(f == FT - 1),
                    )
                if tail_mode[0] and tail_ctr[0] % 2 == 0:
                    nc.scalar.copy(osb[:, oc : oc + ow], ps)
                else:
                    nc.vector.tensor_copy(osb[:, oc : oc + ow], ps)
                nc.sync.dma_start(out_d[i * 128 : (i + 1) * 128, oc : oc + ow], osb[:, oc : oc + ow])
                if ci == len(OCHUNKS) - 1:
                    del osb_state[i]

            # ---- startup: emit exactly the projection groups the first two
            # score tiles need, pre-issue those tiles, then the rest ----
            proj_qk(0, [(0, 0), (0, 1), (1, 0)])
            for z in range(min(2, TT)):
                pending_sts.append(st_tile(z, *head_args(0, 0)))
            proj_qk(0, [(1, 1), (0, 2), (0, 3), (1, 2), (1, 3)])
            # two more score tiles BEFORE any V work, so the exp stream is
            # never gated by the V matmuls waiting on the late xv DMA
            for z in range(2, 4):
                pending_sts.append(st_tile(z, *head_args(0, 0)))
            # V tile i is first needed at head 0's CT step i: emit tile 0/1
            # up front and drip the rest into head 0's pipeline
            proj_v(range(2))

            def v_filler(i):
                if i + 2 < TT:
                    proj_v([i + 2])
                if i == TT - 1:
                    first_head[0] = False

            half_tiles = T2 // 128 if NSH == 2 else 0
            # sh-outer order: all pairs finish queries [0,T2) first, so that
            # block's out-projection drips through the whole sh=1 phase
            seq = [
                (2 * p + z, sh)
                for sh in range(NSH)
                for p in range(NH // 2)
                for z in (0, 1)
            ]
            for pos, (h, sh) in enumerate(seq):
                p = h // 2
                nxt = seq[pos + 1] if pos + 1 < len(seq) else None
                head(h, sh, v_filler if (h, sh) == (0, 0) else None, nxt=nxt)
                if h % 2 == 0 and sh == 0 and p + 1 < FT:
                    # queue pair p+1's projections a full head before pair
                    # p+1 starts, so its pre-issued score tiles never block
                    work_q.extend(
                        (lambda jj=p + 1, g=(t, n): proj_qk(jj, [g]))
                        for t in range(2)
                        for n in range(NSCH)
                    )
                if h % 2 == 1:
                    # after the LAST pair at this sh: that query block's
                    # out-projection becomes computable; drip it
                    if p == NH // 2 - 1:
                        tiles = range(sh * half_tiles, (sh + 1) * half_tiles)
                        work_q.extend(
                            (lambda ii=i, cc=ci: outproj_chunk(ii, cc))
                            for i in tiles
                            for ci in range(len(OCHUNKS))
                        )
            # tail: whatever the windows didn't absorb, pipelined 4-deep
            # across both free psum pools
            tail_mode[0] = True
            for w in work_q:
                w()
            del work_q[:]

    nc.compile()
    return nc


def _pad_fp8_bias(b):
    """[1, F] f32 -> [1, 2F] fp8 with a zeroed second half (DoubleRow pad)."""
    out = np.zeros((1, 2 * b.shape[1]), np.float32)
    out[:, : b.shape[1]] = b
    return out.astype(NP_FP8)


def _fold_fp8(arr2d, ncols, token_split=False):
    """[768, ncols] f32 -> [128, 3*2*ncols] fp8 contraction-folded; with
    token_split, columns ordered [half, j, t, ncols/2]."""
    a = arr2d.reshape(3, 2, 2, 64, ncols).transpose(0, 1, 3, 2, 4)
    a = a.reshape(3, 128, 2, ncols)
    if token_split:
        h = ncols // 2
        a = a.reshape(3, 128, 2, 2, h).transpose(3, 0, 1, 2, 4)  # [half, j, p, t, h]
        a = a.transpose(2, 0, 1, 3, 4).reshape(128, 3 * 2 * ncols)
    else:
        a = a.transpose(1, 0, 2, 3).reshape(128, 3 * 2 * ncols)
    return np.ascontiguousarray(a).astype(NP_FP8)


def shard_inputs(query, key, value, wq, bq, wk, bk, wv, bv, wo):
    """Build the 8 per-core input maps (host-side cast/fold/slice)."""
    in_maps = []
    xT = {}
    for b in range(B):
        xT[b] = (
            _fold_fp8(np.ascontiguousarray(query[b].T), S, token_split=True),
            _fold_fp8(np.ascontiguousarray(key[b].T), S, token_split=True),
            np.ascontiguousarray(value[b].T.reshape(6, 128, S).transpose(1, 0, 2).reshape(128, 6 * S)).astype(NP_BF16),
        )
    gw = {}
    for g in range(2):
        hs = slice(g * G, (g + 1) * G)
        gw[g] = dict(
            wq=_fold_fp8(W_SCALE * wq[hs].transpose(1, 0, 2).reshape(E, G * DH), G * DH),
            wk=_fold_fp8(W_SCALE * wk[hs].transpose(1, 0, 2).reshape(E, G * DH), G * DH),
            wv=np.ascontiguousarray(
                wv[hs].transpose(1, 0, 2).reshape(E, G * DH).reshape(6, 128, G * DH).transpose(1, 0, 2).reshape(128, 6 * G * DH)
            ).astype(NP_BF16),
            wo=np.ascontiguousarray(wo[g * G * DH : (g + 1) * G * DH, :]).astype(NP_BF16),
            bq=np.ascontiguousarray(W_SCALE * bq[hs].reshape(3, 128).T).astype(np.float32),
            bk=np.ascontiguousarray(W_SCALE * bk[hs].reshape(3, 128).T).astype(np.float32),
            bv=np.ascontiguousarray(bv[hs].reshape(1, G * DH)).astype(NP_BF16),
        )
    for c in range(N_CORES):
        b, g = c // 2, c % 2
        m = dict(xqT=xT[b][0], xkT=xT[b][1], xvT=xT[b][2])
        m.update(gw[g])
        in_maps.append(m)
    return in_maps


_CACHED_NC = None


def kernel(query, key, value, wq, bq, wk, bk, wv, bv, wo, bo):
    global _CACHED_NC
    query, key, value = (np.asarray(a, np.float32) for a in (query, key, value))
    wq, bq, wk, bk, wv, bv, wo, bo = (
        np.asarray(a, np.float32) for a in (wq, bq, wk, bk, wv, bv, wo, bo)
    )
    in_maps = shard_inputs(query, key, value, wq, bq, wk, bk, wv, bv, wo)
    if _CACHED_NC is None:
        _CACHED_NC = build_nc()
    res = run_bass_kernel_spmd(_CACHED_NC, in_maps, list(range(N_CORES)))
    out = np.empty((B, S, E), np.float32)
    for b in range(B):
        out[b] = (
            res.results[2 * b]["out"].astype(np.float32)
            + res.results[2 * b + 1]["out"].astype(np.float32)
            + bo[None, :]
        )
    return out


# revision 36
# speedup vs baseline: 1.2925x; 1.0587x over previous
"""Multi-head attention (B=4, S=2048, E=768, H=12, Dh=64) on 8 TRN2 NeuronCores.

Sharding: batch x head-group tensor parallel. Core c handles batch b = c//2 and
head group g = c%2 (6 heads each). Each core computes its heads' Q/K/V
projections, full attention over the 2048-token sequence, and a partial
out-projection over its 384 concat-features. The host sums the two partials per
batch and adds the output bias.

Device layout notes:
 - Q/K projection inputs (x^T, w, biases) are hosted in fp8e4 with the
   contraction dim folded [64, 2, .] so those matmuls run in DoubleRow perf
   mode (2 contraction rows/cycle). Weights are host-scaled x16 to clear
   fp8e4's subnormal range; the 1/16 is folded into the PSUM->SBUF cast.
 - Q^T/K^T are produced feature-major [128, 2T] fp8e4 with a zeroed upper
   half: score matmuls run DoubleRow with k-tile 0 = the real 64-row dh
   contraction and k-tile 1 = zeros, streaming 2 output cols/cycle.
 - The V path stays bf16: V quantization error enters the context linearly
   (unlike Q/K noise, which the softmax damps), and measured error triples
   with V in fp8.
 - V is token-major, each head augmented with 64 ones columns so the PV
   matmul emits the softmax denominator replicated on psum partitions 64-127.
 - Scores are computed transposed (S^T tiles [128 keys, S queries]); softmax
   exp is split across TWO engines: most tiles on ScalarE (table exp straight
   out of PSUM), a DVE_EXP_FRAC slice on VectorE via a Schraudolph bitcast
   approximation (i16 = round(x*128/ln2 + 16249); bitcast to bf16 ~= e^x to
   +-4%, which washes out under the ~2048-token softmax average). VectorE
   tiles use a dedicated 1-bank psum pool (two [128,512] halves) so the
   ScalarE stream's double-buffer rotation never waits on them.
 - PSUM (8 banks): ScalarE S^T 2x2 + VectorE-S^T/projection pool 2x1 +
   ctx 2. The PE stream is software-pipelined by hand; projection and
   out-projection chunks drain one-per-window from a work queue into the
   exp-bound attention windows (sh-outer head order so the first query
   block's out-projection overlaps the second block's attention).
 - Output partials are stored bf16 (summed in f32 on host with the bias);
   stores are per-384-column chunk so the tail drains while computing.
"""

import math
import os
import sys
from contextlib import ExitStack

import numpy as np

for _p in ("/opt/trn_rl_repo", "/root/.axon_site/_ro/trn_rl_repo"):
    if os.path.isdir(_p) and _p not in sys.path:
        sys.path.append(_p)

# NTFF tracing hooks (antenv.axon_hooks) don't exist in this container;
# make sure an ambient BASS_TRACE can't route execution into that path.
os.environ["BASS_NEVER_TRACE"] = "1"

import ml_dtypes  # noqa: E402

import concourse.bass as bass  # noqa: E402
import concourse.tile as tile  # noqa: E402
from concourse import bacc, mybir  # noqa: E402
from concourse.bass_utils import run_bass_kernel_spmd  # noqa: E402

BF16 = mybir.dt.bfloat16
F32 = mybir.dt.float32
FP8 = mybir.dt.float8e4
I16 = mybir.dt.int16
NP_BF16 = ml_dtypes.bfloat16
NP_FP8 = ml_dtypes.float8_e4m3

B, S, E, H, DH = 4, 2048, 768, 12, 64
N_CORES = 8
G = H // 2  # heads per core (6)

W_SCALE = 16.0  # host premultiplier on wq/wk/wv/bq/bk/bv (fp8 subnormal dodge)

# Schraudolph exp-approx constants (bf16 bitcast): i16 = st*SCH_A1 + SCH_B
SCH_A = 128.0 / math.log(2.0)
SCH_B = 16256.0 - 7.4 + 0.5
# fraction of exp tiles routed to VectorE instead of ScalarE
DVE_EXP_FRAC = float(os.environ.get("DVE_EXP_FRAC", "0.27"))

DR = mybir.MatmulPerfMode.DoubleRow


def build_nc(T=S, EMB=E, NH=G, dh=DH, OUT=E, trace_label=""):
    """Emit the per-core Bass/Tile program. All cores run this same program."""
    assert T % 128 == 0 and EMB % 128 == 0 and dh == 64 and NH % 2 == 0
    FEAT = NH * dh
    assert FEAT % 128 == 0
    EC = EMB // 128  # 128-row contraction chunks for projections
    EP = EC // 2  # fp8-folded [64,2,...] chunk-pairs per 256 emb rows
    TT = T // 128  # token tiles
    FT = FEAT // 128  # feature tiles (head pairs)
    SCH = min(512, T)  # matmul moving free-dim chunk
    NSCH = T // SCH
    T2 = max(128, T // 2)  # attention query-half width (2 PSUM banks)
    NSH = T // T2  # query halves per head
    SCH2 = min(512, T2)
    NSCH2 = T2 // SCH2
    _ock = OUT // 2 if 128 < OUT <= 1024 and OUT % 2 == 0 else 512
    OCHUNKS = [(o, min(_ock, OUT - o)) for o in range(0, OUT, _ock)]
    scale = 1.0 / math.sqrt(dh)
    inv_w = 1.0 / W_SCALE

    nc = bacc.Bacc("TRN2", target_bir_lowering=False, debug=False, num_devices=N_CORES)

    # ---- DRAM I/O ----
    # x^T and projection weights fp8, contraction-folded: tile j holds emb rows
    # [256j, 256j+256) as [c*64+p, t*T + s] with e = 256j + 128c + 64t + p.
    xqT_d = nc.dram_tensor("xqT", [128, EP * 2 * T], FP8, kind="ExternalInput").ap()
    xkT_d = nc.dram_tensor("xkT", [128, EP * 2 * T], FP8, kind="ExternalInput").ap()
    xvT_d = nc.dram_tensor("xvT", [128, EC * T], BF16, kind="ExternalInput").ap()
    wq_d = nc.dram_tensor("wq", [128, EP * 2 * FEAT], FP8, kind="ExternalInput").ap()
    wk_d = nc.dram_tensor("wk", [128, EP * 2 * FEAT], FP8, kind="ExternalInput").ap()
    wv_d = nc.dram_tensor("wv", [128, EC * FEAT], BF16, kind="ExternalInput").ap()
    wo_d = nc.dram_tensor("wo", [FEAT, OUT], BF16, kind="ExternalInput").ap()
    bq_d = nc.dram_tensor("bq", [128, FEAT // 128], F32, kind="ExternalInput").ap()
    bk_d = nc.dram_tensor("bk", [128, FEAT // 128], F32, kind="ExternalInput").ap()
    bv_d = nc.dram_tensor("bv", [1, FEAT], BF16, kind="ExternalInput").ap()
    out_d = nc.dram_tensor("out", [T, OUT], BF16, kind="ExternalOutput").ap()

    with tile.TileContext(nc) as tc, ExitStack() as ctx:
        persist = ctx.enter_context(tc.tile_pool(name="persist", bufs=1))

        # ---- persistent SBUF tensors ----
        wq_big = persist.tile([128, EP * 2 * FEAT], FP8, tag="wq", name="wq")
        wk_big = persist.tile([128, EP * 2 * FEAT], FP8, tag="wk", name="wk")
        wv_big = persist.tile([128, EC * FEAT], BF16, tag="wv", name="wv")
        wq_sb = [wq_big[:, j * 2 * FEAT : (j + 1) * 2 * FEAT] for j in range(EP)]
        wk_sb = [wk_big[:, j * 2 * FEAT : (j + 1) * 2 * FEAT] for j in range(EP)]
        wv_sb = [wv_big[:, j * FEAT : (j + 1) * FEAT] for j in range(EC)]
        wo_sb = [persist.tile([128, OUT], BF16, tag=f"wo{j}", name=f"wo{j}") for j in range(FT)]
        bq_sb = persist.tile([128, FEAT // 128], F32, tag="bq", name="bq")
        bk_sb = persist.tile([128, FEAT // 128], F32, tag="bk", name="bk")
        bv_sb = persist.tile([1, FEAT], BF16, tag="bv", name="bv")
        ones_row = persist.tile([1, T], BF16, tag="ones_row", name="ones_row")
        T2X = T // 2
        xq_lo = persist.tile([128, EP * 2 * T2X], FP8, tag="xqlo", name="xqlo")
        xq_hi = persist.tile([128, EP * 2 * T2X], FP8, tag="xqhi", name="xqhi")
        xk_lo = persist.tile([128, EP * 2 * T2X], FP8, tag="xklo", name="xklo")
        xk_hi = persist.tile([128, EP * 2 * T2X], FP8, tag="xkhi", name="xkhi")
        TQ = T // 4
        xv_q = [
            persist.tile([128, EC * TQ], BF16, tag=f"xvq{k}", name=f"xvq{k}")
            for k in range(4)
        ]
        # per-(chunk-pair, token-half) views [128, (t, T/2)]
        xqT_sb = [
            (xq_lo[:, j * 2 * T2X : (j + 1) * 2 * T2X], xq_hi[:, j * 2 * T2X : (j + 1) * 2 * T2X])
            for j in range(EP)
        ]
        xkT_sb = [
            (xk_lo[:, j * 2 * T2X : (j + 1) * 2 * T2X], xk_hi[:, j * 2 * T2X : (j + 1) * 2 * T2X])
            for j in range(EP)
        ]

        # q^T/k^T fp8, upper T columns zero (DoubleRow zero k-tile)
        qT_sb = [persist.tile([128, 2 * T], FP8, tag=f"qT{j}", name=f"qT{j}") for j in range(FT)]
        kT_sb = [persist.tile([128, 2 * T], FP8, tag=f"kT{j}", name=f"kT{j}") for j in range(FT)]
        # V token-major, each head augmented with 64 ones columns so the PV
        # matmul emits the softmax denominator replicated on partitions 64-127
        v_sb = [persist.tile([128, NH * (dh + 64)], BF16, tag=f"v{i}", name=f"v{i}") for i in range(TT)]
        cn_sb = [persist.tile([128, T], BF16, tag=f"cn{j}", name=f"cn{j}") for j in range(FT)]

        # ---- weight/bias/x loads (Q path first: it gates head 0; then K,
        # then V which head 0's PV needs, then the out-proj weights) ----
        # token-split loads: the first score tiles need only q/k tokens
        # [0, T/2); load the lo halves first so exp_0 isn't gated on all of x
        HX = EP * 2 * T2X
        nc.sync.dma_start(wq_big[:], wq_d[:])
        nc.sync.dma_start(xq_lo[:], xqT_d[:, 0:HX])
        nc.sync.dma_start(bq_sb[:], bq_d[:])
        nc.sync.dma_start(bk_sb[:], bk_d[:])
        nc.sync.dma_start(wk_big[:], wk_d[:])
        nc.sync.dma_start(xk_lo[:], xkT_d[:, 0:HX])
        nc.sync.dma_start(bv_sb[:], bv_d[:])
        nc.sync.dma_start(wv_big[:], wv_d[:])
        QW = EC * TQ
        nc.sync.dma_start(xv_q[0][:], xvT_d[:, 0:QW])
        nc.sync.dma_start(xk_hi[:], xkT_d[:, HX:])
        for k in range(1, 4):
            nc.sync.dma_start(xv_q[k][:], xvT_d[:, k * QW : (k + 1) * QW])
        nc.sync.dma_start(xq_hi[:], xqT_d[:, HX:])
        for j in range(FT):
            nc.sync.dma_start(wo_sb[j][:], wo_d[j * 128 : (j + 1) * 128, :])
        nc.gpsimd.memset(ones_row[:], 1.0)
        # zero halves of q^T/k^T (DoubleRow zero k-tile; never rewritten) on
        # the otherwise-idle Pool engine
        for j in range(FT):
            nc.gpsimd.memset(qT_sb[j][:, T : 2 * T], 0.0)
            nc.gpsimd.memset(kT_sb[j][:, T : 2 * T], 0.0)
        # ones columns of augmented V (written once)
        for i in range(TT):
            vview = v_sb[i][:].rearrange("p (h x) -> p h x", x=dh + 64)
            nc.gpsimd.memset(vview[:, :, dh:], 1.0)

        def dr(ap2w):
            """[p, (2,W)] fp8-folded view of a [128, 2W] tile slice."""
            return ap2w.rearrange("p (t w) -> p t w", t=2)

        def dr2(aphw):
            """[p, (2, T/2)] fp8-folded view of a token-half slice."""
            return aphw.rearrange("p (t w) -> p t w", t=2)

        # ---- compute: projections + attention + out-projection ----
        # PSUM budget (8 banks): proj 2 (bufs=2 x 1 bank) + ST 4 (bufs=2 x 2)
        # + ctx 2 (bufs=1 x 2). Everything coexists, so Tile can overlap the
        # phases; PE instruction order is software-pipelined by hand.
        with (
            tc.tile_pool(name="stpsum", bufs=2, space="PSUM") as stpool,
            tc.tile_pool(name="dvpsum", bufs=2, space="PSUM") as dvpool,
            tc.tile_pool(name="ctpsum", bufs=1, space="PSUM") as ctpool,
            tc.tile_pool(name="ptpool", bufs=6) as ptpool,
            tc.tile_pool(name="normpool", bufs=3) as npool,
            tc.tile_pool(name="outsb", bufs=4) as osbpool,
        ):

            def proj_qk(j, groups):
                qk = (
                    (wq_sb, bq_sb, xqT_sb, qT_sb),
                    (wk_sb, bk_sb, xkT_sb, kT_sb),
                )
                for t, n in groups:
                    w_sb, b_sb, x_sb, dst = qk[t]
                    ps0 = dvpool.tile([128, SCH2], F32, tag="dv", name="dv")
                    ps = ps0[:, 0:SCH]
                    nh2 = NSCH // 2
                    xsl = n * SCH - (n // nh2) * T2X
                    for e in range(EP):
                        nc.tensor.matmul(
                            ps,
                            dr(w_sb[e])[:, :, j * 128 : (j + 1) * 128],
                            dr2(x_sb[e][n // nh2])[:, :, xsl : xsl + SCH],
                            start=(e == 0),
                            stop=(e == EP - 1),
                            perf_mode=DR,
                        )
                    # bias is per-PSUM-partition here (features on partitions):
                    # add it in the copy and fold out the x16 weight prescale
                    nc.vector.tensor_scalar(
                        dst[j][:, n * SCH : (n + 1) * SCH],
                        ps,
                        b_sb[:, j : j + 1],
                        inv_w,
                        mybir.AluOpType.add,
                        mybir.AluOpType.mult,
                    )

            def proj_v(tiles=None):
                for i in tiles if tiles is not None else range(TT):
                    ps = dvpool.tile([128, SCH2], F32, tag="dv", name="dv")
                    q, s0v = i // 4, (i % 4) * 128
                    for e in range(EC):
                        nc.tensor.matmul(
                            ps[:, 0:FEAT],
                            xv_q[q][:, e * TQ + s0v : e * TQ + s0v + 128],
                            wv_sb[e],
                            start=(e == 0),
                            stop=False,
                        )
                    nc.tensor.matmul(
                        ps[:, 0:FEAT], ones_row[:, 0:128], bv_sb[:], start=False, stop=True
                    )
                    dst = v_sb[i][:].rearrange("p (h x) -> p h x", x=dh + 64)[:, :, 0:dh]
                    srcv = ps[:, 0:FEAT].rearrange("p (h d) -> p h d", d=dh)
                    nc.vector.tensor_copy(dst, srcv)

            first_head = [True]
            exp_counter = [0, 0]  # [tiles seen, tiles sent to DVE]

            def st_tile(i, kT_h, qT_h, s0):
                # route a DVE_EXP_FRAC slice of score tiles to a dedicated
                # 1-bank psum pool + VectorE exp, so the ScalarE stream's
                # buffer rotation never blocks on them. Head (0,0) is
                # PE-bound on the JIT V projections so it stays on ScalarE.
                if not first_head[0]:
                    exp_counter[0] += 1
                on_dve = int(exp_counter[0] * DVE_EXP_FRAC) > exp_counter[1]
                if on_dve:
                    exp_counter[1] += 1
                    halves = []
                    for n in range(NSCH2):
                        h = dvpool.tile([128, SCH2], F32, tag="dv", name="dv")
                        nc.tensor.matmul(
                            h[:],
                            dr(kT_h)[:, :, i * 128 : (i + 1) * 128],
                            dr(qT_h)[:, :, s0 + n * SCH2 : s0 + (n + 1) * SCH2],
                            start=True,
                            stop=True,
                            perf_mode=DR,
                        )
                        halves.append(h)
                    return ("dve", halves)
                st = stpool.tile([128, T2], F32, tag="st", name="st")
                for n in range(NSCH2):
                    nc.tensor.matmul(
                        st[:, n * SCH2 : (n + 1) * SCH2],
                        dr(kT_h)[:, :, i * 128 : (i + 1) * 128],
                        dr(qT_h)[:, :, s0 + n * SCH2 : s0 + (n + 1) * SCH2],
                        start=True,
                        stop=True,
                        perf_mode=DR,
                    )
                return ("act", st)

            pending_sts = []
            work_q = []

            def exp_tile(kind_st):
                kind, st = kind_st
                if kind == "dve":
                    pt = ptpool.tile([128, T2], I16, tag="pt", name="pt")
                    for n, h in enumerate(st):
                        nc.vector.tensor_scalar(
                            pt[:, n * SCH2 : (n + 1) * SCH2],
                            h[:],
                            SCH_A * scale,
                            SCH_B,
                            mybir.AluOpType.mult,
                            mybir.AluOpType.add,
                        )
                    return pt[:].bitcast(BF16)
                pt = ptpool.tile([128, T2], BF16, tag="pt", name="pt")
                nc.scalar.activation(
                    pt[:], st[:], mybir.ActivationFunctionType.Exp, scale=scale
                )
                return pt[:]

            def head_args(h, sh):
                ft, half = h // 2, (h % 2) * 64
                return (
                    kT_sb[ft][half : half + 64, :],
                    qT_sb[ft][half : half + 64, :],
                    sh * T2,
                )

            def head(h, sh, filler=None, nxt=None):
                # keeps 2 score tiles in flight and pre-issues the NEXT
                # head's first 2 before this head's last context matmul, so
                # the exp stream never stalls at head boundaries
                ft, half = h // 2, (h % 2) * 64
                kT_h, qT_h, s0 = head_args(h, sh)
                ct = ctpool.tile([128, T2], F32, tag="ct", name="ct")
                sts = pending_sts[:]
                del pending_sts[:]
                while len(sts) < min(2, TT):
                    sts.append(st_tile(len(sts), kT_h, qT_h, s0))
                look = len(sts)
                nissued = 0
                for i in range(TT):
                    st = sts.pop(0)
                    pt = exp_tile(st)
                    if i + look < TT:
                        sts.append(st_tile(i + look, kT_h, qT_h, s0))
                    elif nxt is not None and nissued < min(4, TT):
                        pending_sts.append(st_tile(nissued, *head_args(*nxt)))
                        nissued += 1
                    if filler is not None:
                        filler(i)
                    elif work_q and i < TT - 2:
                        work_q.pop(0)()
                    for n in range(NSCH2):
                        nc.tensor.matmul(
                            ct[:, n * SCH2 : (n + 1) * SCH2],
                            v_sb[i][:, h * (dh + 64) : (h + 1) * (dh + 64)],
                            pt[:, n * SCH2 : (n + 1) * SCH2],
                            start=(i == 0),
                            stop=(i == TT - 1),
                        )

                # normalize: cn[f, s] = ct[f, s] * (1 / ct[64.., s]); split
                # so downstream out-proj tiles unblock per 512-query chunk
                recip = npool.tile([64, T2], F32, tag="recip", name="recip")
                for n in range(NSCH2):
                    c = slice(n * SCH2, (n + 1) * SCH2)
                    nc.vector.reciprocal(recip[:, c], ct[64:128, c])
                    nc.vector.tensor_tensor(
                        cn_sb[ft][half : half + 64, s0 + n * SCH2 : s0 + (n + 1) * SCH2],
                        ct[0:64, c],
                        recip[:, c],
                        op=mybir.AluOpType.mult,
                    )

            osb_state = {}
            tail_mode = [False]
            tail_ctr = [0]

            def outproj_chunk(i, ci):
                from_st = False
                if tail_mode[0]:
                    tail_ctr[0] += 1
                    from_st = tail_ctr[0] % 2 == 0
                oc, ow = OCHUNKS[ci]
                if ci == 0:
                    osb_state[i] = osbpool.tile([128, OUT], BF16, tag="osb", name="osb")
                osb = osb_state[i]
                if from_st:
                    ps0 = stpool.tile([128, T2], F32, tag="st", name="st")
                else:
                    ps0 = dvpool.tile([128, SCH2], F32, tag="dv", name="dv")
                ps = ps0[:, 0:ow]
                for f in range(FT):
                    nc.tensor.matmul(
                        ps,
                        cn_sb[f][:, i * 128 : (i + 1) * 128],
                        wo_sb[f][:, oc : oc + ow],
                        start=(f == 0),
                        stop=(f == FT - 1),
                    )
                if tail_mode[0] and tail_ctr[0] % 2 == 0:
                    nc.scalar.copy(osb[:, oc : oc + ow], ps)
                else:
                    nc.vector.tensor_copy(osb[:, oc : oc + ow], ps)
                nc.sync.dma_start(out_d[i * 128 : (i + 1) * 128, oc : oc + ow], osb[:, oc : oc + ow])
                if ci == len(OCHUNKS) - 1:
                    del osb_state[i]

            # ---- startup: emit exactly the projection groups the first two
            # score tiles need, pre-issue those tiles, then the rest ----
            proj_qk(0, [(0, 0), (0, 1), (1, 0)])
            for z in range(min(2, TT)):
                pending_sts.append(st_tile(z, *head_args(0, 0)))
            proj_qk(0, [(1, 1)])
            # two more score tiles BEFORE any V work, so the exp stream is
            # never gated by the V matmuls waiting on the late xv DMA
            for z in range(2, 4):
                pending_sts.append(st_tile(z, *head_args(0, 0)))
            # V tile i is first needed at head 0's CT step i: emit tile 0/1
            # up front and drip the rest into head 0's pipeline
            proj_v(range(2))
            # hi-token-half q projections aren't needed until sh=1: queue them
            work_q.extend(
                (lambda g=(0, n): proj_qk(0, [g])) for n in (2, 3)
            )

            def v_filler(i):
                if i + 2 < TT:
                    proj_v([i + 2])
                # hi-half K projections: head 0 runs with lookahead 4, so
                # score tile 8 (the first reader of kT cols [1024,2048)) is
                # EMITTED at window 4 -- these writers must be emitted
                # strictly before it or the dependency is never recorded
                if i == 1:
                    proj_qk(0, [(1, 2)])
                elif i == 2:
                    proj_qk(0, [(1, 3)])
                if i == TT - 1:
                    first_head[0] = False

            half_tiles = T2 // 128 if NSH == 2 else 0
            # sh-outer order: all pairs finish queries [0,T2) first, so that
            # block's out-projection drips through the whole sh=1 phase
            seq = [
                (2 * p + z, sh)
                for sh in range(NSH)
                for p in range(NH // 2)
                for z in (0, 1)
            ]
            for pos, (h, sh) in enumerate(seq):
                p = h // 2
                nxt = seq[pos + 1] if pos + 1 < len(seq) else None
                head(h, sh, v_filler if (h, sh) == (0, 0) else None, nxt=nxt)
                if h % 2 == 0 and sh == 0 and p + 1 < FT:
                    # queue pair p+1's projections a full head before pair
                    # p+1 starts, so its pre-issued score tiles never block
                    work_q.extend(
                        (lambda jj=p + 1, g=(t, n): proj_qk(jj, [g]))
                        for t in range(2)
                        for n in range(NSCH)
                    )
                if h % 2 == 1:
                    # after the LAST pair at this sh: that query block's
                    # out-projection becomes computable; drip it
                    if p == NH // 2 - 1:
                        tiles = range(sh * half_tiles, (sh + 1) * half_tiles)
                        work_q.extend(
                            (lambda ii=i, cc=ci: outproj_chunk(ii, cc))
                            for i in tiles
                            for ci in range(len(OCHUNKS))
                        )
            # tail: whatever the windows didn't absorb, pipelined 4-deep
            # across both free psum pools
            tail_mode[0] = True
            for w in work_q:
                w()
            del work_q[:]

    nc.compile()
    return nc


def _pad_fp8_bias(b):
    """[1, F] f32 -> [1, 2F] fp8 with a zeroed second half (DoubleRow pad)."""
    out = np.zeros((1, 2 * b.shape[1]), np.float32)
    out[:, : b.shape[1]] = b
    return out.astype(NP_FP8)


def _fold_fp8(arr2d, ncols, token_split=False):
    """[768, ncols] f32 -> [128, 3*2*ncols] fp8 contraction-folded; with
    token_split, columns ordered [half, j, t, ncols/2]."""
    a = arr2d.reshape(3, 2, 2, 64, ncols).transpose(0, 1, 3, 2, 4)
    a = a.reshape(3, 128, 2, ncols)
    if token_split:
        h = ncols // 2
        a = a.reshape(3, 128, 2, 2, h).transpose(3, 0, 1, 2, 4)  # [half, j, p, t, h]
        a = a.transpose(2, 0, 1, 3, 4).reshape(128, 3 * 2 * ncols)
    else:
        a = a.transpose(1, 0, 2, 3).reshape(128, 3 * 2 * ncols)
    return np.ascontiguousarray(a).astype(NP_FP8)


def shard_inputs(query, key, value, wq, bq, wk, bk, wv, bv, wo):
    """Build the 8 per-core input maps (host-side cast/fold/slice)."""
    in_maps = []
    xT = {}
    for b in range(B):
        xT[b] = (
            _fold_fp8(np.ascontiguousarray(query[b].T), S, token_split=True),
            _fold_fp8(np.ascontiguousarray(key[b].T), S, token_split=True),
            np.ascontiguousarray(
                value[b].T.reshape(6, 128, 4, S // 4).transpose(1, 2, 0, 3).reshape(128, 6 * S)
            ).astype(NP_BF16),
        )
    gw = {}
    for g in range(2):
        hs = slice(g * G, (g + 1) * G)
        gw[g] = dict(
            wq=_fold_fp8(W_SCALE * wq[hs].transpose(1, 0, 2).reshape(E, G * DH), G * DH),
            wk=_fold_fp8(W_SCALE * wk[hs].transpose(1, 0, 2).reshape(E, G * DH), G * DH),
            wv=np.ascontiguousarray(
                wv[hs].transpose(1, 0, 2).reshape(E, G * DH).reshape(6, 128, G * DH).transpose(1, 0, 2).reshape(128, 6 * G * DH)
            ).astype(NP_BF16),
            wo=np.ascontiguousarray(wo[g * G * DH : (g + 1) * G * DH, :]).astype(NP_BF16),
            bq=np.ascontiguousarray(W_SCALE * bq[hs].reshape(3, 128).T).astype(np.float32),
            bk=np.ascontiguousarray(W_SCALE * bk[hs].reshape(3, 128).T).astype(np.float32),
            bv=np.ascontiguousarray(bv[hs].reshape(1, G * DH)).astype(NP_BF16),
        )
    for c in range(N_CORES):
        b, g = c // 2, c % 2
        m = dict(xqT=xT[b][0], xkT=xT[b][1], xvT=xT[b][2])
        m.update(gw[g])
        in_maps.append(m)
    return in_maps


_CACHED_NC = None


def kernel(query, key, value, wq, bq, wk, bk, wv, bv, wo, bo):
    global _CACHED_NC
    query, key, value = (np.asarray(a, np.float32) for a in (query, key, value))
    wq, bq, wk, bk, wv, bv, wo, bo = (
        np.asarray(a, np.float32) for a in (wq, bq, wk, bk, wv, bv, wo, bo)
    )
    in_maps = shard_inputs(query, key, value, wq, bq, wk, bk, wv, bv, wo)
    if _CACHED_NC is None:
        _CACHED_NC = build_nc()
    res = run_bass_kernel_spmd(_CACHED_NC, in_maps, list(range(N_CORES)))
    out = np.empty((B, S, E), np.float32)
    for b in range(B):
        out[b] = (
            res.results[2 * b]["out"].astype(np.float32)
            + res.results[2 * b + 1]["out"].astype(np.float32)
            + bo[None, :]
        )
    return out


# revision 38
# speedup vs baseline: 1.3093x; 1.0130x over previous
"""Multi-head attention (B=4, S=2048, E=768, H=12, Dh=64) on 8 TRN2 NeuronCores.

Sharding: batch x head-group tensor parallel. Core c handles batch b = c//2 and
head group g = c%2 (6 heads each). Each core computes its heads' Q/K/V
projections, full attention over the 2048-token sequence, and a partial
out-projection over its 384 concat-features. The host sums the two partials per
batch and adds the output bias.

Device layout notes:
 - Q/K projection inputs (x^T, w, biases) are hosted in fp8e4 with the
   contraction dim folded [64, 2, .] so those matmuls run in DoubleRow perf
   mode (2 contraction rows/cycle). Weights are host-scaled x16 to clear
   fp8e4's subnormal range; the 1/16 is folded into the PSUM->SBUF cast.
 - Q^T/K^T are produced feature-major [128, 2T] fp8e4 with a zeroed upper
   half: score matmuls run DoubleRow with k-tile 0 = the real 64-row dh
   contraction and k-tile 1 = zeros, streaming 2 output cols/cycle.
 - The V path stays bf16: V quantization error enters the context linearly
   (unlike Q/K noise, which the softmax damps), and measured error triples
   with V in fp8.
 - V is token-major, each head augmented with 64 ones columns so the PV
   matmul emits the softmax denominator replicated on psum partitions 64-127.
 - Scores are computed transposed (S^T tiles [128 keys, S queries]); softmax
   exp is split across TWO engines: most tiles on ScalarE (table exp straight
   out of PSUM), a DVE_EXP_FRAC slice on VectorE via a Schraudolph bitcast
   approximation (i16 = round(x*128/ln2 + 16249); bitcast to bf16 ~= e^x to
   +-4%, which washes out under the ~2048-token softmax average). VectorE
   tiles use a dedicated 1-bank psum pool (two [128,512] halves) so the
   ScalarE stream's double-buffer rotation never waits on them.
 - PSUM (8 banks): ScalarE S^T 2x2 + VectorE-S^T/projection pool 2x1 +
   ctx 2. The PE stream is software-pipelined by hand; projection and
   out-projection chunks drain one-per-window from a work queue into the
   exp-bound attention windows (sh-outer head order so the first query
   block's out-projection overlaps the second block's attention).
 - Output partials are stored bf16 (summed in f32 on host with the bias);
   stores are per-384-column chunk so the tail drains while computing.
"""

import math
import os
import sys
from contextlib import ExitStack

import numpy as np

for _p in ("/opt/trn_rl_repo", "/root/.axon_site/_ro/trn_rl_repo"):
    if os.path.isdir(_p) and _p not in sys.path:
        sys.path.append(_p)

# NTFF tracing hooks (antenv.axon_hooks) don't exist in this container;
# make sure an ambient BASS_TRACE can't route execution into that path.
os.environ["BASS_NEVER_TRACE"] = "1"

import ml_dtypes  # noqa: E402

import concourse.bass as bass  # noqa: E402
import concourse.tile as tile  # noqa: E402
from concourse import bacc, mybir  # noqa: E402
from concourse.bass_utils import run_bass_kernel_spmd  # noqa: E402

BF16 = mybir.dt.bfloat16
F32 = mybir.dt.float32
FP8 = mybir.dt.float8e4
I16 = mybir.dt.int16
NP_BF16 = ml_dtypes.bfloat16
NP_FP8 = ml_dtypes.float8_e4m3

B, S, E, H, DH = 4, 2048, 768, 12, 64
N_CORES = 8
G = H // 2  # heads per core (6)

W_SCALE = 16.0  # host premultiplier on wq/wk/wv/bq/bk/bv (fp8 subnormal dodge)

# Schraudolph exp-approx constants (bf16 bitcast): i16 = st*SCH_A1 + SCH_B
SCH_A = 128.0 / math.log(2.0)
SCH_B = 16256.0 - 7.4 + 0.5
# fraction of exp tiles routed to VectorE instead of ScalarE
DVE_EXP_FRAC = float(os.environ.get("DVE_EXP_FRAC", "0.25"))

DR = mybir.MatmulPerfMode.DoubleRow


def build_nc(T=S, EMB=E, NH=G, dh=DH, OUT=E, trace_label=""):
    """Emit the per-core Bass/Tile program. All cores run this same program."""
    assert T % 128 == 0 and EMB % 128 == 0 and dh == 64 and NH % 2 == 0
    FEAT = NH * dh
    assert FEAT % 128 == 0
    EC = EMB // 128  # 128-row contraction chunks for projections
    EP = EC // 2  # fp8-folded [64,2,...] chunk-pairs per 256 emb rows
    TT = T // 128  # token tiles
    FT = FEAT // 128  # feature tiles (head pairs)
    SCH = min(512, T)  # matmul moving free-dim chunk
    NSCH = T // SCH
    T2 = max(128, T // 2)  # attention query-half width (2 PSUM banks)
    NSH = T // T2  # query halves per head
    SCH2 = min(512, T2)
    NSCH2 = T2 // SCH2
    _ock = OUT // 2 if 128 < OUT <= 1024 and OUT % 2 == 0 else 512
    OCHUNKS = [(o, min(_ock, OUT - o)) for o in range(0, OUT, _ock)]
    scale = 1.0 / math.sqrt(dh)
    inv_w = 1.0 / W_SCALE

    nc = bacc.Bacc("TRN2", target_bir_lowering=False, debug=False, num_devices=N_CORES)

    # ---- DRAM I/O ----
    # x^T and projection weights fp8, contraction-folded: tile j holds emb rows
    # [256j, 256j+256) as [c*64+p, t*T + s] with e = 256j + 128c + 64t + p.
    xqT_d = nc.dram_tensor("xqT", [128, EP * 2 * T], FP8, kind="ExternalInput").ap()
    xkT_d = nc.dram_tensor("xkT", [128, EP * 2 * T], FP8, kind="ExternalInput").ap()
    xvT_d = nc.dram_tensor("xvT", [128, EC * T], BF16, kind="ExternalInput").ap()
    wq_d = nc.dram_tensor("wq", [128, EP * 2 * FEAT], FP8, kind="ExternalInput").ap()
    wk_d = nc.dram_tensor("wk", [128, EP * 2 * FEAT], FP8, kind="ExternalInput").ap()
    wv_d = nc.dram_tensor("wv", [128, EC * FEAT], BF16, kind="ExternalInput").ap()
    wo_d = nc.dram_tensor("wo", [FEAT, OUT], BF16, kind="ExternalInput").ap()
    bq_d = nc.dram_tensor("bq", [128, FEAT // 128], F32, kind="ExternalInput").ap()
    bk_d = nc.dram_tensor("bk", [128, FEAT // 128], F32, kind="ExternalInput").ap()
    bv_d = nc.dram_tensor("bv", [1, FEAT], BF16, kind="ExternalInput").ap()
    out_d = nc.dram_tensor("out", [T, OUT], BF16, kind="ExternalOutput").ap()

    with tile.TileContext(nc) as tc, ExitStack() as ctx:
        persist = ctx.enter_context(tc.tile_pool(name="persist", bufs=1))

        # ---- persistent SBUF tensors ----
        wq_big = persist.tile([128, EP * 2 * FEAT], FP8, tag="wq", name="wq")
        wk_big = persist.tile([128, EP * 2 * FEAT], FP8, tag="wk", name="wk")
        wv_big = persist.tile([128, EC * FEAT], BF16, tag="wv", name="wv")
        wq_sb = [wq_big[:, j * 2 * FEAT : (j + 1) * 2 * FEAT] for j in range(EP)]
        wk_sb = [wk_big[:, j * 2 * FEAT : (j + 1) * 2 * FEAT] for j in range(EP)]
        wv_sb = [wv_big[:, j * FEAT : (j + 1) * FEAT] for j in range(EC)]
        wo_sb = [persist.tile([128, OUT], BF16, tag=f"wo{j}", name=f"wo{j}") for j in range(FT)]
        bq_sb = persist.tile([128, FEAT // 128], F32, tag="bq", name="bq")
        bk_sb = persist.tile([128, FEAT // 128], F32, tag="bk", name="bk")
        bv_sb = persist.tile([1, FEAT], BF16, tag="bv", name="bv")
        ones_row = persist.tile([1, T], BF16, tag="ones_row", name="ones_row")
        T2X = T // 2
        xq_lo = persist.tile([128, EP * 2 * T2X], FP8, tag="xqlo", name="xqlo")
        xq_hi = persist.tile([128, EP * 2 * T2X], FP8, tag="xqhi", name="xqhi")
        xk_lo = persist.tile([128, EP * 2 * T2X], FP8, tag="xklo", name="xklo")
        xk_hi = persist.tile([128, EP * 2 * T2X], FP8, tag="xkhi", name="xkhi")
        TQ = T // 4
        xv_q = [
            persist.tile([128, EC * TQ], BF16, tag=f"xvq{k}", name=f"xvq{k}")
            for k in range(4)
        ]
        # per-(chunk-pair, token-half) views [128, (t, T/2)]
        xqT_sb = [
            (xq_lo[:, j * 2 * T2X : (j + 1) * 2 * T2X], xq_hi[:, j * 2 * T2X : (j + 1) * 2 * T2X])
            for j in range(EP)
        ]
        xkT_sb = [
            (xk_lo[:, j * 2 * T2X : (j + 1) * 2 * T2X], xk_hi[:, j * 2 * T2X : (j + 1) * 2 * T2X])
            for j in range(EP)
        ]

        # q^T/k^T fp8, upper T columns zero (DoubleRow zero k-tile)
        qT_sb = [persist.tile([128, 2 * T], FP8, tag=f"qT{j}", name=f"qT{j}") for j in range(FT)]
        kT_sb = [persist.tile([128, 2 * T], FP8, tag=f"kT{j}", name=f"kT{j}") for j in range(FT)]
        # V token-major, each head augmented with 64 ones columns so the PV
        # matmul emits the softmax denominator replicated on partitions 64-127
        v_sb = [persist.tile([128, NH * (dh + 64)], BF16, tag=f"v{i}", name=f"v{i}") for i in range(TT)]
        cn_sb = [persist.tile([128, T], BF16, tag=f"cn{j}", name=f"cn{j}") for j in range(FT)]

        # ---- weight/bias/x loads (Q path first: it gates head 0; then K,
        # then V which head 0's PV needs, then the out-proj weights) ----
        # token-split loads: the first score tiles need only q/k tokens
        # [0, T/2); load the lo halves first so exp_0 isn't gated on all of x
        HX = EP * 2 * T2X
        nc.sync.dma_start(wq_big[:], wq_d[:])
        nc.sync.dma_start(xq_lo[:], xqT_d[:, 0:HX])
        nc.sync.dma_start(bq_sb[:], bq_d[:])
        nc.sync.dma_start(bk_sb[:], bk_d[:])
        nc.sync.dma_start(wk_big[:], wk_d[:])
        nc.sync.dma_start(xk_lo[:], xkT_d[:, 0:HX])
        nc.sync.dma_start(bv_sb[:], bv_d[:])
        nc.sync.dma_start(wv_big[:], wv_d[:])
        QW = EC * TQ
        nc.sync.dma_start(xv_q[0][:], xvT_d[:, 0:QW])
        nc.sync.dma_start(xk_hi[:], xkT_d[:, HX:])
        for k in range(1, 4):
            nc.sync.dma_start(xv_q[k][:], xvT_d[:, k * QW : (k + 1) * QW])
        nc.sync.dma_start(xq_hi[:], xqT_d[:, HX:])
        for j in range(FT):
            nc.sync.dma_start(wo_sb[j][:], wo_d[j * 128 : (j + 1) * 128, :])
        nc.gpsimd.memset(ones_row[:], 1.0)
        # zero halves of q^T/k^T (DoubleRow zero k-tile; never rewritten) on
        # the otherwise-idle Pool engine
        for j in range(FT):
            nc.gpsimd.memset(qT_sb[j][:, T : 2 * T], 0.0)
            nc.gpsimd.memset(kT_sb[j][:, T : 2 * T], 0.0)
        # ones columns of augmented V (written once)
        for i in range(TT):
            vview = v_sb[i][:].rearrange("p (h x) -> p h x", x=dh + 64)
            nc.gpsimd.memset(vview[:, :, dh:], 1.0)

        def dr(ap2w):
            """[p, (2,W)] fp8-folded view of a [128, 2W] tile slice."""
            return ap2w.rearrange("p (t w) -> p t w", t=2)

        def dr2(aphw):
            """[p, (2, T/2)] fp8-folded view of a token-half slice."""
            return aphw.rearrange("p (t w) -> p t w", t=2)

        # ---- compute: projections + attention + out-projection ----
        # PSUM budget (8 banks): proj 2 (bufs=2 x 1 bank) + ST 4 (bufs=2 x 2)
        # + ctx 2 (bufs=1 x 2). Everything coexists, so Tile can overlap the
        # phases; PE instruction order is software-pipelined by hand.
        with (
            tc.tile_pool(name="stpsum", bufs=2, space="PSUM") as stpool,
            tc.tile_pool(name="dvpsum", bufs=2, space="PSUM") as dvpool,
            tc.tile_pool(name="ctpsum", bufs=1, space="PSUM") as ctpool,
            tc.tile_pool(name="ptpool", bufs=6) as ptpool,
            tc.tile_pool(name="normpool", bufs=3) as npool,
            tc.tile_pool(name="outsb", bufs=4) as osbpool,
        ):

            def proj_qk(j, groups):
                qk = (
                    (wq_sb, bq_sb, xqT_sb, qT_sb),
                    (wk_sb, bk_sb, xkT_sb, kT_sb),
                )
                for t, n in groups:
                    w_sb, b_sb, x_sb, dst = qk[t]
                    ps0 = dvpool.tile([128, SCH2], F32, tag="dv", name="dv")
                    ps = ps0[:, 0:SCH]
                    nh2 = NSCH // 2
                    xsl = n * SCH - (n // nh2) * T2X
                    for e in range(EP):
                        nc.tensor.matmul(
                            ps,
                            dr(w_sb[e])[:, :, j * 128 : (j + 1) * 128],
                            dr2(x_sb[e][n // nh2])[:, :, xsl : xsl + SCH],
                            start=(e == 0),
                            stop=(e == EP - 1),
                            perf_mode=DR,
                        )
                    # bias is per-PSUM-partition here (features on partitions):
                    # add it in the copy and fold out the x16 weight prescale
                    nc.vector.tensor_scalar(
                        dst[j][:, n * SCH : (n + 1) * SCH],
                        ps,
                        b_sb[:, j : j + 1],
                        inv_w,
                        mybir.AluOpType.add,
                        mybir.AluOpType.mult,
                    )

            def proj_v(tiles=None):
                for i in tiles if tiles is not None else range(TT):
                    ps = dvpool.tile([128, SCH2], F32, tag="dv", name="dv")
                    q, s0v = i // 4, (i % 4) * 128
                    for e in range(EC):
                        nc.tensor.matmul(
                            ps[:, 0:FEAT],
                            xv_q[q][:, e * TQ + s0v : e * TQ + s0v + 128],
                            wv_sb[e],
                            start=(e == 0),
                            stop=False,
                        )
                    nc.tensor.matmul(
                        ps[:, 0:FEAT], ones_row[:, 0:128], bv_sb[:], start=False, stop=True
                    )
                    dst = v_sb[i][:].rearrange("p (h x) -> p h x", x=dh + 64)[:, :, 0:dh]
                    srcv = ps[:, 0:FEAT].rearrange("p (h d) -> p h d", d=dh)
                    nc.vector.tensor_copy(dst, srcv)

            first_head = [True]
            exp_counter = [0, 0]  # [tiles seen, tiles sent to DVE]

            def st_tile(i, kT_h, qT_h, s0):
                # route a DVE_EXP_FRAC slice of score tiles to a dedicated
                # 1-bank psum pool + VectorE exp, so the ScalarE stream's
                # buffer rotation never blocks on them. Head (0,0) is
                # PE-bound on the JIT V projections so it stays on ScalarE.
                if not first_head[0]:
                    exp_counter[0] += 1
                # force tile 1 onto VectorE: of the 4 score tiles pre-issued
                # across each head boundary, routing one to the separate
                # VectorE psum pool makes 3 physically buffered (2 ScalarE +
                # 1 VectorE pair), enough runway to coast over the previous
                # head's normalize stall
                on_dve = (not first_head[0]) and (
                    i == 1 or int(exp_counter[0] * DVE_EXP_FRAC) > exp_counter[1]
                )
                if on_dve:
                    exp_counter[1] += 1
                    halves = []
                    for n in range(NSCH2):
                        h = dvpool.tile([128, SCH2], F32, tag="dv", name="dv")
                        nc.tensor.matmul(
                            h[:],
                            dr(kT_h)[:, :, i * 128 : (i + 1) * 128],
                            dr(qT_h)[:, :, s0 + n * SCH2 : s0 + (n + 1) * SCH2],
                            start=True,
                            stop=True,
                            perf_mode=DR,
                        )
                        halves.append(h)
                    return ("dve", halves)
                st = stpool.tile([128, T2], F32, tag="st", name="st")
                for n in range(NSCH2):
                    nc.tensor.matmul(
                        st[:, n * SCH2 : (n + 1) * SCH2],
                        dr(kT_h)[:, :, i * 128 : (i + 1) * 128],
                        dr(qT_h)[:, :, s0 + n * SCH2 : s0 + (n + 1) * SCH2],
                        start=True,
                        stop=True,
                        perf_mode=DR,
                    )
                return ("act", st)

            pending_sts = []
            work_q = []

            def exp_tile(kind_st):
                kind, st = kind_st
                if kind == "dve":
                    pt = ptpool.tile([128, T2], I16, tag="pt", name="pt")
                    for n, h in enumerate(st):
                        nc.vector.tensor_scalar(
                            pt[:, n * SCH2 : (n + 1) * SCH2],
                            h[:],
                            SCH_A * scale,
                            SCH_B,
                            mybir.AluOpType.mult,
                            mybir.AluOpType.add,
                        )
                    return pt[:].bitcast(BF16)
                pt = ptpool.tile([128, T2], BF16, tag="pt", name="pt")
                nc.scalar.activation(
                    pt[:], st[:], mybir.ActivationFunctionType.Exp, scale=scale
                )
                return pt[:]

            def head_args(h, sh):
                ft, half = h // 2, (h % 2) * 64
                return (
                    kT_sb[ft][half : half + 64, :],
                    qT_sb[ft][half : half + 64, :],
                    sh * T2,
                )

            def head(h, sh, filler=None, nxt=None):
                # keeps 2 score tiles in flight and pre-issues the NEXT
                # head's first 2 before this head's last context matmul, so
                # the exp stream never stalls at head boundaries
                ft, half = h // 2, (h % 2) * 64
                kT_h, qT_h, s0 = head_args(h, sh)
                ct = ctpool.tile([128, T2], F32, tag="ct", name="ct")
                sts = pending_sts[:]
                del pending_sts[:]
                while len(sts) < min(2, TT):
                    sts.append(st_tile(len(sts), kT_h, qT_h, s0))
                look = len(sts)
                nissued = 0
                for i in range(TT):
                    st = sts.pop(0)
                    pt = exp_tile(st)
                    if i + look < TT:
                        sts.append(st_tile(i + look, kT_h, qT_h, s0))
                    elif nxt is not None and nissued < min(4, TT):
                        pending_sts.append(st_tile(nissued, *head_args(*nxt)))
                        nissued += 1
                    if filler is not None:
                        filler(i)
                    elif work_q and i < TT - 2:
                        work_q.pop(0)()
                    for n in range(NSCH2):
                        nc.tensor.matmul(
                            ct[:, n * SCH2 : (n + 1) * SCH2],
                            v_sb[i][:, h * (dh + 64) : (h + 1) * (dh + 64)],
                            pt[:, n * SCH2 : (n + 1) * SCH2],
                            start=(i == 0),
                            stop=(i == TT - 1),
                        )

                # normalize: cn[f, s] = ct[f, s] * (1 / ct[64.., s]); split
                # so downstream out-proj tiles unblock per 512-query chunk
                recip = npool.tile([64, T2], F32, tag="recip", name="recip")
                for n in range(NSCH2):
                    c = slice(n * SCH2, (n + 1) * SCH2)
                    nc.vector.reciprocal(recip[:, c], ct[64:128, c])
                    nc.vector.tensor_tensor(
                        cn_sb[ft][half : half + 64, s0 + n * SCH2 : s0 + (n + 1) * SCH2],
                        ct[0:64, c],
                        recip[:, c],
                        op=mybir.AluOpType.mult,
                    )

            osb_state = {}
            tail_mode = [False]
            tail_ctr = [0]

            def outproj_chunk(i, ci):
                from_st = False
                if tail_mode[0]:
                    tail_ctr[0] += 1
                    from_st = tail_ctr[0] % 2 == 0
                oc, ow = OCHUNKS[ci]
                if ci == 0:
                    osb_state[i] = osbpool.tile([128, OUT], BF16, tag="osb", name="osb")
                osb = osb_state[i]
                if from_st:
                    ps0 = stpool.tile([128, T2], F32, tag="st", name="st")
                else:
                    ps0 = dvpool.tile([128, SCH2], F32, tag="dv", name="dv")
                ps = ps0[:, 0:ow]
                for f in range(FT):
                    nc.tensor.matmul(
                        ps,
                        cn_sb[f][:, i * 128 : (i + 1) * 128],
                        wo_sb[f][:, oc : oc + ow],
                        start=(f == 0),
                        stop=(f == FT - 1),
                    )
                if tail_mode[0] and tail_ctr[0] % 2 == 0:
                    nc.scalar.copy(osb[:, oc : oc + ow], ps)
                else:
                    nc.vector.tensor_copy(osb[:, oc : oc + ow], ps)
                nc.sync.dma_start(out_d[i * 128 : (i + 1) * 128, oc : oc + ow], osb[:, oc : oc + ow])
                if ci == len(OCHUNKS) - 1:
                    del osb_state[i]

            # ---- startup: emit exactly the projection groups the first two
            # score tiles need, pre-issue those tiles, then the rest ----
            proj_qk(0, [(0, 0), (0, 1), (1, 0)])
            for z in range(min(2, TT)):
                pending_sts.append(st_tile(z, *head_args(0, 0)))
            proj_qk(0, [(1, 1)])
            # two more score tiles BEFORE any V work, so the exp stream is
            # never gated by the V matmuls waiting on the late xv DMA
            for z in range(2, 4):
                pending_sts.append(st_tile(z, *head_args(0, 0)))
            # V tile i is first needed at head 0's CT step i: emit tile 0/1
            # up front and drip the rest into head 0's pipeline
            proj_v(range(2))
            # hi-token-half q projections aren't needed until sh=1: queue them
            work_q.extend(
                (lambda g=(0, n): proj_qk(0, [g])) for n in (2, 3)
            )

            def v_filler(i):
                if i + 2 < TT:
                    proj_v([i + 2])
                # hi-half K projections: head 0 runs with lookahead 4, so
                # score tile 8 (the first reader of kT cols [1024,2048)) is
                # EMITTED at window 4 -- these writers must be emitted
                # strictly before it or the dependency is never recorded
                if i == 1:
                    proj_qk(0, [(1, 2)])
                elif i == 2:
                    proj_qk(0, [(1, 3)])
                if i == TT - 1:
                    first_head[0] = False

            half_tiles = T2 // 128 if NSH == 2 else 0
            # sh-outer order: all pairs finish queries [0,T2) first, so that
            # block's out-projection drips through the whole sh=1 phase
            seq = [
                (2 * p + z, sh)
                for sh in range(NSH)
                for p in range(NH // 2)
                for z in (0, 1)
            ]
            for pos, (h, sh) in enumerate(seq):
                p = h // 2
                nxt = seq[pos + 1] if pos + 1 < len(seq) else None
                head(h, sh, v_filler if (h, sh) == (0, 0) else None, nxt=nxt)
                if h % 2 == 0 and sh == 0 and p + 1 < FT:
                    # queue pair p+1's projections a full head before pair
                    # p+1 starts, so its pre-issued score tiles never block
                    work_q.extend(
                        (lambda jj=p + 1, g=(t, n): proj_qk(jj, [g]))
                        for t in range(2)
                        for n in range(NSCH)
                    )
                if h % 2 == 1:
                    # after the LAST pair at this sh: that query block's
                    # out-projection becomes computable; drip it
                    if p == NH // 2 - 1:
                        tiles = range(sh * half_tiles, (sh + 1) * half_tiles)
                        work_q.extend(
                            (lambda ii=i, cc=ci: outproj_chunk(ii, cc))
                            for i in tiles
                            for ci in range(len(OCHUNKS))
                        )
            # tail: whatever the windows didn't absorb, pipelined 4-deep
            # across both free psum pools
            tail_mode[0] = True
            for w in work_q:
                w()
            del work_q[:]

    nc.compile()
    return nc


def _pad_fp8_bias(b):
    """[1, F] f32 -> [1, 2F] fp8 with a zeroed second half (DoubleRow pad)."""
    out = np.zeros((1, 2 * b.shape[1]), np.float32)
    out[:, : b.shape[1]] = b
    return out.astype(NP_FP8)


def _fold_fp8(arr2d, ncols, token_split=False):
    """[768, ncols] f32 -> [128, 3*2*ncols] fp8 contraction-folded; with
    token_split, columns ordered [half, j, t, ncols/2]."""
    a = arr2d.reshape(3, 2, 2, 64, ncols).transpose(0, 1, 3, 2, 4)
    a = a.reshape(3, 128, 2, ncols)
    if token_split:
        h = ncols // 2
        a = a.reshape(3, 128, 2, 2, h).transpose(3, 0, 1, 2, 4)  # [half, j, p, t, h]
        a = a.transpose(2, 0, 1, 3, 4).reshape(128, 3 * 2 * ncols)
    else:
        a = a.transpose(1, 0, 2, 3).reshape(128, 3 * 2 * ncols)
    return np.ascontiguousarray(a).astype(NP_FP8)


def shard_inputs(query, key, value, wq, bq, wk, bk, wv, bv, wo):
    """Build the 8 per-core input maps (host-side cast/fold/slice)."""
    in_maps = []
    xT = {}
    for b in range(B):
        xT[b] = (
            _fold_fp8(np.ascontiguousarray(query[b].T), S, token_split=True),
            _fold_fp8(np.ascontiguousarray(key[b].T), S, token_split=True),
            np.ascontiguousarray(
                value[b].T.reshape(6, 128, 4, S // 4).transpose(1, 2, 0, 3).reshape(128, 6 * S)
            ).astype(NP_BF16),
        )
    gw = {}
    for g in range(2):
        hs = slice(g * G, (g + 1) * G)
        gw[g] = dict(
            wq=_fold_fp8(W_SCALE * wq[hs].transpose(1, 0, 2).reshape(E, G * DH), G * DH),
            wk=_fold_fp8(W_SCALE * wk[hs].transpose(1, 0, 2).reshape(E, G * DH), G * DH),
            wv=np.ascontiguousarray(
                wv[hs].transpose(1, 0, 2).reshape(E, G * DH).reshape(6, 128, G * DH).transpose(1, 0, 2).reshape(128, 6 * G * DH)
            ).astype(NP_BF16),
            wo=np.ascontiguousarray(wo[g * G * DH : (g + 1) * G * DH, :]).astype(NP_BF16),
            bq=np.ascontiguousarray(W_SCALE * bq[hs].reshape(3, 128).T).astype(np.float32),
            bk=np.ascontiguousarray(W_SCALE * bk[hs].reshape(3, 128).T).astype(np.float32),
            bv=np.ascontiguousarray(bv[hs].reshape(1, G * DH)).astype(NP_BF16),
        )
    for c in range(N_CORES):
        b, g = c // 2, c % 2
        m = dict(xqT=xT[b][0], xkT=xT[b][1], xvT=xT[b][2])
        m.update(gw[g])
        in_maps.append(m)
    return in_maps


_CACHED_NC = None


def kernel(query, key, value, wq, bq, wk, bk, wv, bv, wo, bo):
    global _CACHED_NC
    query, key, value = (np.asarray(a, np.float32) for a in (query, key, value))
    wq, bq, wk, bk, wv, bv, wo, bo = (
        np.asarray(a, np.float32) for a in (wq, bq, wk, bk, wv, bv, wo, bo)
    )
    in_maps = shard_inputs(query, key, value, wq, bq, wk, bk, wv, bv, wo)
    if _CACHED_NC is None:
        _CACHED_NC = build_nc()
    res = run_bass_kernel_spmd(_CACHED_NC, in_maps, list(range(N_CORES)))
    out = np.empty((B, S, E), np.float32)
    for b in range(B):
        out[b] = (
            res.results[2 * b]["out"].astype(np.float32)
            + res.results[2 * b + 1]["out"].astype(np.float32)
            + bo[None, :]
        )
    return out
